# revision 1
# baseline (speedup 1.0000x reference)
"""PointNet++-lite segmentation on 8 Trainium2 cores (batch-parallel, one
point cloud per core). Self-contained: hardcodes shapes from the problem spec.

Per-core pipeline (all on device):
  embed MLP -> SA1 (KNN top-32 of 16384, gather, 2-layer MLP, max-pool)
  -> SA2 (KNN top-32 of 1024) -> FP2/FP1 (3-NN inverse-distance interp)
  -> global-max head MLP -> (16384, 13) logits.

KNN ranking uses m = 2 q.x - |x|^2 (row-constant |q|^2 dropped), computed by
the PE with a 5-row augmented contraction; selection runs on the vector
engine's max8/max_index/match_replace reading PSUM directly.
"""

from contextlib import ExitStack

import numpy as np

import concourse.bass as bass
import concourse.mybir as mybir
from concourse.bacc import Bacc
from concourse.bass_utils import run_bass_kernel_spmd
from concourse.masks import make_identity
from concourse.tile import TileContext

F32 = mybir.dt.float32
U16 = mybir.dt.uint16
U8 = mybir.dt.uint8
I16 = mybir.dt.int16
U32 = mybir.dt.uint32
I32 = mybir.dt.int32
AF = mybir.ActivationFunctionType
ALU = mybir.AluOpType
AX = mybir.AxisListType

P = 128
N = 16384
S1, K1 = 1024, 32
S2, K2 = 256, 32
NCLS = 13
NEG = -3.0e38

NT = N // P        # 128 point tiles
T1 = S1 // P       # 8 SA1 query tiles
T2 = S2 // P       # 2 SA2 query tiles
TAB0_W = 128       # [feat0(64), xyz(3), zero, pad...] 512B rows
TAB1_W = 192       # [feat1(128), xyz1(3), pad...] 768B rows


def build_nc():
    nc = Bacc()

    x_in = nc.dram_tensor("x", [N, 6], F32, kind="ExternalInput")
    xT_in = nc.dram_tensor("xT", [6, N], F32, kind="ExternalInput")
    i1_in = nc.dram_tensor("i1w", [P, S1 // 16], I16, kind="ExternalInput")
    i2_in = nc.dram_tensor("i2w", [P, S2 // 16], I16, kind="ExternalInput")
    wdecl = [
        ("embw", [6, 64]), ("embb", [64, 1]),
        ("w1", [67, 128]), ("b1r", [1, 128]), ("w2", [128, 128]), ("b2", [128, 1]),
        ("v1", [131, 256]), ("c1r", [1, 256]), ("v2", [256, 256]), ("c2", [128, 2]),
        ("f2w1", [384, 128]), ("f2b1", [128, 1]), ("f2w2", [128, 128]), ("f2b2", [128, 1]),
        ("f1w1", [192, 128]), ("f1b1", [128, 1]), ("f1w2", [128, 128]), ("f1b2", [128, 1]),
        ("hw1", [256, 128]), ("hb1", [128, 1]), ("hw2", [128, 64]), ("hb2", [64, 1]),
        ("hw3", [64, 13]), ("hb3", [13, 1]),
    ]
    din = {nm: nc.dram_tensor(nm, sh, F32, kind="ExternalInput") for nm, sh in wdecl}
    out_d = nc.dram_tensor("out", [N, NCLS], F32, kind="ExternalOutput")

    tab0_d = nc.dram_tensor("tab0", [N, TAB0_W], F32)
    tab1_d = nc.dram_tensor("tab1", [S1, TAB1_W], F32)
    tab2_d = nc.dram_tensor("tab2", [S2, 256], F32)
    tabf_d = nc.dram_tensor("tabf", [S1, 128], F32)
    f0T_d = nc.dram_tensor("f0T", [64, N], F32)
    fuT_d = nc.dram_tensor("fuT", [128, N], F32)
    ib1_ds = [nc.dram_tensor(f"ib1_{t}", [16, 256], I16) for t in range(T1)]
    ib2_ds = [nc.dram_tensor(f"ib2_{t}", [16, 256], I16) for t in range(T2)]
    ibf2_d = nc.dram_tensor("ibf2", [16, 192], I16)
    ibf1_d = nc.dram_tensor("ibf1", [16, NT * 3 * 8], I16)
    ones_d = nc.dram_tensor("ones_d", [1, N], F32)
    zeros_d = nc.dram_tensor("zeros_d", [1, N], F32)

    with TileContext(nc) as tc, ExitStack() as ctx:
        cst = ctx.enter_context(tc.tile_pool(name="cst", bufs=1))
        psA = ctx.enter_context(tc.tile_pool(name="psA", bufs=2, space="PSUM"))
        psT = ctx.enter_context(tc.tile_pool(name="psT", bufs=2, space="PSUM"))

        ident = cst.tile([P, P], F32, tag="ident", name="ident")
        make_identity(nc, ident[:])
        _onerow = cst.tile([P, P], F32, tag="_onerow", name="_onerow")
        nc.vector.memset(_onerow[:], 1.0)
        nc.sync.dma_start(
            out=ones_d[:].rearrange("r (t p) -> r t p", p=P), in_=_onerow[:])
        _zrow = cst.tile([P, P], F32, tag="_zrow", name="_zrow")
        nc.vector.memset(_zrow[:], 0.0)
        nc.sync.dma_start(
            out=zeros_d[:].rearrange("r (t p) -> r t p", p=P), in_=_zrow[:])

        def mmtile():
            return psA.tile([P, 512], F32, tag="mm", name="mm")

        def trans(in_ap):
            """PE transpose: in_(p,f) -> psum slice (f,p)."""
            pt = psT.tile([P, 512], F32, tag="trans", name="trans")
            k = in_ap.shape[0]
            f = in_ap.shape[-1]
            nc.tensor.matmul(out=pt[:f, :k], lhsT=in_ap, rhs=ident[:k, :k],
                             is_transpose=True, start=True, stop=True)
            return pt[:f, :k]

        # ---------------- constants / weights ----------------
        def load(name, src, shape, dtype=F32):
            t = cst.tile(list(shape), dtype, tag=name, name=name)
            nc.sync.dma_start(out=t[:], in_=src)
            return t

        embw = load("embw", din["embw"][:], (6, 64))
        embb = load("embb", din["embb"][:], (64, 1))
        w2sb = load("w2sb", din["w2"][:], (128, 128))
        b2sb = load("b2sb", din["b2"][:], (128, 1))
        c2sb = load("c2sb", din["c2"][:], (128, 2))
        f2b1 = load("f2b1", din["f2b1"][:], (128, 1))
        f2w2 = load("f2w2", din["f2w2"][:], (128, 128))
        f2b2 = load("f2b2", din["f2b2"][:], (128, 1))
        f1b1 = load("f1b1", din["f1b1"][:], (128, 1))
        f1w2 = load("f1w2", din["f1w2"][:], (128, 128))
        f1b2 = load("f1b2", din["f1b2"][:], (128, 1))
        hb1 = load("hb1", din["hb1"][:], (128, 1))
        hw2 = load("hw2", din["hw2"][:], (128, 64))
        hb2 = load("hb2", din["hb2"][:], (64, 1))
        hw3 = load("hw3", din["hw3"][:], (64, 13))
        hb3 = load("hb3", din["hb3"][:], (13, 1))

        w1aug = cst.tile([68, 128], F32, tag="w1aug", name="w1aug")
        nc.sync.dma_start(out=w1aug[0:64, :], in_=din["w1"][3:67, :])
        nc.sync.dma_start(out=w1aug[64:67, :], in_=din["w1"][0:3, :])
        nc.sync.dma_start(out=w1aug[67:68, :], in_=zeros_d[0:1, 0:128])
        w1q = cst.tile([4, 128], F32, tag="w1q", name="w1q")
        nc.sync.dma_start(out=w1q[0:3, :], in_=din["w1"][0:3, :])
        nc.sync.dma_start(out=w1q[3:4, :], in_=din["b1r"][:])

        v1A = [load(f"v1A{h}", din["v1"][3:131, h * 128:(h + 1) * 128], (128, 128))
               for h in range(2)]
        v1rel, v1q = [], []
        for h in range(2):
            sl = slice(h * 128, (h + 1) * 128)
            t = cst.tile([3, 128], F32, tag=f"v1rel{h}", name=f"v1rel{h}")
            nc.sync.dma_start(out=t[:], in_=din["v1"][0:3, sl])
            v1rel.append(t)
            t2 = cst.tile([4, 128], F32, tag=f"v1q{h}", name=f"v1q{h}")
            nc.sync.dma_start(out=t2[0:3, :], in_=din["v1"][0:3, sl])
            nc.sync.dma_start(out=t2[3:4, :], in_=din["c1r"][0:1, sl])
            v1q.append(t2)
        v2sb = [[load(f"v2{k}{h}",
                      din["v2"][k * 128:(k + 1) * 128, h * 128:(h + 1) * 128],
                      (128, 128)) for h in range(2)] for k in range(2)]
        f2w1 = [load(f"f2w1{k}", din["f2w1"][k * 128:(k + 1) * 128, :], (128, 128))
                for k in range(3)]
        f1w1a = load("f1w1a", din["f1w1"][0:64, :], (64, 128))
        f1w1b = load("f1w1b", din["f1w1"][64:192, :], (128, 128))
        hw1a = load("hw1a", din["hw1"][0:128, :], (128, 128))
        hw1b = load("hw1b", din["hw1"][128:256, :], (128, 128))
        i1sb = load("i1sb", i1_in[:], (P, S1 // 16), I16)
        i2sb = load("i2sb", i2_in[:], (P, S2 // 16), I16)

        offs = cst.tile([P, 512], U16, tag="offs", name="offs")
        nc.gpsimd.iota(out=offs[:], pattern=[[256, 64], [0, 8]], base=0,
                       channel_multiplier=0)

        def wrap_idx(pool, src_i16, bounce_d, tag):
            """src (128, M) i16, flat order i = j*128+p -> replicated wrapped
            idx tile (128, 8*M) via DRAM bounce."""
            M = src_i16.shape[-1]
            bw = bounce_d[:, :].rearrange("c (j e) -> c j e", e=8)
            for ph in range(8):
                nc.sync.dma_start(out=bw[:, 0:M, ph],
                                  in_=src_i16[ph * 16:(ph + 1) * 16, :])
            idxt = pool.tile([P, 8 * M], I16, tag=tag, name=tag)
            for g in range(8):
                nc.sync.dma_start(out=idxt[g * 16:(g + 1) * 16, :],
                                  in_=bounce_d[:, :])
            return idxt

        # persistent cross-stage tensors
        sq_pm = cst.tile([P, NT], F32, tag="sq_pm", name="sq_pm")
        bigT = cst.tile([5, N], F32, tag="bigT", name="bigT")
        gq = cst.tile([P, T1, TAB0_W], F32, tag="gq", name="gq")
        xyz1t = cst.tile([5, S1], F32, tag="xyz1t", name="xyz1t")
        lhsqA = cst.tile([5, P], F32, tag="lhsqA", name="lhsqA")
        nc.sync.dma_start(out=lhsqA[3:4, :], in_=ones_d[0:1, 0:P])
        nc.sync.dma_start(out=lhsqA[4:5, :], in_=zeros_d[0:1, 0:P])
        lhsqB = cst.tile([5, P], F32, tag="lhsqB", name="lhsqB")
        nc.sync.dma_start(out=lhsqB[3:4, :], in_=ones_d[0:1, 0:P])
        nc.sync.dma_start(out=lhsqB[4:5, :], in_=zeros_d[0:1, 0:P])
        nqb1 = cst.tile([4, 512], F32, tag="nqb1", name="nqb1")
        nc.sync.dma_start(out=nqb1[3:4, :], in_=ones_d[0:1, 0:512])
        nqb2 = cst.tile([4, 512], F32, tag="nqb2", name="nqb2")
        nc.sync.dma_start(out=nqb2[3:4, :], in_=ones_d[0:1, 0:512])
        feat1T = cst.tile([P, S1], F32, tag="feat1T", name="feat1T")
        sq1_pm = cst.tile([P, T1], F32, tag="sq1_pm", name="sq1_pm")
        gq2 = cst.tile([P, T2, TAB1_W], F32, tag="gq2", name="gq2")
        rhsF2 = cst.tile([5, S2], F32, tag="rhsF2", name="rhsF2")
        f1upT = cst.tile([P, S1], F32, tag="f1upT", name="f1upT")
        rhs2x = cst.tile([5, S1], F32, tag="rhs2x", name="rhs2x")
        v81 = cst.tile([P, NT, 8], F32, tag="v81", name="v81")
        p81 = cst.tile([P, NT, 8], U16, tag="p81", name="p81")
        p31 = cst.tile([P, NT * 3], I16, tag="p31", name="p31")
        wn1 = cst.tile([P, NT, 3], F32, tag="wn1", name="wn1")
        gfacc = cst.tile([P, 1], F32, tag="gfacc", name="gfacc")
        biasH = cst.tile([P, 1], F32, tag="biasH", name="biasH")

        def interp_weights(pool, sqpm_ap, v8_ap, nt, w_out):
            """d2 = |q|^2 - m -> dist -> normalized inv-dist weights -> w_out"""
            d2 = pool.tile([P, nt, 3], F32, tag="ipd2", name="ipd2")
            nc.vector.tensor_tensor(
                out=d2[:], in0=sqpm_ap.unsqueeze(2).to_broadcast([P, nt, 3]),
                in1=v8_ap, op=ALU.subtract)
            nc.scalar.activation(out=d2[:], in_=d2[:], func=AF.Relu)
            nc.scalar.activation(out=d2[:], in_=d2[:], func=AF.Sqrt)
            nc.vector.tensor_scalar_max(d2[:], d2[:], 1e-10)
            nc.vector.reciprocal(out=w_out, in_=d2[:])
            ws = pool.tile([P, nt], F32, tag="ipws", name="ipws")
            nc.vector.tensor_reduce(out=ws[:], in_=w_out, axis=AX.X, op=ALU.add)
            nc.vector.reciprocal(out=ws[:], in_=ws[:])
            nc.vector.tensor_tensor(
                out=w_out, in0=w_out,
                in1=ws[:].unsqueeze(2).to_broadcast([P, nt, 3]), op=ALU.mult)

        # ============ stage 0+1: geometry, embed, tab0 ============
        with tc.tile_pool(name="st01", bufs=2) as wk:
            xz = wk.tile([P, NT, 64], F32, tag="xz", name="xz", bufs=1)
            nc.sync.dma_start(
                out=xz[:, :, 0:3],
                in_=x_in.rearrange("(t p) c -> p t c", p=P)[:, :, 0:3])
            nc.vector.memset(xz[:, :, 3:64], 0.0)
            sqt = wk.tile([P, NT, 3], F32, tag="sqt", name="sqt", bufs=1)
            nc.vector.tensor_tensor(out=sqt[:], in0=xz[:, :, 0:3],
                                    in1=xz[:, :, 0:3], op=ALU.mult)
            nc.vector.tensor_reduce(out=sq_pm[:], in_=sqt[:], axis=AX.X,
                                    op=ALU.add)

            nc.sync.dma_start(out=bigT[0:3, :], in_=xT_in[0:3, :])
            nc.sync.dma_start(out=bigT[4:5, :], in_=ones_d[0:1, :])
            nsq_pm = wk.tile([P, P], F32, tag="nsq_pm", name="nsq_pm", bufs=1)
            nc.scalar.activation(out=nsq_pm[:], in_=trans(sq_pm[:]),
                                 func=AF.Copy, scale=-1.0)
            nc.sync.dma_start(
                out=bigT[3:4, :].rearrange("r (t p) -> r t p", p=P),
                in_=nsq_pm[:])

            nc.sync.dma_start(
                out=tab0_d.rearrange("(t p) c -> p t c", p=P)[:, :, 64:128],
                in_=xz[:])
            for g in range(16):
                stage = wk.tile([P, 8, 64], F32, tag="tab0stage", name="tab0stage")
                for cc in range(2):
                    c = g * 2 + cc
                    xc = wk.tile([6, 512], F32, tag="xc", name="xc", bufs=3)
                    nc.sync.dma_start(out=xc[:], in_=xT_in[:, c * 512:(c + 1) * 512])
                    pe = mmtile()
                    nc.tensor.matmul(out=pe[:64, :], lhsT=embw[:],
                                     rhs=xc[:],
                                     start=True, stop=True)
                    f0c = wk.tile([64, 512], F32, tag="f0c", name="f0c", bufs=3)
                    nc.scalar.activation(out=f0c[:], in_=pe[:64, :], func=AF.Relu,
                                         bias=embb[:])
                    nc.sync.dma_start(out=f0T_d[:, c * 512:(c + 1) * 512],
                                      in_=f0c[:])
                    for t4 in range(4):
                        pt = trans(f0c[:, t4 * 128:(t4 + 1) * 128])
                        nc.scalar.activation(out=stage[:, cc * 4 + t4, :],
                                             in_=pt, func=AF.Copy)
                nc.sync.dma_start(
                    out=tab0_d.rearrange("(t p) c -> p t c", p=P)[
                        :, g * 8:(g + 1) * 8, 0:64],
                    in_=stage[:])

        # ============ stage 2: SA1 ============
        with tc.tile_pool(name="sa1", bufs=2) as wk:
            nc.gpsimd.dma_gather(gq[:], tab0_d[:], i1sb[:], S1, S1, TAB0_W)
            nc.sync.dma_start(out=xyz1t[4:5, :], in_=ones_d[0:1, 0:S1])
            sq1t = wk.tile([P, T1, 3], F32, tag="sq1t", name="sq1t", bufs=1)
            nc.vector.tensor_tensor(out=sq1t[:], in0=gq[:, :, 64:67],
                                    in1=gq[:, :, 64:67], op=ALU.mult)
            nc.vector.tensor_reduce(out=sq1_pm[:], in_=sq1t[:], axis=AX.X,
                                    op=ALU.add)
            nsq1 = wk.tile([T1, P], F32, tag="nsq1", name="nsq1", bufs=1)
            nc.scalar.activation(out=nsq1[:], in_=trans(sq1_pm[:]),
                                 func=AF.Copy, scale=-1.0)
            nc.sync.dma_start(
                out=xyz1t[3:4, :].rearrange("r (t p) -> r t p", p=P),
                in_=nsq1[:])

            for qt in range(T1):
                sl1 = slice(qt * P, (qt + 1) * P)
                pQ = trans(gq[:, qt, 64:67])
                nc.scalar.activation(out=xyz1t[0:3, sl1], in_=pQ, func=AF.Copy)
                nc.scalar.activation(
                    out=nqb1[0:3, :].rearrange("r (j q) -> r j q", q=P),
                    in_=pQ.unsqueeze(1).to_broadcast([3, 4, P]),
                    func=AF.Copy, scale=-1.0)
                nc.scalar.activation(out=lhsqA[0:3, :], in_=pQ, func=AF.Copy,
                                     scale=2.0)

                candV = wk.tile([P, 512], F32, tag="candV", name="candV")
                candI = wk.tile([P, 512], U16, tag="candI", name="candI")
                for c in range(32):
                    pm = mmtile()
                    nc.tensor.matmul(out=pm[:], lhsT=lhsqA[:],
                                     rhs=bigT[:, c * 512:(c + 1) * 512],
                                     start=True, stop=True)
                    for w in range(2):
                        j = c * 2 + w
                        nc.vector.max(out=candV[:, j * 8:(j + 1) * 8],
                                      in_=pm[:, w * 256:(w + 1) * 256])
                        nc.vector.max_index(out=candI[:, j * 8:(j + 1) * 8],
                                            in_max=candV[:, j * 8:(j + 1) * 8],
                                            in_values=pm[:, w * 256:(w + 1) * 256])
                nc.vector.tensor_tensor(out=candI[:], in0=candI[:], in1=offs[:],
                                        op=ALU.add)
                candVw = wk.tile([P, 512], F32, tag="candVw", name="candVw")
                nc.vector.tensor_copy(out=candVw[:], in_=candV[:])
                selV = wk.tile([P, K1], F32, tag="selV", name="selV")
                for r in range(4):
                    rs = slice(r * 8, (r + 1) * 8)
                    nc.vector.max(out=selV[:, rs], in_=candVw[:])
                    if r < 3:
                        nc.vector.match_replace(out=candVw[:],
                                                in_to_replace=selV[:, rs],
                                                in_values=candVw[:],
                                                imm_value=NEG)
                mask = wk.tile([P, 512], U8, tag="selmask", name="selmask")
                nc.vector.tensor_scalar(out=mask[:], in0=candV[:],
                                        scalar1=selV[:, 31:32], scalar2=None,
                                        op0=ALU.is_ge)
                candIf = wk.tile([P, 512], F32, tag="candIf", name="candIf")
                nc.vector.tensor_copy(out=candIf[:], in_=candI[:])
                arr = wk.tile([P, 512], F32, tag="selarr", name="selarr")
                nc.vector.memset(arr[:], -1.0)
                nc.vector.copy_predicated(out=arr[:], mask=mask[:],
                                          data=candIf[:])
                selIf = wk.tile([P, K1], F32, tag="selIf", name="selIf")
                for r in range(4):
                    rs = slice(r * 8, (r + 1) * 8)
                    nc.vector.max(out=selIf[:, rs], in_=arr[:])
                    if r < 3:
                        nc.vector.match_replace(out=arr[:],
                                                in_to_replace=selIf[:, rs],
                                                in_values=arr[:],
                                                imm_value=-1.0)
                nbr16 = wk.tile([P, K1], I16, tag="nbr16", name="nbr16")
                nc.vector.tensor_copy(out=nbr16[:], in_=selIf[:])
                idxt = wrap_idx(wk, nbr16[:], ib1_ds[qt], "idxt1")
                gn = wk.tile([P, K1, TAB0_W], F32, tag="gn", name="gn")
                for k in range(4):
                    nc.gpsimd.dma_gather(gn[:, k * 8:(k + 1) * 8, :], tab0_d[:],
                                         idxt[:, k * 64:(k + 1) * 64],
                                         1024, 1024, TAB0_W)

                acc = wk.tile([P, P], F32, tag="sa1acc", name="sa1acc")
                for c in range(8):
                    pg = psT.tile([P, 512], F32, tag="trans", name="trans")
                    for j in range(4):
                        nc.tensor.matmul(out=pg[0:68, j * 128:(j + 1) * 128],
                                         lhsT=gn[:, c * 4 + j, 0:68], rhs=ident[:],
                                         is_transpose=True, start=True, stop=True)
                    gt = wk.tile([68, 512], F32, tag="gt", name="gt", bufs=3)
                    nc.scalar.activation(out=gt[:], in_=pg[0:68, :],
                                         func=AF.Copy)
                    pz = mmtile()
                    nc.tensor.matmul(out=pz[:], lhsT=w1aug[:], rhs=gt[:],
                                     start=True, stop=False)
                    nc.tensor.matmul(out=pz[:], lhsT=w1q[:], rhs=nqb1[:],
                                     start=False, stop=True)
                    h1 = wk.tile([P, 512], F32, tag="h1", name="h1", bufs=3)
                    nc.scalar.activation(out=h1[:], in_=pz[:], func=AF.Relu)
                    pz2 = mmtile()
                    nc.tensor.matmul(out=pz2[:], lhsT=w2sb[:], rhs=h1[:],
                                     start=True, stop=True)
                    red = wk.tile([P, P], F32, tag="sa1red", name="sa1red",
                                  bufs=3)
                    nc.vector.tensor_reduce(
                        out=red[:], in_=pz2[:].rearrange("f (s q) -> f q s", q=P),
                        axis=AX.X, op=ALU.max)
                    if c == 0:
                        nc.vector.tensor_copy(out=acc[:], in_=red[:])
                    else:
                        nc.vector.tensor_tensor(out=acc[:], in0=acc[:],
                                                in1=red[:], op=ALU.max)
                nc.scalar.activation(out=feat1T[:, sl1], in_=acc[:],
                                     func=AF.Relu, bias=b2sb[:])

            stage1 = wk.tile([P, T1, TAB1_W], F32, tag="stage1", name="stage1",
                             bufs=1)
            for t in range(T1):
                pf = trans(feat1T[:, t * P:(t + 1) * P])
                nc.scalar.activation(out=stage1[:, t, 0:128], in_=pf,
                                     func=AF.Copy)
            nc.vector.tensor_copy(out=stage1[:, :, 128:131], in_=gq[:, :, 64:67])
            nc.vector.memset(stage1[:, :, 131:192], 0.0)
            nc.sync.dma_start(out=tab1_d.rearrange("(t p) c -> p t c", p=P),
                              in_=stage1[:])

        # ============ stage 3: SA2 ============
        with tc.tile_pool(name="sa2", bufs=2) as wk, \
             tc.tile_pool(name="psSel", bufs=2, space="PSUM") as psS:
            nc.gpsimd.dma_gather(gq2[:], tab1_d[:], i2sb[:], S2, S2, TAB1_W)
            nc.sync.dma_start(out=rhsF2[3:4, :], in_=zeros_d[0:1, 0:S2])
            sq2t = wk.tile([P, T2, 3], F32, tag="sq2t", name="sq2t", bufs=1)
            nc.vector.tensor_tensor(out=sq2t[:], in0=gq2[:, :, 128:131],
                                    in1=gq2[:, :, 128:131], op=ALU.mult)
            sq2_pm = wk.tile([P, T2], F32, tag="sq2_pm", name="sq2_pm", bufs=1)
            nc.vector.tensor_reduce(out=sq2_pm[:], in_=sq2t[:], axis=AX.X,
                                    op=ALU.add)
            nsq2 = wk.tile([T2, P], F32, tag="nsq2", name="nsq2", bufs=1)
            nc.scalar.activation(out=nsq2[:], in_=trans(sq2_pm[:]),
                                 func=AF.Copy, scale=-1.0)
            nc.sync.dma_start(
                out=rhsF2[4:5, :].rearrange("r (t p) -> r t p", p=P),
                in_=nsq2[:])

            feat2T = [cst.tile([P, S2], F32, tag=f"feat2T{h}", name=f"feat2T{h}")
                      for h in range(2)]
            for t2 in range(T2):
                sl2 = slice(t2 * P, (t2 + 1) * P)
                pQ = trans(gq2[:, t2, 128:131])
                nc.scalar.activation(out=lhsqB[0:3, :], in_=pQ, func=AF.Copy,
                                     scale=2.0)
                nc.scalar.activation(out=rhsF2[0:3, sl2], in_=pQ, func=AF.Copy,
                                     scale=2.0)
                nc.scalar.activation(
                    out=nqb2[0:3, :].rearrange("r (j q) -> r j q", q=P),
                    in_=pQ.unsqueeze(1).to_broadcast([3, 4, P]),
                    func=AF.Copy, scale=-1.0)

                pm2 = psS.tile([P, S1], F32, tag="sel", name="sel")
                for hh in range(2):
                    nc.tensor.matmul(out=pm2[:, hh * 512:(hh + 1) * 512],
                                     lhsT=lhsqB[:],
                                     rhs=xyz1t[:, hh * 512:(hh + 1) * 512],
                                     start=True, stop=True)
                selV2 = wk.tile([P, K2], F32, tag="selV2", name="selV2")
                selI2 = wk.tile([P, K2], U16, tag="selI2", name="selI2")
                for r in range(4):
                    rs = slice(r * 8, (r + 1) * 8)
                    nc.vector.max(out=selV2[:, rs], in_=pm2[:])
                    nc.vector.max_index(out=selI2[:, rs], in_max=selV2[:, rs],
                                        in_values=pm2[:])
                    if r < 3:
                        nc.vector.match_replace(out=pm2[:],
                                                in_to_replace=selV2[:, rs],
                                                in_values=pm2[:], imm_value=NEG)
                nbr2 = wk.tile([P, K2], I16, tag="nbr2", name="nbr2")
                nc.vector.tensor_copy(out=nbr2[:], in_=selI2[:])
                idxt2 = wrap_idx(wk, nbr2[:], ib2_ds[t2], "idxt2")
                gn2 = wk.tile([P, K2, TAB1_W], F32, tag="gn2", name="gn2")
                for k in range(4):
                    nc.gpsimd.dma_gather(gn2[:, k * 8:(k + 1) * 8, :],
                                         tab1_d[:],
                                         idxt2[:, k * 64:(k + 1) * 64],
                                         1024, 1024, TAB1_W)

                acc2 = [wk.tile([P, P], F32, tag=f"sa2acc{h}", name=f"sa2acc{h}")
                        for h in range(2)]
                for c in range(8):
                    pga = psT.tile([P, 512], F32, tag="trans", name="trans")
                    pgb = psT.tile([P, 512], F32, tag="trans", name="trans")
                    for j in range(4):
                        nc.tensor.matmul(out=pga[:, j * 128:(j + 1) * 128],
                                         lhsT=gn2[:, c * 4 + j, 0:128],
                                         rhs=ident[:], is_transpose=True,
                                         start=True, stop=True)
                        nc.tensor.matmul(out=pgb[0:3, j * 128:(j + 1) * 128],
                                         lhsT=gn2[:, c * 4 + j, 128:131],
                                         rhs=ident[:], is_transpose=True,
                                         start=True, stop=True)
                    gta = wk.tile([P, 512], F32, tag="gta", name="gta", bufs=3)
                    gtb = wk.tile([3, 512], F32, tag="gtb", name="gtb", bufs=3)
                    nc.scalar.activation(out=gta[:], in_=pga[:], func=AF.Copy)
                    nc.scalar.activation(out=gtb[:], in_=pgb[0:3, :],
                                         func=AF.Copy)
                    h1c = []
                    for h in range(2):
                        pz = mmtile()
                        nc.tensor.matmul(out=pz[:], lhsT=v1A[h][:], rhs=gta[:],
                                         start=True, stop=False)
                        nc.tensor.matmul(out=pz[:], lhsT=v1rel[h][:], rhs=gtb[:],
                                         start=False, stop=False)
                        nc.tensor.matmul(out=pz[:], lhsT=v1q[h][:], rhs=nqb2[:],
                                         start=False, stop=True)
                        hh_ = wk.tile([P, 512], F32, tag=f"h1c{h}",
                                      name=f"h1c{h}", bufs=3)
                        nc.scalar.activation(out=hh_[:], in_=pz[:], func=AF.Relu)
                        h1c.append(hh_)
                    for h in range(2):
                        pz = mmtile()
                        nc.tensor.matmul(out=pz[:], lhsT=v2sb[0][h][:],
                                         rhs=h1c[0][:], start=True, stop=False)
                        nc.tensor.matmul(out=pz[:], lhsT=v2sb[1][h][:],
                                         rhs=h1c[1][:], start=False, stop=True)
                        red = wk.tile([P, P], F32, tag="sa2red", name="sa2red",
                                      bufs=3)
                        nc.vector.tensor_reduce(
                            out=red[:],
                            in_=pz[:].rearrange("f (s q) -> f q s", q=P),
                            axis=AX.X, op=ALU.max)
                        if c == 0:
                            nc.vector.tensor_copy(out=acc2[h][:], in_=red[:])
                        else:
                            nc.vector.tensor_tensor(out=acc2[h][:],
                                                    in0=acc2[h][:],
                                                    in1=red[:], op=ALU.max)
                for h in range(2):
                    nc.scalar.activation(out=feat2T[h][:, sl2], in_=acc2[h][:],
                                         func=AF.Relu, bias=c2sb[:, h:h + 1])

            stage2 = wk.tile([P, T2, 256], F32, tag="stage2", name="stage2",
                             bufs=1)
            for t2 in range(T2):
                for h in range(2):
                    pf = trans(feat2T[h][:, t2 * P:(t2 + 1) * P])
                    nc.scalar.activation(out=stage2[:, t2, h * 128:(h + 1) * 128],
                                         in_=pf, func=AF.Copy)
            nc.sync.dma_start(out=tab2_d.rearrange("(t p) c -> p t c", p=P),
                              in_=stage2[:])

        # ============ stage 4: FP2 ============
        with tc.tile_pool(name="fp2", bufs=2) as wk:
            v8f = wk.tile([P, T1, 8], F32, tag="v8f", name="v8f", bufs=1)
            p8f = wk.tile([P, T1, 8], U16, tag="p8f", name="p8f", bufs=1)
            for qt in range(T1):
                pm3 = mmtile()
                nc.tensor.matmul(out=pm3[:, 0:S2],
                                 lhsT=xyz1t[:, qt * P:(qt + 1) * P],
                                 rhs=rhsF2[:], start=True, stop=True)
                nc.vector.max(out=v8f[:, qt, :], in_=pm3[:, 0:S2])
                nc.vector.max_index(out=p8f[:, qt, :], in_max=v8f[:, qt, :],
                                    in_values=pm3[:, 0:S2])

            wn2 = wk.tile([P, T1, 3], F32, tag="wn2", name="wn2", bufs=1)
            interp_weights(wk, sq1_pm[:], v8f[:, :, 0:3], T1, wn2[:])
            p3f = wk.tile([P, T1 * 3], I16, tag="p3f", name="p3f", bufs=1)
            nc.vector.tensor_copy(out=p3f[:].rearrange("p (t j) -> p t j", j=3),
                                  in_=p8f[:, :, 0:3])
            idxtf2 = wrap_idx(wk, p3f[:], ibf2_d, "idxtf2")
            gi2 = wk.tile([P, T1, 3, 256], F32, tag="gi2", name="gi2", bufs=1)
            gi2v = gi2[:].rearrange("p t j c -> p (t j) c")
            for k in range(3):
                nc.gpsimd.dma_gather(gi2v[:, k * 8:(k + 1) * 8, :], tab2_d[:],
                                     idxtf2[:, k * 64:(k + 1) * 64],
                                     1024, 1024, 256)
            nc.vector.tensor_tensor(
                out=gi2[:], in0=gi2[:],
                in1=wn2[:].unsqueeze(3).to_broadcast([P, T1, 3, 256]),
                op=ALU.mult)
            it2pm = wk.tile([P, T1, 256], F32, tag="it2pm", name="it2pm", bufs=1)
            nc.vector.tensor_reduce(out=it2pm[:],
                                    in_=gi2[:].rearrange("p t j c -> p t c j"),
                                    axis=AX.X, op=ALU.add)
            itT2 = [wk.tile([P, S1], F32, tag=f"itT2{h}", name=f"itT2{h}",
                            bufs=1) for h in range(2)]
            for t in range(T1):
                for h in range(2):
                    pf = trans(it2pm[:, t, h * 128:(h + 1) * 128])
                    nc.scalar.activation(out=itT2[h][:, t * P:(t + 1) * P],
                                         in_=pf, func=AF.Copy)
            for c in range(2):
                cs = slice(c * 512, (c + 1) * 512)
                pz = mmtile()
                nc.tensor.matmul(out=pz[:], lhsT=f2w1[0][:], rhs=feat1T[:, cs],
                                 start=True, stop=False)
                nc.tensor.matmul(out=pz[:], lhsT=f2w1[1][:], rhs=itT2[0][:, cs],
                                 start=False, stop=False)
                nc.tensor.matmul(out=pz[:], lhsT=f2w1[2][:], rhs=itT2[1][:, cs],
                                 start=False, stop=True)
                hf = wk.tile([P, 512], F32, tag="fp2h", name="fp2h", bufs=3)
                nc.scalar.activation(out=hf[:], in_=pz[:], func=AF.Relu,
                                     bias=f2b1[:])
                pz2 = mmtile()
                nc.tensor.matmul(out=pz2[:], lhsT=f2w2[:], rhs=hf[:],
                                 start=True, stop=True)
                nc.scalar.activation(out=f1upT[:, cs], in_=pz2[:], func=AF.Relu,
                                     bias=f2b2[:])
            stagef = wk.tile([P, T1, 128], F32, tag="stagef", name="stagef",
                             bufs=1)
            for t in range(T1):
                pf = trans(f1upT[:, t * P:(t + 1) * P])
                nc.scalar.activation(out=stagef[:, t, :], in_=pf, func=AF.Copy)
            nc.sync.dma_start(out=tabf_d.rearrange("(t p) c -> p t c", p=P),
                              in_=stagef[:])

        # ============ stage 5: FP1 ============
        with tc.tile_pool(name="fp1", bufs=2) as wk, \
             tc.tile_pool(name="psSel1", bufs=2, space="PSUM") as psS:
            nc.scalar.activation(out=rhs2x[0:3, :], in_=xyz1t[0:3, :],
                                 func=AF.Copy, scale=2.0)
            nc.sync.dma_start(out=rhs2x[3:4, :], in_=zeros_d[0:1, 0:S1])
            nc.sync.dma_start(out=rhs2x[4:5, :], in_=xyz1t[3:4, :])

            for qt in range(NT):
                pm4 = psS.tile([P, S1], F32, tag="sel", name="sel")
                for hh in range(2):
                    nc.tensor.matmul(out=pm4[:, hh * 512:(hh + 1) * 512],
                                     lhsT=bigT[:, qt * P:(qt + 1) * P],
                                     rhs=rhs2x[:, hh * 512:(hh + 1) * 512],
                                     start=True, stop=True)
                nc.vector.max(out=v81[:, qt, :], in_=pm4[:])
                nc.vector.max_index(out=p81[:, qt, :], in_max=v81[:, qt, :],
                                    in_values=pm4[:])

            interp_weights(wk, sq_pm[:], v81[:, :, 0:3], NT, wn1[:])
            nc.vector.tensor_copy(out=p31[:].rearrange("p (t j) -> p t j", j=3),
                                  in_=p81[:, :, 0:3])
            idxtf1 = wrap_idx(wk, p31[:], ibf1_d, "idxtf1")

            GT = 8
            for g in range(NT // GT):
                gi1 = wk.tile([P, GT, 3, 128], F32, tag="gi1", name="gi1")
                gi1v = gi1[:].rearrange("p t j c -> p (t j) c")
                for k in range(3):
                    nc.gpsimd.dma_gather(
                        gi1v[:, k * 8:(k + 1) * 8, :], tabf_d[:],
                        idxtf1[:, g * 192 + k * 64:g * 192 + (k + 1) * 64],
                        1024, 1024, 128)
                nc.vector.tensor_tensor(
                    out=gi1[:], in0=gi1[:],
                    in1=wn1[:, g * GT:(g + 1) * GT, :].unsqueeze(3).to_broadcast(
                        [P, GT, 3, 128]),
                    op=ALU.mult)
                it1pm = wk.tile([P, GT, 128], F32, tag="it1pm", name="it1pm")
                nc.vector.tensor_reduce(
                    out=it1pm[:], in_=gi1[:].rearrange("p t j c -> p t c j"),
                    axis=AX.X, op=ALU.add)
                itT1 = wk.tile([P, GT * 128], F32, tag="itT1", name="itT1")
                for t in range(GT):
                    pf = trans(it1pm[:, t, :])
                    nc.scalar.activation(out=itT1[:, t * P:(t + 1) * P], in_=pf,
                                         func=AF.Copy)
                f0Tc = wk.tile([64, GT * 128], F32, tag="f0Tc", name="f0Tc")
                nc.sync.dma_start(out=f0Tc[:],
                                  in_=f0T_d[:, g * GT * P:(g + 1) * GT * P])
                for c in range(2):
                    cs = slice(c * 512, (c + 1) * 512)
                    gcs = slice(g * GT * P + c * 512, g * GT * P + (c + 1) * 512)
                    pz = mmtile()
                    nc.tensor.matmul(out=pz[:], lhsT=f1w1b[:], rhs=itT1[:, cs],
                                     start=True, stop=False)
                    nc.tensor.matmul(out=pz[:], lhsT=f1w1a[:], rhs=f0Tc[:, cs],
                                     start=False, stop=True)
                    hf = wk.tile([P, 512], F32, tag="fp1h", name="fp1h", bufs=3)
                    nc.scalar.activation(out=hf[:], in_=pz[:], func=AF.Relu,
                                         bias=f1b1[:])
                    pz2 = mmtile()
                    nc.tensor.matmul(out=pz2[:], lhsT=f1w2[:], rhs=hf[:],
                                     start=True, stop=True)
                    fu = wk.tile([P, 512], F32, tag="fuc", name="fuc", bufs=3)
                    nc.scalar.activation(out=fu[:], in_=pz2[:], func=AF.Relu,
                                         bias=f1b2[:])
                    nc.sync.dma_start(out=fuT_d[:, gcs], in_=fu[:])
                    red = wk.tile([P, 1], F32, tag="gfred", name="gfred", bufs=3)
                    nc.vector.tensor_reduce(out=red[:], in_=fu[:], axis=AX.X,
                                            op=ALU.max)
                    if g == 0 and c == 0:
                        nc.vector.tensor_copy(out=gfacc[:], in_=red[:])
                    else:
                        nc.vector.tensor_tensor(out=gfacc[:], in0=gfacc[:],
                                                in1=red[:], op=ALU.max)

        # ============ stage 6: head ============
        with tc.tile_pool(name="head", bufs=2) as wk:
            pc = mmtile()
            nc.tensor.matmul(out=pc[:, 0:1], lhsT=hw1b[:], rhs=gfacc[:],
                             start=True, stop=True)
            nc.vector.tensor_tensor(out=biasH[:], in0=pc[:, 0:1], in1=hb1[:],
                                    op=ALU.add)
            for g in range(8):
                ostage = wk.tile([P, 16, 13], F32, tag="ostage", name="ostage")
                for c4 in range(4):
                    c = g * 4 + c4
                    cs = slice(c * 512, (c + 1) * 512)
                    fuc = wk.tile([P, 512], F32, tag="hfuc", name="hfuc", bufs=3)
                    nc.sync.dma_start(out=fuc[:], in_=fuT_d[:, cs])
                    pz = mmtile()
                    nc.tensor.matmul(out=pz[:], lhsT=hw1a[:], rhs=fuc[:],
                                     start=True, stop=True)
                    h1 = wk.tile([P, 512], F32, tag="hh1", name="hh1", bufs=3)
                    nc.scalar.activation(out=h1[:], in_=pz[:], func=AF.Relu,
                                         bias=biasH[:])
                    pz2 = mmtile()
                    nc.tensor.matmul(out=pz2[:64, :], lhsT=hw2[:], rhs=h1[:],
                                     start=True, stop=True)
                    h2 = wk.tile([64, 512], F32, tag="hh2", name="hh2", bufs=3)
                    nc.scalar.activation(out=h2[:], in_=pz2[:64, :],
                                         func=AF.Relu, bias=hb2[:])
                    pz3 = mmtile()
                    nc.tensor.matmul(out=pz3[:13, :], lhsT=hw3[:], rhs=h2[:],
                                     start=True, stop=True)
                    oT = wk.tile([13, 512], F32, tag="hoT", name="hoT", bufs=3)
                    nc.vector.tensor_tensor(
                        out=oT[:], in0=pz3[:13, :],
                        in1=hb3[:, 0:1].to_broadcast([13, 512]), op=ALU.add)
                    po = psT.tile([P, 512], F32, tag="trans", name="trans")
                    for t in range(4):
                        nc.tensor.matmul(out=po[:, t * 13:(t + 1) * 13],
                                         lhsT=oT[:, t * 128:(t + 1) * 128],
                                         rhs=ident[0:13, 0:13],
                                         is_transpose=True, start=True, stop=True)
                    nc.scalar.activation(
                        out=ostage[:, c4 * 4:(c4 + 1) * 4, :],
                        in_=po[:, 0:52].rearrange("p (t c) -> p t c", c=13),
                        func=AF.Copy)
                nc.sync.dma_start(
                    out=out_d.rearrange("(t p) c -> p t c", p=P)[
                        :, g * 16:(g + 1) * 16, :],
                    in_=ostage[:])

    return nc


# ---------------------------------------------------------------- host side
_CACHED_NC = None


def _get_nc():
    global _CACHED_NC
    if _CACHED_NC is None:
        nc = build_nc()
        nc.finalize()
        _CACHED_NC = nc
    return _CACHED_NC


def _per_core_inputs(b, inputs):
    x = np.ascontiguousarray(np.asarray(inputs["x"][b]), dtype=np.float32)
    i1 = np.asarray(inputs["idx_s1"][b]).astype(np.int16)
    i1w = np.tile(i1.reshape(S1 // 16, 16).T, (8, 1))
    i2 = np.asarray(inputs["idx_s2"][b]).astype(np.int16)
    i2w = np.tile(i2.reshape(S2 // 16, 16).T, (8, 1))
    f32 = lambda a: np.ascontiguousarray(np.asarray(a), dtype=np.float32)
    return {
        "x": x,
        "xT": np.ascontiguousarray(x.T),
        "i1w": np.ascontiguousarray(i1w),
        "i2w": np.ascontiguousarray(i2w),
        "embw": f32(inputs["embed_w"]),
        "embb": f32(inputs["embed_b"]).reshape(64, 1),
        "w1": f32(inputs["sa1_w1"]),
        "b1r": f32(inputs["sa1_b1"]).reshape(1, 128),
        "w2": f32(inputs["sa1_w2"]),
        "b2": f32(inputs["sa1_b2"]).reshape(128, 1),
        "v1": f32(inputs["sa2_w1"]),
        "c1r": f32(inputs["sa2_b1"]).reshape(1, 256),
        "v2": f32(inputs["sa2_w2"]),
        "c2": np.ascontiguousarray(f32(inputs["sa2_b2"]).reshape(2, 128).T),
        "f2w1": f32(inputs["fp2_w1"]),
        "f2b1": f32(inputs["fp2_b1"]).reshape(128, 1),
        "f2w2": f32(inputs["fp2_w2"]),
        "f2b2": f32(inputs["fp2_b2"]).reshape(128, 1),
        "f1w1": f32(inputs["fp1_w1"]),
        "f1b1": f32(inputs["fp1_b1"]).reshape(128, 1),
        "f1w2": f32(inputs["fp1_w2"]),
        "f1b2": f32(inputs["fp1_b2"]).reshape(128, 1),
        "hw1": f32(inputs["head_w1"]),
        "hb1": f32(inputs["head_b1"]).reshape(128, 1),
        "hw2": f32(inputs["head_w2"]),
        "hb2": f32(inputs["head_b2"]).reshape(64, 1),
        "hw3": f32(inputs["head_w3"]),
        "hb3": f32(inputs["head_b3"]).reshape(13, 1),
    }


def run(inputs, trace=False, **kw):
    nc = _get_nc()
    B = inputs["x"].shape[0]
    in_maps = [_per_core_inputs(b, inputs) for b in range(B)]
    res = run_bass_kernel_spmd(nc, in_maps, core_ids=list(range(B)),
                               trace=trace, **kw)
    out = np.stack([res.results[b]["out"] for b in range(B)])
    return out, res


def kernel(**inputs):
    return run(inputs)[0]


if __name__ == "__main__":
    build_nc()
    print("built ok")



# revision 8
# speedup vs baseline: 1.4081x; 1.4081x over previous
"""PointNet++-lite segmentation on 8 Trainium2 cores (batch-parallel, one
point cloud per core). Self-contained: hardcodes shapes from the problem spec.

Per-core pipeline (all on device):
  embed MLP -> SA1 (KNN top-32 of 16384, gather, 2-layer MLP, max-pool)
  -> SA2 (KNN top-32 of 1024) -> FP2/FP1 (3-NN inverse-distance interp)
  -> global-max head MLP -> (16384, 13) logits.

fp16 datapath: all PE matmuls run on fp16 operands (4x the fp32 rate), with
fp32 PSUM accumulation.  KNN ranking uses m = 2 q.x - |x|^2 (row-constant
|q|^2 dropped); |x|^2 enters the fp16 matmul split into hi+lo fp16 halves so
m keeps ~22 mantissa bits (self-distances stay ~0, exact inverse-distance
weights).  Neighbor tables are fp16 rows in DRAM; SA1/SA2 gathers use
dma_gather transpose mode which lands features on partitions, removing the
per-neighbor PE transposes.  Selection runs on the vector engine max8 /
max_index over 1024-wide PSUM blocks.
"""

from contextlib import ExitStack

import numpy as np

import concourse.bass as bass
import concourse.mybir as mybir
from concourse.bacc import Bacc
from concourse.bass_utils import run_bass_kernel_spmd
from concourse.masks import make_identity
from concourse.tile import TileContext

F32 = mybir.dt.float32
F16 = mybir.dt.float16
U16 = mybir.dt.uint16
U8 = mybir.dt.uint8
I16 = mybir.dt.int16
AF = mybir.ActivationFunctionType
ALU = mybir.AluOpType
AX = mybir.AxisListType

P = 128
N = 16384
S1, K1 = 1024, 32
S2, K2 = 256, 32
NCLS = 13
NEG = -3.0e38

NT = N // P        # 128 point tiles
T1 = S1 // P       # 8 SA1 query tiles
T2 = S2 // P       # 2 SA2 query tiles
NG = 16            # FP1 groups (8 tiles each)
GT = NT // NG      # tiles per FP1 group


def build_nc():
    nc = Bacc()

    xh_in = nc.dram_tensor("xh", [N, 6], F16, kind="ExternalInput")
    xTh_in = nc.dram_tensor("xTh", [6, N], F16, kind="ExternalInput")
    i1_in = nc.dram_tensor("i1w", [P, S1 // 16], I16, kind="ExternalInput")
    i2_in = nc.dram_tensor("i2w", [P, S2 // 16], I16, kind="ExternalInput")
    wdecl16 = [
        ("embw", [6, 64]),
        ("w1", [67, 128]), ("b1r", [1, 128]), ("w2", [128, 128]),
        ("v1", [131, 256]), ("c1r", [1, 256]), ("v2", [256, 256]),
        ("f2w1", [384, 128]), ("f2w2", [128, 128]),
        ("f1w1", [192, 128]), ("f1w2", [128, 128]),
        ("hw1", [256, 128]), ("hw2", [128, 64]), ("hw3", [64, 13]),
    ]
    wdecl32 = [
        ("embb", [64, 1]), ("b2", [128, 1]), ("c2", [128, 2]),
        ("f2b1", [128, 1]), ("f2b2", [128, 1]),
        ("f1b1", [128, 1]), ("f1b2", [128, 1]),
        ("hb1", [128, 1]), ("hb2", [64, 1]), ("hb3", [13, 1]),
    ]
    din = {nm: nc.dram_tensor(nm, sh, F16, kind="ExternalInput")
           for nm, sh in wdecl16}
    din.update({nm: nc.dram_tensor(nm, sh, F32, kind="ExternalInput")
                for nm, sh in wdecl32})
    out_d = nc.dram_tensor("out", [N, NCLS], F32, kind="ExternalOutput")

    tab0_d = nc.dram_tensor("tab0", [N, 128], F16)
    tab1_d = nc.dram_tensor("tab1", [S1, 256], F16)
    tab2_d = nc.dram_tensor("tab2", [S2, 256], F16)
    tabf_d = nc.dram_tensor("tabf", [S1, 128], F16)
    ib1_ds = [nc.dram_tensor(f"ib1_{t}", [16, 256], I16) for t in range(T1)]
    ib2_ds = [nc.dram_tensor(f"ib2_{t}", [16, 256], I16) for t in range(T2)]
    ibf2_d = nc.dram_tensor("ibf2", [16, 192], I16)
    ones2_d = nc.dram_tensor("ones2_d", [2, 128], F16)
    ones1_d = nc.dram_tensor("ones1_d", [1, 512], F16)
    ibf1_ds = [nc.dram_tensor(f"ibf1_{g}", [16, 192], I16) for g in range(NG)]

    with TileContext(nc) as tc, ExitStack() as ctx:
        cst = ctx.enter_context(tc.tile_pool(name="cst", bufs=1))
        psB = ctx.enter_context(tc.tile_pool(name="psB", bufs=2, space="PSUM"))
        psA = ctx.enter_context(tc.tile_pool(name="psA", bufs=2, space="PSUM"))
        psT = ctx.enter_context(tc.tile_pool(name="psT", bufs=2, space="PSUM"))

        identh = cst.tile([P, P], F16, tag="identh", name="identh")
        make_identity(nc, identh[:])

        def bigtile():
            return psB.tile([P, 1024], F32, tag="big", name="big")

        def mmtile():
            return psA.tile([P, 512], F32, tag="mm", name="mm")

        def trans16(in_ap):
            """PE transpose of fp16 data: in_(p,f) -> fp16 psum (f,p)."""
            pt = psT.tile([P, 512], F16, tag="trans", name="trans")
            k = in_ap.shape[0]
            f = in_ap.shape[-1]
            nc.tensor.matmul(out=pt[:f, :k], lhsT=in_ap, rhs=identh[:k, :k],
                             is_transpose=True, start=True, stop=True)
            return pt[:f, :k]

        # ---------------- constants / weights ----------------
        def load(name, src, shape, dtype=F16):
            t = cst.tile(list(shape), dtype, tag=name, name=name)
            nc.sync.dma_start(out=t[:], in_=src)
            return t

        embw = load("embw", din["embw"][:], (6, 64))
        embb = load("embb", din["embb"][:], (64, 1), F32)
        w2sb = load("w2sb", din["w2"][:], (128, 128))
        b2sb = load("b2sb", din["b2"][:], (128, 1), F32)
        c2sb = load("c2sb", din["c2"][:], (128, 2), F32)
        f2b1 = load("f2b1", din["f2b1"][:], (128, 1), F32)
        f2w2 = load("f2w2", din["f2w2"][:], (128, 128))
        f2b2 = load("f2b2", din["f2b2"][:], (128, 1), F32)
        f1b1 = load("f1b1", din["f1b1"][:], (128, 1), F32)
        f1w2 = load("f1w2", din["f1w2"][:], (128, 128))
        f1b2 = load("f1b2", din["f1b2"][:], (128, 1), F32)
        hb1 = load("hb1", din["hb1"][:], (128, 1), F32)
        hw2 = load("hw2", din["hw2"][:], (128, 64))
        hb2 = load("hb2", din["hb2"][:], (64, 1), F32)
        hw3 = load("hw3", din["hw3"][:], (64, 13))
        hb3 = load("hb3", din["hb3"][:], (13, 1), F32)

        # SA1 grouped-MLP weights: rows [feat(64), rel_xyz(3)]
        w1aug = cst.tile([67, 128], F16, tag="w1aug", name="w1aug")
        nc.sync.dma_start(out=w1aug[0:64, :], in_=din["w1"][3:67, :])
        nc.sync.dma_start(out=w1aug[64:67, :], in_=din["w1"][0:3, :])
        w1q = cst.tile([4, 128], F16, tag="w1q", name="w1q")
        nc.sync.dma_start(out=w1q[0:3, :], in_=din["w1"][0:3, :])
        nc.sync.dma_start(out=w1q[3:4, :], in_=din["b1r"][:])

        v1A = [load(f"v1A{h}", din["v1"][3:131, h * 128:(h + 1) * 128],
                    (128, 128)) for h in range(2)]
        v1rel, v1q = [], []
        for h in range(2):
            sl = slice(h * 128, (h + 1) * 128)
            t = cst.tile([3, 128], F16, tag=f"v1rel{h}", name=f"v1rel{h}")
            nc.sync.dma_start(out=t[:], in_=din["v1"][0:3, sl])
            v1rel.append(t)
            t2 = cst.tile([4, 128], F16, tag=f"v1q{h}", name=f"v1q{h}")
            nc.sync.dma_start(out=t2[0:3, :], in_=din["v1"][0:3, sl])
            nc.sync.dma_start(out=t2[3:4, :], in_=din["c1r"][0:1, sl])
            v1q.append(t2)
        v2sb = [[load(f"v2{k}{h}",
                      din["v2"][k * 128:(k + 1) * 128, h * 128:(h + 1) * 128],
                      (128, 128)) for h in range(2)] for k in range(2)]
        f2w1 = [load(f"f2w1{k}", din["f2w1"][k * 128:(k + 1) * 128, :],
                     (128, 128)) for k in range(3)]
        f1w1a = load("f1w1a", din["f1w1"][0:64, :], (64, 128))
        f1w1b = load("f1w1b", din["f1w1"][64:192, :], (128, 128))
        hw1a = load("hw1a", din["hw1"][0:128, :], (128, 128))
        hw1b = load("hw1b", din["hw1"][128:256, :], (128, 128))
        i1sb = load("i1sb", i1_in[:], (P, S1 // 16), I16)
        i2sb = load("i2sb", i2_in[:], (P, S2 // 16), I16)

        # block offsets for 1024-wide max8 blocks: j*1024, 8 copies each
        offs = cst.tile([P, 128], U16, tag="offs", name="offs")
        nc.gpsimd.iota(out=offs[:], pattern=[[1024, 16], [0, 8]], base=0,
                       channel_multiplier=0)

        ones2 = cst.tile([2, P], F16, tag="ones2", name="ones2")
        nc.vector.memset(ones2[:], 1.0)
        nc.sync.dma_start(out=ones2_d[:], in_=ones2[:])
        onesr = cst.tile([1, 512], F16, tag="onesr", name="onesr")
        nc.vector.memset(onesr[:], 1.0)
        nc.sync.dma_start(out=ones1_d[:], in_=onesr[:])

        def wrap_idx(pool, src_i16, bounce_d, tag):
            """src (128, M) i16 -> replicated wrapped idx tile (128, 8*M)
            via DRAM bounce.  Flat gather slot j*128+q reads src[q, j]."""
            M = src_i16.shape[-1]
            bw = bounce_d[:, :].rearrange("c (j e) -> c j e", e=8)
            for ph in range(8):
                nc.sync.dma_start(out=bw[:, 0:M, ph],
                                  in_=src_i16[ph * 16:(ph + 1) * 16, :])
            idxt = pool.tile([P, 8 * M], I16, tag=tag, name=tag)
            for g in range(8):
                nc.sync.dma_start(out=idxt[g * 16:(g + 1) * 16, :],
                                  in_=bounce_d[:, :])
            return idxt

        # persistent cross-stage tensors
        sq_pm = cst.tile([P, NT], F32, tag="sq_pm", name="sq_pm")
        bigT = cst.tile([5, N], F16, tag="bigT", name="bigT")
        f0TS = cst.tile([64, N], F16, tag="f0TS", name="f0TS")
        fuTS = cst.tile([P, N], F16, tag="fuTS", name="fuTS")
        gq = cst.tile([P, T1, 128], F16, tag="gq", name="gq")
        xyz1a = cst.tile([3, S1], F16, tag="xyz1a", name="xyz1a")
        sqn1 = cst.tile([2, S1], F16, tag="sqn1", name="sqn1")
        rhs2a = cst.tile([3, S1], F16, tag="rhs2a", name="rhs2a")
        rhsF2a = cst.tile([3, S2], F16, tag="rhsF2a", name="rhsF2a")
        rhsF2b = cst.tile([2, S2], F16, tag="rhsF2b", name="rhsF2b")
        sq1_pm = cst.tile([P, T1], F32, tag="sq1_pm", name="sq1_pm")
        feat1T = cst.tile([P, S1], F16, tag="feat1T", name="feat1T")
        f1upT = cst.tile([P, S1], F16, tag="f1upT", name="f1upT")
        gfacc = cst.tile([P, 1], F32, tag="gfacc", name="gfacc")
        biasH = cst.tile([P, 1], F32, tag="biasH", name="biasH")

        # ============ stage 0+1: geometry, embed, tab0 ============
        with tc.tile_pool(name="st01", bufs=2) as wk:
            xzh = wk.tile([P, NT, 6], F16, tag="xzh", name="xzh", bufs=1)
            nc.sync.dma_start(
                out=xzh[:], in_=xh_in.rearrange("(t p) c -> p t c", p=P))
            sqt = wk.tile([P, NT, 3], F32, tag="sqt", name="sqt", bufs=1)
            nc.vector.tensor_tensor(out=sqt[:], in0=xzh[:, :, 0:3],
                                    in1=xzh[:, :, 0:3], op=ALU.mult)
            nc.vector.tensor_reduce(out=sq_pm[:], in_=sqt[:], axis=AX.X,
                                    op=ALU.add)
            # split |x|^2 into fp16 hi+lo halves (negated for the m matmul)
            hi16 = wk.tile([P, NT], F16, tag="hi16", name="hi16", bufs=1)
            nc.vector.tensor_copy(out=hi16[:], in_=sq_pm[:])
            hi32 = wk.tile([P, NT], F32, tag="hi32", name="hi32", bufs=1)
            nc.vector.tensor_copy(out=hi32[:], in_=hi16[:])
            lo32 = wk.tile([P, NT], F32, tag="lo32", name="lo32", bufs=1)
            nc.vector.tensor_tensor(out=lo32[:], in0=sq_pm[:], in1=hi32[:],
                                    op=ALU.subtract)
            lo16 = wk.tile([P, NT], F16, tag="lo16", name="lo16", bufs=1)
            nc.vector.tensor_copy(out=lo16[:], in_=lo32[:])
            nhi = wk.tile([P, NT], F16, tag="nhi", name="nhi", bufs=1)
            nc.scalar.activation(out=nhi[:], in_=hi16[:], func=AF.Copy,
                                 scale=-1.0)
            nlo = wk.tile([P, NT], F16, tag="nlo", name="nlo", bufs=1)
            nc.scalar.activation(out=nlo[:], in_=lo16[:], func=AF.Copy,
                                 scale=-1.0)

            nc.sync.dma_start(out=bigT[0:3, :], in_=xTh_in[0:3, :])
            nc.sync.dma_start(
                out=bigT[3:4, :].rearrange("r (t p) -> r t p", p=P),
                in_=nhi[:])
            nc.sync.dma_start(
                out=bigT[4:5, :].rearrange("r (t p) -> r t p", p=P),
                in_=nlo[:])

            for g in range(16):
                stage = wk.tile([P, 8, 128], F16, tag="tab0stage",
                                name="tab0stage")
                sl8 = slice(g * 8, (g + 1) * 8)
                nc.vector.tensor_copy(out=stage[:, :, 64:67],
                                      in_=xzh[:, sl8, 0:3])
                nc.vector.tensor_copy(out=stage[:, :, 67:68],
                                      in_=hi16[:, sl8].unsqueeze(2))
                nc.vector.tensor_copy(out=stage[:, :, 68:69],
                                      in_=lo16[:, sl8].unsqueeze(2))
                nc.vector.memset(stage[:, :, 69:128], 0.0)
                for cc in range(2):
                    c = g * 2 + cc
                    xc = wk.tile([6, 512], F16, tag="xc", name="xc", bufs=3)
                    nc.sync.dma_start(out=xc[:],
                                      in_=xTh_in[:, c * 512:(c + 1) * 512])
                    pe = mmtile()
                    nc.tensor.matmul(out=pe[:64, :], lhsT=embw[:], rhs=xc[:],
                                     start=True, stop=True)
                    nc.scalar.activation(out=f0TS[:, c * 512:(c + 1) * 512],
                                         in_=pe[:64, :], func=AF.Relu,
                                         bias=embb[:])
                    pt = psT.tile([P, 512], F16, tag="trans", name="trans")
                    for t4 in range(4):
                        sl = slice(c * 512 + t4 * 128, c * 512 + (t4 + 1) * 128)
                        nc.tensor.matmul(out=pt[:, t4 * 64:(t4 + 1) * 64],
                                         lhsT=f0TS[:, sl], rhs=identh[:64, :64],
                                         is_transpose=True, start=True,
                                         stop=True)
                    nc.scalar.activation(
                        out=stage[:, cc * 4:(cc + 1) * 4, 0:64],
                        in_=pt[:, 0:256].rearrange("p (j c) -> p j c", c=64),
                        func=AF.Copy)
                nc.sync.dma_start(
                    out=tab0_d.rearrange("(t p) c -> p t c", p=P)[:, sl8, :],
                    in_=stage[:])

        # ============ stage 2: SA1 ============
        with tc.tile_pool(name="sa1", bufs=2) as wk:
            nc.gpsimd.dma_gather(gq[:], tab0_d[:], i1sb[:], S1, S1, 128)
            nc.vector.tensor_tensor(out=sq1_pm[:], in0=gq[:, :, 67],
                                    in1=gq[:, :, 68], op=ALU.add)

            for qt in range(T1):
                sl1 = slice(qt * P, (qt + 1) * P)
                pQ3 = trans16(gq[:, qt, 64:67])
                pQb = trans16(gq[:, qt, 67:69])
                lhsq = wk.tile([5, P], F16, tag="lhsq", name="lhsq")
                nc.scalar.activation(out=lhsq[0:3, :], in_=pQ3[:],
                                     func=AF.Copy, scale=2.0)
                nc.sync.dma_start(out=lhsq[3:5, :], in_=ones2_d[:])
                nqb = wk.tile([4, 512], F16, tag="nqb", name="nqb")
                nc.scalar.activation(
                    out=nqb[0:3, :].rearrange("r (j q) -> r j q", q=P),
                    in_=pQ3[:].unsqueeze(1).to_broadcast([3, 4, P]),
                    func=AF.Copy, scale=-1.0)
                nc.sync.dma_start(out=nqb[3:4, :], in_=ones1_d[:])
                nc.scalar.activation(out=xyz1a[:, sl1], in_=pQ3[:],
                                     func=AF.Copy)
                nc.scalar.activation(out=sqn1[:, sl1], in_=pQb[:],
                                     func=AF.Copy, scale=-1.0)
                nc.scalar.activation(out=rhs2a[:, sl1], in_=pQ3[:],
                                     func=AF.Copy, scale=2.0)

                candV = wk.tile([P, 128], F32, tag="candV", name="candV")
                candI = wk.tile([P, 128], U16, tag="candI", name="candI")
                for c in range(16):
                    pm = bigtile()
                    for hh in range(2):
                        nc.tensor.matmul(
                            out=pm[:, hh * 512:(hh + 1) * 512], lhsT=lhsq[:],
                            rhs=bigT[:, c * 1024 + hh * 512:
                                     c * 1024 + (hh + 1) * 512],
                            start=True, stop=True)
                    nc.vector.max(out=candV[:, c * 8:(c + 1) * 8], in_=pm[:])
                    nc.vector.max_index(out=candI[:, c * 8:(c + 1) * 8],
                                        in_max=candV[:, c * 8:(c + 1) * 8],
                                        in_values=pm[:])
                nc.vector.tensor_tensor(out=candI[:], in0=candI[:], in1=offs[:],
                                        op=ALU.add)
                candVw = wk.tile([P, 128], F32, tag="candVw", name="candVw")
                nc.vector.tensor_copy(out=candVw[:], in_=candV[:])
                selV = wk.tile([P, K1], F32, tag="selV", name="selV")
                for r in range(4):
                    rs = slice(r * 8, (r + 1) * 8)
                    nc.vector.max(out=selV[:, rs], in_=candVw[:])
                    if r < 3:
                        nc.vector.match_replace(out=candVw[:],
                                                in_to_replace=selV[:, rs],
                                                in_values=candVw[:],
                                                imm_value=NEG)
                mask = wk.tile([P, 128], U8, tag="selmask", name="selmask")
                nc.vector.tensor_scalar(out=mask[:], in0=candV[:],
                                        scalar1=selV[:, 31:32], scalar2=None,
                                        op0=ALU.is_ge)
                candIf = wk.tile([P, 128], F32, tag="candIf", name="candIf")
                nc.vector.tensor_copy(out=candIf[:], in_=candI[:])
                arr = wk.tile([P, 128], F32, tag="selarr", name="selarr")
                nc.vector.memset(arr[:], -1.0)
                nc.vector.copy_predicated(out=arr[:], mask=mask[:],
                                          data=candIf[:])
                selIf = wk.tile([P, K1], F32, tag="selIf", name="selIf")
                for r in range(4):
                    rs = slice(r * 8, (r + 1) * 8)
                    nc.vector.max(out=selIf[:, rs], in_=arr[:])
                    if r < 3:
                        nc.vector.match_replace(out=arr[:],
                                                in_to_replace=selIf[:, rs],
                                                in_values=arr[:],
                                                imm_value=-1.0)
                nbr16 = wk.tile([P, K1], I16, tag="nbr16", name="nbr16")
                nc.vector.tensor_copy(out=nbr16[:], in_=selIf[:])
                idxt = wrap_idx(wk, nbr16[:], ib1_ds[qt], "idxt1")
                gn = wk.tile([P, K1, 128], F16, tag="gn", name="gn")
                for k in range(4):
                    nc.gpsimd.dma_gather(gn[:, k * 8:(k + 1) * 8, :],
                                         tab0_d[:],
                                         idxt[:, k * 64:(k + 1) * 64],
                                         1024, 1024, 128)

                acc = wk.tile([P, P], F32, tag="sa1acc", name="sa1acc")
                for c in range(8):
                    pg = psT.tile([P, 512], F16, tag="trans", name="trans")
                    for j in range(4):
                        nc.tensor.matmul(out=pg[0:67, j * 128:(j + 1) * 128],
                                         lhsT=gn[:, c * 4 + j, 0:67],
                                         rhs=identh[:], is_transpose=True,
                                         start=True, stop=True)
                    gtc = wk.tile([67, 512], F16, tag="gtc", name="gtc",
                                  bufs=3)
                    nc.scalar.activation(out=gtc[:], in_=pg[0:67, :],
                                         func=AF.Copy)
                    pz = mmtile()
                    nc.tensor.matmul(out=pz[:], lhsT=w1aug[:], rhs=gtc[:],
                                     start=True, stop=False)
                    nc.tensor.matmul(out=pz[:], lhsT=w1q[:], rhs=nqb[:, 0:512],
                                     start=False, stop=True)
                    h1 = wk.tile([P, 512], F16, tag="h1", name="h1", bufs=3)
                    nc.scalar.activation(out=h1[:], in_=pz[:], func=AF.Relu)
                    pz2 = mmtile()
                    nc.tensor.matmul(out=pz2[:], lhsT=w2sb[:], rhs=h1[:],
                                     start=True, stop=True)
                    red = wk.tile([P, P], F32, tag="sa1red", name="sa1red",
                                  bufs=3)
                    nc.vector.tensor_reduce(
                        out=red[:],
                        in_=pz2[:].rearrange("f (s q) -> f q s", q=P),
                        axis=AX.X, op=ALU.max)
                    if c == 0:
                        nc.vector.tensor_copy(out=acc[:], in_=red[:])
                    else:
                        nc.vector.tensor_tensor(out=acc[:], in0=acc[:],
                                                in1=red[:], op=ALU.max)
                nc.scalar.activation(out=feat1T[:, sl1], in_=acc[:],
                                     func=AF.Relu, bias=b2sb[:])

            stage1 = wk.tile([P, T1, 256], F16, tag="stage1", name="stage1",
                             bufs=1)
            for t in range(T1):
                pf = trans16(feat1T[:, t * P:(t + 1) * P])
                nc.scalar.activation(out=stage1[:, t, 0:128], in_=pf,
                                     func=AF.Copy)
            nc.vector.tensor_copy(out=stage1[:, :, 128:133],
                                  in_=gq[:, :, 64:69])
            nc.vector.memset(stage1[:, :, 133:256], 0.0)
            nc.sync.dma_start(out=tab1_d.rearrange("(t p) c -> p t c", p=P),
                              in_=stage1[:])

        # ============ stage 3: SA2 ============
        with tc.tile_pool(name="sa2", bufs=2) as wk:
            gq2 = wk.tile([P, T2, 256], F16, tag="gq2", name="gq2", bufs=1)
            nc.gpsimd.dma_gather(gq2[:], tab1_d[:], i2sb[:], S2, S2, 256)
            sq2_pm = wk.tile([P, T2], F32, tag="sq2_pm", name="sq2_pm", bufs=1)
            nc.vector.tensor_tensor(out=sq2_pm[:], in0=gq2[:, :, 131],
                                    in1=gq2[:, :, 132], op=ALU.add)

            feat2T = [cst.tile([P, S2], F16, tag=f"feat2T{h}",
                               name=f"feat2T{h}") for h in range(2)]
            for t2 in range(T2):
                sl2 = slice(t2 * P, (t2 + 1) * P)
                pQ2a = trans16(gq2[:, t2, 128:131])
                pQ2b = trans16(gq2[:, t2, 131:133])
                lhsqB = wk.tile([3, P], F16, tag="lhsqB", name="lhsqB")
                nc.scalar.activation(out=lhsqB[:], in_=pQ2a[:],
                                     func=AF.Copy, scale=2.0)
                nqb2 = wk.tile([4, 512], F16, tag="nqb2", name="nqb2")
                nc.scalar.activation(
                    out=nqb2[0:3, :].rearrange("r (j q) -> r j q", q=P),
                    in_=pQ2a[:].unsqueeze(1).to_broadcast([3, 4, P]),
                    func=AF.Copy, scale=-1.0)
                nc.sync.dma_start(out=nqb2[3:4, :], in_=ones1_d[:])
                nc.scalar.activation(out=rhsF2a[:, sl2], in_=pQ2a[:],
                                     func=AF.Copy, scale=2.0)
                nc.scalar.activation(out=rhsF2b[:, sl2], in_=pQ2b[:],
                                     func=AF.Copy, scale=-1.0)

                pm2 = bigtile()
                for hh in range(2):
                    hs = slice(hh * 512, (hh + 1) * 512)
                    nc.tensor.matmul(out=pm2[:, hs], lhsT=lhsqB[:],
                                     rhs=xyz1a[:, hs],
                                     start=True, stop=False)
                    nc.tensor.matmul(out=pm2[:, hs], lhsT=ones2[:],
                                     rhs=sqn1[:, hs],
                                     start=False, stop=True)
                selV2 = wk.tile([P, K2], F32, tag="selV2", name="selV2")
                selI2 = wk.tile([P, K2], U16, tag="selI2", name="selI2")
                for r in range(4):
                    rs = slice(r * 8, (r + 1) * 8)
                    nc.vector.max(out=selV2[:, rs], in_=pm2[:])
                    nc.vector.max_index(out=selI2[:, rs], in_max=selV2[:, rs],
                                        in_values=pm2[:])
                    if r < 3:
                        nc.vector.match_replace(out=pm2[:],
                                                in_to_replace=selV2[:, rs],
                                                in_values=pm2[:],
                                                imm_value=NEG)
                nbr2 = wk.tile([P, K2], I16, tag="nbr2", name="nbr2")
                nc.vector.tensor_copy(out=nbr2[:], in_=selI2[:])
                idxt2 = wrap_idx(wk, nbr2[:], ib2_ds[t2], "idxt2")
                gn2 = wk.tile([P, K2, 256], F16, tag="gn2", name="gn2")
                for k in range(4):
                    nc.gpsimd.dma_gather(gn2[:, k * 8:(k + 1) * 8, :],
                                         tab1_d[:],
                                         idxt2[:, k * 64:(k + 1) * 64],
                                         1024, 1024, 256)

                acc2 = [wk.tile([P, P], F32, tag=f"sa2acc{h}",
                                name=f"sa2acc{h}") for h in range(2)]
                for c in range(8):
                    pga = psT.tile([P, 512], F16, tag="trans", name="trans")
                    pgb = psT.tile([P, 512], F16, tag="trans", name="trans")
                    for j in range(4):
                        nc.tensor.matmul(out=pga[:, j * 128:(j + 1) * 128],
                                         lhsT=gn2[:, c * 4 + j, 0:128],
                                         rhs=identh[:], is_transpose=True,
                                         start=True, stop=True)
                        nc.tensor.matmul(out=pgb[0:3, j * 128:(j + 1) * 128],
                                         lhsT=gn2[:, c * 4 + j, 128:131],
                                         rhs=identh[:], is_transpose=True,
                                         start=True, stop=True)
                    gta = wk.tile([P, 512], F16, tag="gta", name="gta", bufs=3)
                    gtb = wk.tile([3, 512], F16, tag="gtb", name="gtb", bufs=3)
                    nc.scalar.activation(out=gta[:], in_=pga[:], func=AF.Copy)
                    nc.scalar.activation(out=gtb[:], in_=pgb[0:3, :],
                                         func=AF.Copy)
                    h1c = []
                    for h in range(2):
                        pz = mmtile()
                        nc.tensor.matmul(out=pz[:], lhsT=v1A[h][:], rhs=gta[:],
                                         start=True, stop=False)
                        nc.tensor.matmul(out=pz[:], lhsT=v1rel[h][:],
                                         rhs=gtb[:], start=False, stop=False)
                        nc.tensor.matmul(out=pz[:], lhsT=v1q[h][:],
                                         rhs=nqb2[:, 0:512],
                                         start=False, stop=True)
                        hh_ = wk.tile([P, 512], F16, tag=f"h1c{h}",
                                      name=f"h1c{h}", bufs=3)
                        nc.scalar.activation(out=hh_[:], in_=pz[:],
                                             func=AF.Relu)
                        h1c.append(hh_)
                    for h in range(2):
                        pz = mmtile()
                        nc.tensor.matmul(out=pz[:], lhsT=v2sb[0][h][:],
                                         rhs=h1c[0][:], start=True, stop=False)
                        nc.tensor.matmul(out=pz[:], lhsT=v2sb[1][h][:],
                                         rhs=h1c[1][:], start=False, stop=True)
                        red = wk.tile([P, P], F32, tag="sa2red",
                                      name="sa2red", bufs=3)
                        nc.vector.tensor_reduce(
                            out=red[:],
                            in_=pz[:].rearrange("f (s q) -> f q s", q=P),
                            axis=AX.X, op=ALU.max)
                        if c == 0:
                            nc.vector.tensor_copy(out=acc2[h][:], in_=red[:])
                        else:
                            nc.vector.tensor_tensor(out=acc2[h][:],
                                                    in0=acc2[h][:],
                                                    in1=red[:], op=ALU.max)
                for h in range(2):
                    nc.scalar.activation(out=feat2T[h][:, sl2],
                                         in_=acc2[h][:], func=AF.Relu,
                                         bias=c2sb[:, h:h + 1])

            stage2 = wk.tile([P, T2, 256], F16, tag="stage2", name="stage2",
                             bufs=1)
            for t2 in range(T2):
                for h in range(2):
                    pf = trans16(feat2T[h][:, t2 * P:(t2 + 1) * P])
                    nc.scalar.activation(
                        out=stage2[:, t2, h * 128:(h + 1) * 128], in_=pf,
                        func=AF.Copy)
            nc.sync.dma_start(out=tab2_d.rearrange("(t p) c -> p t c", p=P),
                              in_=stage2[:])

        # ============ stage 4: FP2 ============
        with tc.tile_pool(name="fp2", bufs=2) as wk:
            v8f = wk.tile([P, T1, 8], F32, tag="v8f", name="v8f", bufs=1)
            p8f = wk.tile([P, T1, 8], U16, tag="p8f", name="p8f", bufs=1)
            for qt in range(T1):
                pm3 = mmtile()
                nc.tensor.matmul(out=pm3[:, 0:S2],
                                 lhsT=xyz1a[:, qt * P:(qt + 1) * P],
                                 rhs=rhsF2a[:], start=True, stop=False)
                nc.tensor.matmul(out=pm3[:, 0:S2], lhsT=ones2[:],
                                 rhs=rhsF2b[:], start=False, stop=True)
                nc.vector.max(out=v8f[:, qt, :], in_=pm3[:, 0:S2])
                nc.vector.max_index(out=p8f[:, qt, :], in_max=v8f[:, qt, :],
                                    in_values=pm3[:, 0:S2])

            # inverse-distance weights
            d2f = wk.tile([P, T1, 3], F32, tag="d2f", name="d2f", bufs=1)
            nc.vector.tensor_tensor(
                out=d2f[:], in0=sq1_pm[:].unsqueeze(2).to_broadcast([P, T1, 3]),
                in1=v8f[:, :, 0:3], op=ALU.subtract)
            nc.scalar.activation(out=d2f[:], in_=d2f[:], func=AF.Relu)
            nc.scalar.activation(out=d2f[:], in_=d2f[:], func=AF.Sqrt)
            nc.vector.tensor_scalar_max(d2f[:], d2f[:], 1e-10)
            wn2 = wk.tile([P, T1, 3], F32, tag="wn2", name="wn2", bufs=1)
            nc.vector.reciprocal(out=wn2[:], in_=d2f[:])
            ws2 = wk.tile([P, T1], F32, tag="ws2", name="ws2", bufs=1)
            nc.vector.tensor_reduce(out=ws2[:], in_=wn2[:], axis=AX.X,
                                    op=ALU.add)
            nc.vector.reciprocal(out=ws2[:], in_=ws2[:])
            nc.vector.tensor_tensor(
                out=wn2[:], in0=wn2[:],
                in1=ws2[:].unsqueeze(2).to_broadcast([P, T1, 3]), op=ALU.mult)
            wn2h = wk.tile([P, T1, 3], F16, tag="wn2h", name="wn2h", bufs=1)
            nc.vector.tensor_copy(out=wn2h[:], in_=wn2[:])

            p3f = wk.tile([P, T1 * 3], I16, tag="p3f", name="p3f", bufs=1)
            nc.vector.tensor_copy(out=p3f[:].rearrange("p (t j) -> p t j", j=3),
                                  in_=p8f[:, :, 0:3])
            idxtf2 = wrap_idx(wk, p3f[:], ibf2_d, "idxtf2")
            gi2 = wk.tile([P, T1, 3, 256], F16, tag="gi2", name="gi2", bufs=1)
            gi2v = gi2[:].rearrange("p t j c -> p (t j) c")
            for k in range(3):
                nc.gpsimd.dma_gather(gi2v[:, k * 8:(k + 1) * 8, :], tab2_d[:],
                                     idxtf2[:, k * 64:(k + 1) * 64],
                                     1024, 1024, 256)
            # weighted 3-NN sum: t0*w0 + t1*w1 + t2*w2
            tmp0 = wk.tile([P, T1, 256], F16, tag="tmp0", name="tmp0", bufs=1)
            nc.vector.tensor_tensor(
                out=tmp0[:], in0=gi2[:, :, 0, :],
                in1=wn2h[:, :, 0:1].to_broadcast([P, T1, 256]), op=ALU.mult)
            tmp1 = wk.tile([P, T1, 256], F16, tag="tmp1", name="tmp1", bufs=1)
            nc.vector.tensor_tensor(
                out=tmp1[:], in0=gi2[:, :, 1, :],
                in1=wn2h[:, :, 1:2].to_broadcast([P, T1, 256]), op=ALU.mult)
            nc.vector.tensor_tensor(out=tmp0[:], in0=tmp0[:], in1=tmp1[:],
                                    op=ALU.add)
            nc.vector.tensor_tensor(
                out=tmp1[:], in0=gi2[:, :, 2, :],
                in1=wn2h[:, :, 2:3].to_broadcast([P, T1, 256]), op=ALU.mult)
            it2 = wk.tile([P, T1, 256], F16, tag="it2", name="it2", bufs=1)
            nc.vector.tensor_tensor(out=it2[:], in0=tmp0[:], in1=tmp1[:],
                                    op=ALU.add)
            itT2 = [wk.tile([P, S1], F16, tag=f"itT2{h}", name=f"itT2{h}",
                            bufs=1) for h in range(2)]
            for t in range(T1):
                for h in range(2):
                    pf = trans16(it2[:, t, h * 128:(h + 1) * 128])
                    nc.scalar.activation(out=itT2[h][:, t * P:(t + 1) * P],
                                         in_=pf, func=AF.Copy)
            for c in range(2):
                cs = slice(c * 512, (c + 1) * 512)
                pz = mmtile()
                nc.tensor.matmul(out=pz[:], lhsT=f2w1[0][:], rhs=feat1T[:, cs],
                                 start=True, stop=False)
                nc.tensor.matmul(out=pz[:], lhsT=f2w1[1][:], rhs=itT2[0][:, cs],
                                 start=False, stop=False)
                nc.tensor.matmul(out=pz[:], lhsT=f2w1[2][:], rhs=itT2[1][:, cs],
                                 start=False, stop=True)
                hf = wk.tile([P, 512], F16, tag="fp2h", name="fp2h", bufs=3)
                nc.scalar.activation(out=hf[:], in_=pz[:], func=AF.Relu,
                                     bias=f2b1[:])
                pz2 = mmtile()
                nc.tensor.matmul(out=pz2[:], lhsT=f2w2[:], rhs=hf[:],
                                 start=True, stop=True)
                nc.scalar.activation(out=f1upT[:, cs], in_=pz2[:], func=AF.Relu,
                                     bias=f2b2[:])
            stagef = wk.tile([P, T1, 128], F16, tag="stagef", name="stagef",
                             bufs=1)
            for t in range(T1):
                pf = trans16(f1upT[:, t * P:(t + 1) * P])
                nc.scalar.activation(out=stagef[:, t, :], in_=pf, func=AF.Copy)
            nc.sync.dma_start(out=tabf_d.rearrange("(t p) c -> p t c", p=P),
                              in_=stagef[:])

        # ============ stage 5: FP1 (16 pipelined groups) ============
        with tc.tile_pool(name="fp1", bufs=2) as wk:
            for g in range(NG):
                v81 = wk.tile([P, GT, 8], F32, tag="v81", name="v81")
                p81 = wk.tile([P, GT, 8], U16, tag="p81", name="p81")
                for j in range(GT):
                    qt = g * GT + j
                    pm4 = bigtile()
                    for hh in range(2):
                        hs = slice(hh * 512, (hh + 1) * 512)
                        nc.tensor.matmul(out=pm4[:, hs],
                                         lhsT=bigT[0:3, qt * P:(qt + 1) * P],
                                         rhs=rhs2a[:, hs],
                                         start=True, stop=False)
                        nc.tensor.matmul(out=pm4[:, hs], lhsT=ones2[:],
                                         rhs=sqn1[:, hs],
                                         start=False, stop=True)
                    nc.vector.max(out=v81[:, j, :], in_=pm4[:])
                    nc.vector.max_index(out=p81[:, j, :], in_max=v81[:, j, :],
                                        in_values=pm4[:])

                sqs = sq_pm[:, g * GT:(g + 1) * GT]
                d21 = wk.tile([P, GT, 3], F32, tag="d21", name="d21")
                nc.vector.tensor_tensor(
                    out=d21[:], in0=sqs.unsqueeze(2).to_broadcast([P, GT, 3]),
                    in1=v81[:, :, 0:3], op=ALU.subtract)
                nc.scalar.activation(out=d21[:], in_=d21[:], func=AF.Relu)
                nc.scalar.activation(out=d21[:], in_=d21[:], func=AF.Sqrt)
                nc.vector.tensor_scalar_max(d21[:], d21[:], 1e-10)
                wn1 = wk.tile([P, GT, 3], F32, tag="wn1", name="wn1")
                nc.vector.reciprocal(out=wn1[:], in_=d21[:])
                ws1 = wk.tile([P, GT], F32, tag="ws1", name="ws1")
                nc.vector.tensor_reduce(out=ws1[:], in_=wn1[:], axis=AX.X,
                                        op=ALU.add)
                nc.vector.reciprocal(out=ws1[:], in_=ws1[:])
                nc.vector.tensor_tensor(
                    out=wn1[:], in0=wn1[:],
                    in1=ws1[:].unsqueeze(2).to_broadcast([P, GT, 3]),
                    op=ALU.mult)
                wn1h = wk.tile([P, GT, 3], F16, tag="wn1h", name="wn1h")
                nc.vector.tensor_copy(out=wn1h[:], in_=wn1[:])

                p31 = wk.tile([P, GT * 3], I16, tag="p31", name="p31")
                nc.vector.tensor_copy(
                    out=p31[:].rearrange("p (t j) -> p t j", j=3),
                    in_=p81[:, :, 0:3])
                idxtf1 = wrap_idx(wk, p31[:], ibf1_ds[g], "idxtf1")
                gi1 = wk.tile([P, GT, 3, 128], F16, tag="gi1", name="gi1")
                gi1v = gi1[:].rearrange("p t j c -> p (t j) c")
                for k in range(3):
                    nc.gpsimd.dma_gather(gi1v[:, k * 8:(k + 1) * 8, :],
                                         tabf_d[:],
                                         idxtf1[:, k * 64:(k + 1) * 64],
                                         1024, 1024, 128)
                ta = wk.tile([P, GT, 128], F16, tag="ta", name="ta")
                nc.vector.tensor_tensor(
                    out=ta[:], in0=gi1[:, :, 0, :],
                    in1=wn1h[:, :, 0:1].to_broadcast([P, GT, 128]),
                    op=ALU.mult)
                tb = wk.tile([P, GT, 128], F16, tag="tb", name="tb")
                nc.vector.tensor_tensor(
                    out=tb[:], in0=gi1[:, :, 1, :],
                    in1=wn1h[:, :, 1:2].to_broadcast([P, GT, 128]),
                    op=ALU.mult)
                nc.vector.tensor_tensor(out=ta[:], in0=ta[:], in1=tb[:],
                                        op=ALU.add)
                nc.vector.tensor_tensor(
                    out=tb[:], in0=gi1[:, :, 2, :],
                    in1=wn1h[:, :, 2:3].to_broadcast([P, GT, 128]),
                    op=ALU.mult)
                it1 = wk.tile([P, GT, 128], F16, tag="it1", name="it1")
                nc.vector.tensor_tensor(out=it1[:], in0=ta[:], in1=tb[:],
                                        op=ALU.add)
                itT1 = wk.tile([P, GT * 128], F16, tag="itT1", name="itT1")
                for t in range(GT):
                    pf = trans16(it1[:, t, :])
                    nc.scalar.activation(out=itT1[:, t * P:(t + 1) * P],
                                         in_=pf, func=AF.Copy)
                for c in range(2):
                    cs = slice(c * 512, (c + 1) * 512)
                    gcs = slice(g * GT * P + c * 512, g * GT * P + (c + 1) * 512)
                    pz = mmtile()
                    nc.tensor.matmul(out=pz[:], lhsT=f1w1b[:], rhs=itT1[:, cs],
                                     start=True, stop=False)
                    nc.tensor.matmul(out=pz[:], lhsT=f1w1a[:],
                                     rhs=f0TS[:, gcs], start=False, stop=True)
                    hf = wk.tile([P, 512], F16, tag="fp1h", name="fp1h",
                                 bufs=3)
                    nc.scalar.activation(out=hf[:], in_=pz[:], func=AF.Relu,
                                         bias=f1b1[:])
                    pz2 = mmtile()
                    nc.tensor.matmul(out=pz2[:], lhsT=f1w2[:], rhs=hf[:],
                                     start=True, stop=True)
                    nc.scalar.activation(out=fuTS[:, gcs], in_=pz2[:],
                                         func=AF.Relu, bias=f1b2[:])
                    red = wk.tile([P, 1], F32, tag="gfred", name="gfred",
                                  bufs=3)
                    nc.vector.tensor_reduce(out=red[:], in_=fuTS[:, gcs],
                                            axis=AX.X, op=ALU.max)
                    if g == 0 and c == 0:
                        nc.vector.tensor_copy(out=gfacc[:], in_=red[:])
                    else:
                        nc.vector.tensor_tensor(out=gfacc[:], in0=gfacc[:],
                                                in1=red[:], op=ALU.max)

        # ============ stage 6: head ============
        with tc.tile_pool(name="head", bufs=2) as wk:
            gfh = wk.tile([P, 1], F16, tag="gfh", name="gfh", bufs=1)
            nc.vector.tensor_copy(out=gfh[:], in_=gfacc[:])
            pc = mmtile()
            nc.tensor.matmul(out=pc[:, 0:1], lhsT=hw1b[:], rhs=gfh[:],
                             start=True, stop=True)
            nc.vector.tensor_tensor(out=biasH[:], in0=pc[:, 0:1], in1=hb1[:],
                                    op=ALU.add)
            for g in range(8):
                ostage = wk.tile([P, 16, 13], F32, tag="ostage", name="ostage")
                for c4 in range(4):
                    c = g * 4 + c4
                    cs = slice(c * 512, (c + 1) * 512)
                    pz = mmtile()
                    nc.tensor.matmul(out=pz[:], lhsT=hw1a[:], rhs=fuTS[:, cs],
                                     start=True, stop=True)
                    h1 = wk.tile([P, 512], F16, tag="hh1", name="hh1", bufs=3)
                    nc.scalar.activation(out=h1[:], in_=pz[:], func=AF.Relu,
                                         bias=biasH[:])
                    pz2 = mmtile()
                    nc.tensor.matmul(out=pz2[:64, :], lhsT=hw2[:], rhs=h1[:],
                                     start=True, stop=True)
                    h2 = wk.tile([64, 512], F16, tag="hh2", name="hh2", bufs=3)
                    nc.scalar.activation(out=h2[:], in_=pz2[:64, :],
                                         func=AF.Relu, bias=hb2[:])
                    pz3 = mmtile()
                    nc.tensor.matmul(out=pz3[:13, :], lhsT=hw3[:], rhs=h2[:],
                                     start=True, stop=True)
                    oT = wk.tile([13, 512], F16, tag="hoT", name="hoT", bufs=3)
                    nc.vector.tensor_tensor(
                        out=oT[:], in0=pz3[:13, :],
                        in1=hb3[:, 0:1].to_broadcast([13, 512]), op=ALU.add)
                    po = psT.tile([P, 512], F16, tag="trans", name="trans")
                    for t in range(4):
                        nc.tensor.matmul(out=po[:, t * 16:t * 16 + 13],
                                         lhsT=oT[:, t * 128:(t + 1) * 128],
                                         rhs=identh[0:13, 0:13],
                                         is_transpose=True, start=True,
                                         stop=True)
                    nc.scalar.activation(
                        out=ostage[:, c4 * 4:(c4 + 1) * 4, :],
                        in_=po[:, 0:64].rearrange(
                            "p (t c) -> p t c", c=16)[:, :, 0:13],
                        func=AF.Copy)
                nc.sync.dma_start(
                    out=out_d.rearrange("(t p) c -> p t c", p=P)[
                        :, g * 16:(g + 1) * 16, :],
                    in_=ostage[:])

    return nc


# ---------------------------------------------------------------- host side
_CACHED_NC = None


def _get_nc():
    global _CACHED_NC
    if _CACHED_NC is None:
        nc = build_nc()
        nc.finalize()
        _CACHED_NC = nc
    return _CACHED_NC


def _per_core_inputs(b, inputs):
    x = np.asarray(inputs["x"][b]).astype(np.float16)
    i1 = np.asarray(inputs["idx_s1"][b]).astype(np.int16)
    i1w = np.tile(i1.reshape(S1 // 16, 16).T, (8, 1))
    i2 = np.asarray(inputs["idx_s2"][b]).astype(np.int16)
    i2w = np.tile(i2.reshape(S2 // 16, 16).T, (8, 1))
    f16 = lambda a: np.ascontiguousarray(np.asarray(a), dtype=np.float16)
    f32 = lambda a: np.ascontiguousarray(np.asarray(a), dtype=np.float32)
    return {
        "xh": np.ascontiguousarray(x),
        "xTh": np.ascontiguousarray(x.T),
        "i1w": np.ascontiguousarray(i1w),
        "i2w": np.ascontiguousarray(i2w),
        "embw": f16(inputs["embed_w"]),
        "embb": f32(inputs["embed_b"]).reshape(64, 1),
        "w1": f16(inputs["sa1_w1"]),
        "b1r": f16(inputs["sa1_b1"]).reshape(1, 128),
        "w2": f16(inputs["sa1_w2"]),
        "b2": f32(inputs["sa1_b2"]).reshape(128, 1),
        "v1": f16(inputs["sa2_w1"]),
        "c1r": f16(inputs["sa2_b1"]).reshape(1, 256),
        "v2": f16(inputs["sa2_w2"]),
        "c2": np.ascontiguousarray(f32(inputs["sa2_b2"]).reshape(2, 128).T),
        "f2w1": f16(inputs["fp2_w1"]),
        "f2b1": f32(inputs["fp2_b1"]).reshape(128, 1),
        "f2w2": f16(inputs["fp2_w2"]),
        "f2b2": f32(inputs["fp2_b2"]).reshape(128, 1),
        "f1w1": f16(inputs["fp1_w1"]),
        "f1b1": f32(inputs["fp1_b1"]).reshape(128, 1),
        "f1w2": f16(inputs["fp1_w2"]),
        "f1b2": f32(inputs["fp1_b2"]).reshape(128, 1),
        "hw1": f16(inputs["head_w1"]),
        "hb1": f32(inputs["head_b1"]).reshape(128, 1),
        "hw2": f16(inputs["head_w2"]),
        "hb2": f32(inputs["head_b2"]).reshape(64, 1),
        "hw3": f16(inputs["head_w3"]),
        "hb3": f32(inputs["head_b3"]).reshape(13, 1),
    }


def run(inputs, trace=False, **kw):
    nc = _get_nc()
    B = inputs["x"].shape[0]
    in_maps = [_per_core_inputs(b, inputs) for b in range(B)]
    res = run_bass_kernel_spmd(nc, in_maps, core_ids=list(range(B)),
                               trace=trace, **kw)
    out = np.stack([res.results[b]["out"] for b in range(B)])
    return out, res


def kernel(**inputs):
    return run(inputs)[0]


if __name__ == "__main__":
    build_nc()
    print("built ok")


# revision 13
# speedup vs baseline: 1.6383x; 1.1635x over previous
"""PointNet++-lite segmentation on 8 Trainium2 cores (batch-parallel, one
point cloud per core). Self-contained: hardcodes shapes from the problem spec.

Per-core pipeline (all on device):
  embed MLP -> SA1 (KNN top-32 of 16384, gather, 2-layer MLP, max-pool)
  -> SA2 (KNN top-32 of 1024) -> FP2/FP1 (3-NN inverse-distance interp)
  -> global-max head MLP -> (16384, 13) logits.

fp16 datapath: all PE matmuls run on fp16 operands (4x the fp32 rate), with
fp32 PSUM accumulation.  KNN ranking uses m = 2 q.x - |x|^2 (row-constant
|q|^2 dropped); |x|^2 enters the fp16 matmul split into hi+lo fp16 halves so
m keeps ~22 mantissa bits (self-distances stay ~0, exact inverse-distance
weights).  Neighbor tables are fp16 rows in DRAM; SA1/SA2 gathers use
dma_gather transpose mode which lands features on partitions, removing the
per-neighbor PE transposes.  Selection runs on the vector engine max8 /
max_index over 1024-wide PSUM blocks.
"""

from contextlib import ExitStack

import numpy as np

import concourse.bass as bass
import concourse.mybir as mybir
from concourse.bacc import Bacc
from concourse.bass_utils import run_bass_kernel_spmd
from concourse.masks import make_identity
from concourse.tile import TileContext

F32 = mybir.dt.float32
F16 = mybir.dt.float16
U16 = mybir.dt.uint16
U8 = mybir.dt.uint8
I16 = mybir.dt.int16
AF = mybir.ActivationFunctionType
ALU = mybir.AluOpType
AX = mybir.AxisListType

P = 128
N = 16384
S1, K1 = 1024, 32
S2, K2 = 256, 32
NCLS = 13
NEG = -3.0e38

NT = N // P        # 128 point tiles
T1 = S1 // P       # 8 SA1 query tiles
T2 = S2 // P       # 2 SA2 query tiles
NG = 16            # FP1 groups (8 tiles each)
GT = NT // NG      # tiles per FP1 group


def build_nc():
    nc = Bacc()

    xh_in = nc.dram_tensor("xh", [N, 6], F16, kind="ExternalInput")
    xTh_in = nc.dram_tensor("xTh", [6, N], F16, kind="ExternalInput")
    i1_in = nc.dram_tensor("i1w", [P, S1 // 16], I16, kind="ExternalInput")
    i2_in = nc.dram_tensor("i2w", [P, S2 // 16], I16, kind="ExternalInput")
    wdecl16 = [
        ("embw", [6, 64]),
        ("w1", [67, 128]), ("b1r", [1, 128]), ("w2", [128, 128]),
        ("v1", [131, 256]), ("c1r", [1, 256]), ("v2", [256, 256]),
        ("f2w1", [384, 128]), ("f2w2", [128, 128]),
        ("f1w1", [192, 128]), ("f1w2", [128, 128]),
        ("hw1", [256, 128]), ("hw2", [128, 64]), ("hw3", [64, 13]),
    ]
    wdecl32 = [
        ("embb", [64, 1]), ("b2", [128, 1]), ("c2", [128, 2]),
        ("f2b1", [128, 1]), ("f2b2", [128, 1]),
        ("f1b1", [128, 1]), ("f1b2", [128, 1]),
        ("hb1", [128, 1]), ("hb2", [64, 1]), ("hb3", [13, 1]),
    ]
    din = {nm: nc.dram_tensor(nm, sh, F16, kind="ExternalInput")
           for nm, sh in wdecl16}
    din.update({nm: nc.dram_tensor(nm, sh, F32, kind="ExternalInput")
                for nm, sh in wdecl32})
    out_d = nc.dram_tensor("out", [N, NCLS], F32, kind="ExternalOutput")

    tab0_d = nc.dram_tensor("tab0", [N, 128], F16)
    tab1_d = nc.dram_tensor("tab1", [S1, 256], F16)
    tab2_d = nc.dram_tensor("tab2", [S2, 256], F16)
    tabf_d = nc.dram_tensor("tabf", [S1, 128], F16)
    ib1_ds = [nc.dram_tensor(f"ib1_{t}", [16, 256], I16) for t in range(T1)]
    ib2_ds = [nc.dram_tensor(f"ib2_{t}", [16, 256], I16) for t in range(T2)]
    ibf2_d = nc.dram_tensor("ibf2", [16, 192], I16)
    ones2_d = nc.dram_tensor("ones2_d", [2, 128], F16)
    ones1_d = nc.dram_tensor("ones1_d", [1, 512], F16)
    ibf1_ds = [nc.dram_tensor(f"ibf1_{g}", [16, 192], I16) for g in range(NG)]

    with TileContext(nc) as tc, ExitStack() as ctx:
        cst = ctx.enter_context(tc.tile_pool(name="cst", bufs=1))
        psB = ctx.enter_context(tc.tile_pool(name="psB", bufs=2, space="PSUM"))
        psA = ctx.enter_context(tc.tile_pool(name="psA", bufs=2, space="PSUM"))
        psT = ctx.enter_context(tc.tile_pool(name="psT", bufs=2, space="PSUM"))

        identh = cst.tile([P, P], F16, tag="identh", name="identh")
        make_identity(nc, identh[:])

        def bigtile():
            return psB.tile([P, 1024], F32, tag="big", name="big")

        def mmtile():
            return psA.tile([P, 512], F32, tag="mm", name="mm")

        def trans16(in_ap):
            """PE transpose of fp16 data: in_(p,f) -> fp16 psum (f,p)."""
            pt = psT.tile([P, 512], F16, tag="trans", name="trans")
            k = in_ap.shape[0]
            f = in_ap.shape[-1]
            nc.tensor.matmul(out=pt[:f, :k], lhsT=in_ap, rhs=identh[:k, :k],
                             is_transpose=True, start=True, stop=True)
            return pt[:f, :k]


        # ---------------- constants / weights ----------------
        def load(name, src, shape, dtype=F16):
            t = cst.tile(list(shape), dtype, tag=name, name=name)
            nc.sync.dma_start(out=t[:], in_=src)
            return t

        embw = load("embw", din["embw"][:], (6, 64))
        embb = load("embb", din["embb"][:], (64, 1), F32)
        w2sb = load("w2sb", din["w2"][:], (128, 128))
        b2sb = load("b2sb", din["b2"][:], (128, 1), F32)
        c2sb = load("c2sb", din["c2"][:], (128, 2), F32)
        f2b1 = load("f2b1", din["f2b1"][:], (128, 1), F32)
        f2w2 = load("f2w2", din["f2w2"][:], (128, 128))
        f2b2 = load("f2b2", din["f2b2"][:], (128, 1), F32)
        f1b1 = load("f1b1", din["f1b1"][:], (128, 1), F32)
        f1w2 = load("f1w2", din["f1w2"][:], (128, 128))
        f1b2 = load("f1b2", din["f1b2"][:], (128, 1), F32)
        hb1 = load("hb1", din["hb1"][:], (128, 1), F32)
        hw2 = load("hw2", din["hw2"][:], (128, 64))
        hb2 = load("hb2", din["hb2"][:], (64, 1), F32)
        hw3 = load("hw3", din["hw3"][:], (64, 13))
        hb3 = load("hb3", din["hb3"][:], (13, 1), F32)

        # SA1 grouped-MLP weights: rows [feat(64), rel_xyz(3)]
        w1aug = cst.tile([67, 128], F16, tag="w1aug", name="w1aug")
        nc.sync.dma_start(out=w1aug[0:64, :], in_=din["w1"][3:67, :])
        nc.sync.dma_start(out=w1aug[64:67, :], in_=din["w1"][0:3, :])
        w1q = cst.tile([4, 128], F16, tag="w1q", name="w1q")
        nc.sync.dma_start(out=w1q[0:3, :], in_=din["w1"][0:3, :])
        nc.sync.dma_start(out=w1q[3:4, :], in_=din["b1r"][:])

        v1A = [load(f"v1A{h}", din["v1"][3:131, h * 128:(h + 1) * 128],
                    (128, 128)) for h in range(2)]
        v1rel, v1q = [], []
        for h in range(2):
            sl = slice(h * 128, (h + 1) * 128)
            t = cst.tile([3, 128], F16, tag=f"v1rel{h}", name=f"v1rel{h}")
            nc.sync.dma_start(out=t[:], in_=din["v1"][0:3, sl])
            v1rel.append(t)
            t2 = cst.tile([4, 128], F16, tag=f"v1q{h}", name=f"v1q{h}")
            nc.sync.dma_start(out=t2[0:3, :], in_=din["v1"][0:3, sl])
            nc.sync.dma_start(out=t2[3:4, :], in_=din["c1r"][0:1, sl])
            v1q.append(t2)
        v2sb = [[load(f"v2{k}{h}",
                      din["v2"][k * 128:(k + 1) * 128, h * 128:(h + 1) * 128],
                      (128, 128)) for h in range(2)] for k in range(2)]
        f2w1 = [load(f"f2w1{k}", din["f2w1"][k * 128:(k + 1) * 128, :],
                     (128, 128)) for k in range(3)]
        f1w1a = load("f1w1a", din["f1w1"][0:64, :], (64, 128))
        f1w1b = load("f1w1b", din["f1w1"][64:192, :], (128, 128))
        hw1a = load("hw1a", din["hw1"][0:128, :], (128, 128))
        hw1b = load("hw1b", din["hw1"][128:256, :], (128, 128))
        i1sb = load("i1sb", i1_in[:], (P, S1 // 16), I16)
        i2sb = load("i2sb", i2_in[:], (P, S2 // 16), I16)

        # block offsets for 1024-wide max8 blocks: j*1024, 8 copies each
        offs = cst.tile([P, 128], U16, tag="offs", name="offs")
        nc.gpsimd.iota(out=offs[:], pattern=[[1024, 16], [0, 8]], base=0,
                       channel_multiplier=0)

        ones2 = cst.tile([2, P], F16, tag="ones2", name="ones2")
        nc.vector.memset(ones2[:], 1.0)
        nc.sync.dma_start(out=ones2_d[:], in_=ones2[:])
        onesr = cst.tile([1, 512], F16, tag="onesr", name="onesr")
        nc.vector.memset(onesr[:], 1.0)
        nc.sync.dma_start(out=ones1_d[:], in_=onesr[:])

        def wrap_idx(pool, src_i16, bounce_d, tag):
            """src (128, M) i16 -> replicated wrapped idx tile (128, 8*M)
            via DRAM bounce.  Flat gather slot j*128+q reads src[q, j]."""
            M = src_i16.shape[-1]
            bw = bounce_d[:, :].rearrange("c (j e) -> c j e", e=8)
            for ph in range(8):
                nc.sync.dma_start(out=bw[:, 0:M, ph],
                                  in_=src_i16[ph * 16:(ph + 1) * 16, :])
            idxt = pool.tile([P, 8 * M], I16, tag=tag, name=tag)
            for g in range(8):
                nc.sync.dma_start(out=idxt[g * 16:(g + 1) * 16, :],
                                  in_=bounce_d[:, :])
            return idxt

        # persistent cross-stage tensors
        sq_pm = cst.tile([P, NT], F32, tag="sq_pm", name="sq_pm")
        bigT = cst.tile([5, N], F16, tag="bigT", name="bigT")
        f0TS = cst.tile([64, N], F16, tag="f0TS", name="f0TS")
        fuTS = cst.tile([P, N], F16, tag="fuTS", name="fuTS")
        gq = cst.tile([P, T1, 128], F16, tag="gq", name="gq")
        xyz1a = cst.tile([3, S1], F16, tag="xyz1a", name="xyz1a")
        sqn1 = cst.tile([2, S1], F16, tag="sqn1", name="sqn1")
        rhs2a = cst.tile([3, S1], F16, tag="rhs2a", name="rhs2a")
        rhsF2a = cst.tile([3, S2], F16, tag="rhsF2a", name="rhsF2a")
        rhsF2b = cst.tile([2, S2], F16, tag="rhsF2b", name="rhsF2b")
        sq1_pm = cst.tile([P, T1], F32, tag="sq1_pm", name="sq1_pm")
        feat1T = cst.tile([P, S1], F16, tag="feat1T", name="feat1T")
        f1upT = cst.tile([P, S1], F16, tag="f1upT", name="f1upT")
        gfacc = cst.tile([P, 1], F32, tag="gfacc", name="gfacc")
        biasH = cst.tile([P, 1], F32, tag="biasH", name="biasH")

        # ============ stage 0+1: geometry, embed, tab0 ============
        with tc.tile_pool(name="st01", bufs=2) as wk:
            xzh = wk.tile([P, NT, 6], F16, tag="xzh", name="xzh", bufs=1)
            nc.sync.dma_start(
                out=xzh[:], in_=xh_in.rearrange("(t p) c -> p t c", p=P))
            sqt = wk.tile([P, NT, 3], F32, tag="sqt", name="sqt", bufs=1)
            nc.vector.tensor_tensor(out=sqt[:], in0=xzh[:, :, 0:3],
                                    in1=xzh[:, :, 0:3], op=ALU.mult)
            nc.vector.tensor_reduce(out=sq_pm[:], in_=sqt[:], axis=AX.X,
                                    op=ALU.add)
            # split |x|^2 into fp16 hi+lo halves (negated for the m matmul)
            hi16 = wk.tile([P, NT], F16, tag="hi16", name="hi16", bufs=1)
            nc.vector.tensor_copy(out=hi16[:], in_=sq_pm[:])
            hi32 = wk.tile([P, NT], F32, tag="hi32", name="hi32", bufs=1)
            nc.vector.tensor_copy(out=hi32[:], in_=hi16[:])
            lo32 = wk.tile([P, NT], F32, tag="lo32", name="lo32", bufs=1)
            nc.vector.tensor_tensor(out=lo32[:], in0=sq_pm[:], in1=hi32[:],
                                    op=ALU.subtract)
            lo16 = wk.tile([P, NT], F16, tag="lo16", name="lo16", bufs=1)
            nc.vector.tensor_copy(out=lo16[:], in_=lo32[:])
            nhi = wk.tile([P, NT], F16, tag="nhi", name="nhi", bufs=1)
            nc.scalar.activation(out=nhi[:], in_=hi16[:], func=AF.Copy,
                                 scale=-1.0)
            nlo = wk.tile([P, NT], F16, tag="nlo", name="nlo", bufs=1)
            nc.scalar.activation(out=nlo[:], in_=lo16[:], func=AF.Copy,
                                 scale=-1.0)

            nc.sync.dma_start(out=bigT[0:3, :], in_=xTh_in[0:3, :])
            nc.sync.dma_start(
                out=bigT[3:4, :].rearrange("r (t p) -> r t p", p=P),
                in_=nhi[:])
            nc.sync.dma_start(
                out=bigT[4:5, :].rearrange("r (t p) -> r t p", p=P),
                in_=nlo[:])

            for g in range(16):
                stage = wk.tile([P, 8, 128], F16, tag="tab0stage",
                                name="tab0stage")
                sl8 = slice(g * 8, (g + 1) * 8)
                nc.vector.tensor_copy(out=stage[:, :, 64:67],
                                      in_=xzh[:, sl8, 0:3])
                nc.vector.tensor_copy(out=stage[:, :, 67:68],
                                      in_=hi16[:, sl8].unsqueeze(2))
                nc.vector.tensor_copy(out=stage[:, :, 68:69],
                                      in_=lo16[:, sl8].unsqueeze(2))
                nc.vector.memset(stage[:, :, 69:128], 0.0)
                for cc in range(2):
                    c = g * 2 + cc
                    xc = wk.tile([6, 512], F16, tag="xc", name="xc", bufs=3)
                    nc.sync.dma_start(out=xc[:],
                                      in_=xTh_in[:, c * 512:(c + 1) * 512])
                    pe = mmtile()
                    nc.tensor.matmul(out=pe[:64, :], lhsT=embw[:], rhs=xc[:],
                                     start=True, stop=True)
                    nc.scalar.activation(out=f0TS[:, c * 512:(c + 1) * 512],
                                         in_=pe[:64, :], func=AF.Relu,
                                         bias=embb[:])
                    pt = psT.tile([P, 512], F16, tag="trans", name="trans")
                    for t4 in range(4):
                        sl = slice(c * 512 + t4 * 128, c * 512 + (t4 + 1) * 128)
                        nc.tensor.matmul(out=pt[:, t4 * 64:(t4 + 1) * 64],
                                         lhsT=f0TS[:, sl], rhs=identh[:64, :64],
                                         is_transpose=True, start=True,
                                         stop=True)
                    nc.scalar.activation(
                        out=stage[:, cc * 4:(cc + 1) * 4, 0:64],
                        in_=pt[:, 0:256].rearrange("p (j c) -> p j c", c=64),
                        func=AF.Copy)
                nc.sync.dma_start(
                    out=tab0_d.rearrange("(t p) c -> p t c", p=P)[:, sl8, :],
                    in_=stage[:])

        # ============ stage 2: SA1 ============
        with tc.tile_pool(name="sa1", bufs=2) as wk:
            nc.gpsimd.dma_gather(gq[:], tab0_d[:], i1sb[:], S1, S1, 128)
            nc.vector.tensor_tensor(out=sq1_pm[:], in0=gq[:, :, 67],
                                    in1=gq[:, :, 68], op=ALU.add)
            qxyz = cst.tile([3, S1], F16, tag="qxyz", name="qxyz")
            qsq = cst.tile([2, S1], F16, tag="qsq", name="qsq")
            for r in range(3):
                nc.sync.dma_start(
                    out=qxyz[r:r + 1, :].rearrange("r (t q) -> r t q", q=P),
                    in_=gq[:, :, 64 + r])
            for r in range(2):
                nc.sync.dma_start(
                    out=qsq[r:r + 1, :].rearrange("r (t q) -> r t q", q=P),
                    in_=gq[:, :, 67 + r])
            nc.scalar.activation(out=xyz1a[:], in_=qxyz[:], func=AF.Copy)
            nc.scalar.activation(out=sqn1[:], in_=qsq[:], func=AF.Copy,
                                 scale=-1.0)
            nc.scalar.activation(out=rhs2a[:], in_=qxyz[:], func=AF.Copy,
                                 scale=2.0)

            def sa1_select(qt):
                sl1 = slice(qt * P, (qt + 1) * P)
                lhsq = wk.tile([5, P], F16, tag="lhsq", name="lhsq")
                nc.scalar.activation(out=lhsq[0:3, :], in_=qxyz[:, sl1],
                                     func=AF.Copy, scale=2.0)
                nc.sync.dma_start(out=lhsq[3:5, :], in_=ones2_d[:])
                nqb = wk.tile([4, 512], F16, tag="nqb", name="nqb")
                nc.scalar.activation(
                    out=nqb[0:3, :].rearrange("r (j q) -> r j q", q=P),
                    in_=qxyz[:, sl1].unsqueeze(1).to_broadcast([3, 4, P]),
                    func=AF.Copy, scale=-1.0)
                nc.sync.dma_start(out=nqb[3:4, :], in_=ones1_d[:])

                candV = wk.tile([P, 128], F32, tag="candV", name="candV")
                candI = wk.tile([P, 128], U16, tag="candI", name="candI")
                for c in range(16):
                    pm = bigtile()
                    for hh in range(2):
                        nc.tensor.matmul(
                            out=pm[:, hh * 512:(hh + 1) * 512], lhsT=lhsq[:],
                            rhs=bigT[:, c * 1024 + hh * 512:
                                     c * 1024 + (hh + 1) * 512],
                            start=True, stop=True)
                    nc.vector.max(out=candV[:, c * 8:(c + 1) * 8], in_=pm[:])
                    nc.vector.max_index(out=candI[:, c * 8:(c + 1) * 8],
                                        in_max=candV[:, c * 8:(c + 1) * 8],
                                        in_values=pm[:])
                nc.vector.tensor_tensor(out=candI[:], in0=candI[:], in1=offs[:],
                                        op=ALU.add)
                candVw = wk.tile([P, 128], F32, tag="candVw", name="candVw")
                nc.vector.tensor_copy(out=candVw[:], in_=candV[:])
                selV = wk.tile([P, K1], F32, tag="selV", name="selV")
                for r in range(4):
                    rs = slice(r * 8, (r + 1) * 8)
                    nc.vector.max(out=selV[:, rs], in_=candVw[:])
                    if r < 3:
                        nc.vector.match_replace(out=candVw[:],
                                                in_to_replace=selV[:, rs],
                                                in_values=candVw[:],
                                                imm_value=NEG)
                mask = wk.tile([P, 128], U8, tag="selmask", name="selmask")
                nc.vector.tensor_scalar(out=mask[:], in0=candV[:],
                                        scalar1=selV[:, 31:32], scalar2=None,
                                        op0=ALU.is_ge)
                candIf = wk.tile([P, 128], F32, tag="candIf", name="candIf")
                nc.vector.tensor_copy(out=candIf[:], in_=candI[:])
                arr = wk.tile([P, 128], F32, tag="selarr", name="selarr")
                nc.vector.memset(arr[:], -1.0)
                nc.vector.copy_predicated(out=arr[:], mask=mask[:],
                                          data=candIf[:])
                selIf = wk.tile([P, K1], F32, tag="selIf", name="selIf")
                for r in range(4):
                    rs = slice(r * 8, (r + 1) * 8)
                    nc.vector.max(out=selIf[:, rs], in_=arr[:])
                    if r < 3:
                        nc.vector.match_replace(out=arr[:],
                                                in_to_replace=selIf[:, rs],
                                                in_values=arr[:],
                                                imm_value=-1.0)
                nbr16 = wk.tile([P, K1], I16, tag="nbr16", name="nbr16")
                nc.vector.tensor_copy(out=nbr16[:], in_=selIf[:])
                idxt = wrap_idx(wk, nbr16[:], ib1_ds[qt], "idxt1")
                gn = wk.tile([P, K1, 128], F16, tag="gn", name="gn")
                for k in range(4):
                    nc.gpsimd.dma_gather(gn[:, k * 8:(k + 1) * 8, :],
                                         tab0_d[:],
                                         idxt[:, k * 64:(k + 1) * 64],
                                         1024, 1024, 128)
                return gn, nqb

            def sa1_mlp(qt, gn, nqb):
                sl1 = slice(qt * P, (qt + 1) * P)
                acc = wk.tile([P, P], F32, tag="sa1acc", name="sa1acc")
                for c in range(8):
                    pg = psT.tile([P, 512], F16, tag="trans", name="trans")
                    for j in range(4):
                        nc.tensor.matmul(out=pg[0:67, j * 128:(j + 1) * 128],
                                         lhsT=gn[:, c * 4 + j, 0:67],
                                         rhs=identh[:], is_transpose=True,
                                         start=True, stop=True)
                    gtc = wk.tile([67, 512], F16, tag="gtc", name="gtc",
                                  bufs=3)
                    nc.scalar.activation(out=gtc[:], in_=pg[0:67, :],
                                         func=AF.Copy)
                    pz = mmtile()
                    nc.tensor.matmul(out=pz[:], lhsT=w1aug[:], rhs=gtc[:],
                                     start=True, stop=False)
                    nc.tensor.matmul(out=pz[:], lhsT=w1q[:], rhs=nqb[:, 0:512],
                                     start=False, stop=True)
                    h1 = wk.tile([P, 512], F16, tag="h1", name="h1", bufs=3)
                    nc.scalar.activation(out=h1[:], in_=pz[:], func=AF.Relu)
                    pz2 = mmtile()
                    nc.tensor.matmul(out=pz2[:], lhsT=w2sb[:], rhs=h1[:],
                                     start=True, stop=True)
                    red = wk.tile([P, P], F32, tag="sa1red", name="sa1red",
                                  bufs=3)
                    nc.vector.tensor_reduce(
                        out=red[:],
                        in_=pz2[:].rearrange("f (s q) -> f q s", q=P),
                        axis=AX.X, op=ALU.max)
                    if c == 0:
                        nc.vector.tensor_copy(out=acc[:], in_=red[:])
                    else:
                        nc.vector.tensor_tensor(out=acc[:], in0=acc[:],
                                                in1=red[:], op=ALU.max)
                nc.scalar.activation(out=feat1T[:, sl1], in_=acc[:],
                                     func=AF.Relu, bias=b2sb[:])

            carry = None
            for qt in range(T1 + 1):
                nxt = sa1_select(qt) if qt < T1 else None
                if carry is not None:
                    sa1_mlp(qt - 1, *carry)
                carry = nxt

            stage1 = wk.tile([P, T1, 256], F16, tag="stage1", name="stage1",
                             bufs=1)
            for t in range(T1):
                pf = trans16(feat1T[:, t * P:(t + 1) * P])
                nc.scalar.activation(out=stage1[:, t, 0:128], in_=pf,
                                     func=AF.Copy)
            nc.vector.tensor_copy(out=stage1[:, :, 128:133],
                                  in_=gq[:, :, 64:69])
            nc.vector.memset(stage1[:, :, 133:256], 0.0)
            nc.sync.dma_start(out=tab1_d.rearrange("(t p) c -> p t c", p=P),
                              in_=stage1[:])

        # ============ stage 3: SA2 ============
        with tc.tile_pool(name="sa2", bufs=2) as wk:
            gq2 = wk.tile([P, T2, 256], F16, tag="gq2", name="gq2", bufs=1)
            nc.gpsimd.dma_gather(gq2[:], tab1_d[:], i2sb[:], S2, S2, 256)
            sq2_pm = wk.tile([P, T2], F32, tag="sq2_pm", name="sq2_pm", bufs=1)
            nc.vector.tensor_tensor(out=sq2_pm[:], in0=gq2[:, :, 131],
                                    in1=gq2[:, :, 132], op=ALU.add)
            q2xyz = wk.tile([3, S2], F16, tag="q2xyz", name="q2xyz", bufs=1)
            q2sq = wk.tile([2, S2], F16, tag="q2sq", name="q2sq", bufs=1)
            for r in range(3):
                nc.sync.dma_start(
                    out=q2xyz[r:r + 1, :].rearrange("r (t q) -> r t q", q=P),
                    in_=gq2[:, :, 128 + r])
            for r in range(2):
                nc.sync.dma_start(
                    out=q2sq[r:r + 1, :].rearrange("r (t q) -> r t q", q=P),
                    in_=gq2[:, :, 131 + r])
            nc.scalar.activation(out=rhsF2a[:], in_=q2xyz[:],
                                 func=AF.Copy, scale=2.0)
            nc.scalar.activation(out=rhsF2b[:], in_=q2sq[:],
                                 func=AF.Copy, scale=-1.0)

            feat2T = [cst.tile([P, S2], F16, tag=f"feat2T{h}",
                               name=f"feat2T{h}") for h in range(2)]
            for t2 in range(T2):
                sl2 = slice(t2 * P, (t2 + 1) * P)
                lhsqB = wk.tile([3, P], F16, tag="lhsqB", name="lhsqB")
                nc.scalar.activation(out=lhsqB[:], in_=q2xyz[:, sl2],
                                     func=AF.Copy, scale=2.0)
                nqb2 = wk.tile([4, 512], F16, tag="nqb2", name="nqb2")
                nc.scalar.activation(
                    out=nqb2[0:3, :].rearrange("r (j q) -> r j q", q=P),
                    in_=q2xyz[:, sl2].unsqueeze(1).to_broadcast([3, 4, P]),
                    func=AF.Copy, scale=-1.0)
                nc.sync.dma_start(out=nqb2[3:4, :], in_=ones1_d[:])

                pm2 = bigtile()
                for hh in range(2):
                    hs = slice(hh * 512, (hh + 1) * 512)
                    nc.tensor.matmul(out=pm2[:, hs], lhsT=lhsqB[:],
                                     rhs=xyz1a[:, hs],
                                     start=True, stop=False)
                    nc.tensor.matmul(out=pm2[:, hs], lhsT=ones2[:],
                                     rhs=sqn1[:, hs],
                                     start=False, stop=True)
                selV2 = wk.tile([P, K2], F32, tag="selV2", name="selV2")
                selI2 = wk.tile([P, K2], U16, tag="selI2", name="selI2")
                for r in range(4):
                    rs = slice(r * 8, (r + 1) * 8)
                    nc.vector.max(out=selV2[:, rs], in_=pm2[:])
                    nc.vector.max_index(out=selI2[:, rs], in_max=selV2[:, rs],
                                        in_values=pm2[:])
                    if r < 3:
                        nc.vector.match_replace(out=pm2[:],
                                                in_to_replace=selV2[:, rs],
                                                in_values=pm2[:],
                                                imm_value=NEG)
                nbr2 = wk.tile([P, K2], I16, tag="nbr2", name="nbr2")
                nc.vector.tensor_copy(out=nbr2[:], in_=selI2[:])
                idxt2 = wrap_idx(wk, nbr2[:], ib2_ds[t2], "idxt2")
                gn2 = wk.tile([P, K2, 256], F16, tag="gn2", name="gn2")
                for k in range(4):
                    nc.gpsimd.dma_gather(gn2[:, k * 8:(k + 1) * 8, :],
                                         tab1_d[:],
                                         idxt2[:, k * 64:(k + 1) * 64],
                                         1024, 1024, 256)

                acc2 = [wk.tile([P, P], F32, tag=f"sa2acc{h}",
                                name=f"sa2acc{h}") for h in range(2)]
                for c in range(8):
                    pga = psT.tile([P, 512], F16, tag="trans", name="trans")
                    pgb = psT.tile([P, 512], F16, tag="trans", name="trans")
                    for j in range(4):
                        nc.tensor.matmul(out=pga[:, j * 128:(j + 1) * 128],
                                         lhsT=gn2[:, c * 4 + j, 0:128],
                                         rhs=identh[:], is_transpose=True,
                                         start=True, stop=True)
                        nc.tensor.matmul(out=pgb[0:3, j * 128:(j + 1) * 128],
                                         lhsT=gn2[:, c * 4 + j, 128:131],
                                         rhs=identh[:], is_transpose=True,
                                         start=True, stop=True)
                    gta = wk.tile([P, 512], F16, tag="gta", name="gta", bufs=3)
                    gtb = wk.tile([3, 512], F16, tag="gtb", name="gtb", bufs=3)
                    nc.scalar.activation(out=gta[:], in_=pga[:], func=AF.Copy)
                    nc.scalar.activation(out=gtb[:], in_=pgb[0:3, :],
                                         func=AF.Copy)
                    h1c = []
                    for h in range(2):
                        pz = mmtile()
                        nc.tensor.matmul(out=pz[:], lhsT=v1A[h][:], rhs=gta[:],
                                         start=True, stop=False)
                        nc.tensor.matmul(out=pz[:], lhsT=v1rel[h][:],
                                         rhs=gtb[:], start=False, stop=False)
                        nc.tensor.matmul(out=pz[:], lhsT=v1q[h][:],
                                         rhs=nqb2[:, 0:512],
                                         start=False, stop=True)
                        hh_ = wk.tile([P, 512], F16, tag=f"h1c{h}",
                                      name=f"h1c{h}", bufs=3)
                        nc.scalar.activation(out=hh_[:], in_=pz[:],
                                             func=AF.Relu)
                        h1c.append(hh_)
                    for h in range(2):
                        pz = mmtile()
                        nc.tensor.matmul(out=pz[:], lhsT=v2sb[0][h][:],
                                         rhs=h1c[0][:], start=True, stop=False)
                        nc.tensor.matmul(out=pz[:], lhsT=v2sb[1][h][:],
                                         rhs=h1c[1][:], start=False, stop=True)
                        red = wk.tile([P, P], F32, tag="sa2red",
                                      name="sa2red", bufs=3)
                        nc.vector.tensor_reduce(
                            out=red[:],
                            in_=pz[:].rearrange("f (s q) -> f q s", q=P),
                            axis=AX.X, op=ALU.max)
                        if c == 0:
                            nc.vector.tensor_copy(out=acc2[h][:], in_=red[:])
                        else:
                            nc.vector.tensor_tensor(out=acc2[h][:],
                                                    in0=acc2[h][:],
                                                    in1=red[:], op=ALU.max)
                for h in range(2):
                    nc.scalar.activation(out=feat2T[h][:, sl2],
                                         in_=acc2[h][:], func=AF.Relu,
                                         bias=c2sb[:, h:h + 1])

            stage2 = wk.tile([P, T2, 256], F16, tag="stage2", name="stage2",
                             bufs=1)
            for t2 in range(T2):
                for h in range(2):
                    pf = trans16(feat2T[h][:, t2 * P:(t2 + 1) * P])
                    nc.scalar.activation(
                        out=stage2[:, t2, h * 128:(h + 1) * 128], in_=pf,
                        func=AF.Copy)
            nc.sync.dma_start(out=tab2_d.rearrange("(t p) c -> p t c", p=P),
                              in_=stage2[:])

        # ============ stage 4: FP2 ============
        with tc.tile_pool(name="fp2", bufs=2) as wk:
            v8f = wk.tile([P, T1, 8], F32, tag="v8f", name="v8f", bufs=1)
            p8f = wk.tile([P, T1, 8], U16, tag="p8f", name="p8f", bufs=1)
            for qt in range(T1):
                pm3 = mmtile()
                nc.tensor.matmul(out=pm3[:, 0:S2],
                                 lhsT=xyz1a[:, qt * P:(qt + 1) * P],
                                 rhs=rhsF2a[:], start=True, stop=False)
                nc.tensor.matmul(out=pm3[:, 0:S2], lhsT=ones2[:],
                                 rhs=rhsF2b[:], start=False, stop=True)
                nc.vector.max(out=v8f[:, qt, :], in_=pm3[:, 0:S2])
                nc.vector.max_index(out=p8f[:, qt, :], in_max=v8f[:, qt, :],
                                    in_values=pm3[:, 0:S2])

            # inverse-distance weights
            d2f = wk.tile([P, T1, 3], F32, tag="d2f", name="d2f", bufs=1)
            nc.vector.tensor_tensor(
                out=d2f[:], in0=sq1_pm[:].unsqueeze(2).to_broadcast([P, T1, 3]),
                in1=v8f[:, :, 0:3], op=ALU.subtract)
            nc.scalar.activation(out=d2f[:], in_=d2f[:], func=AF.Relu)
            nc.scalar.activation(out=d2f[:], in_=d2f[:], func=AF.Sqrt)
            nc.vector.tensor_scalar_max(d2f[:], d2f[:], 1e-10)
            wn2 = wk.tile([P, T1, 3], F32, tag="wn2", name="wn2", bufs=1)
            nc.vector.reciprocal(out=wn2[:], in_=d2f[:])
            ws2 = wk.tile([P, T1], F32, tag="ws2", name="ws2", bufs=1)
            nc.vector.tensor_reduce(out=ws2[:], in_=wn2[:], axis=AX.X,
                                    op=ALU.add)
            nc.vector.reciprocal(out=ws2[:], in_=ws2[:])
            nc.vector.tensor_tensor(
                out=wn2[:], in0=wn2[:],
                in1=ws2[:].unsqueeze(2).to_broadcast([P, T1, 3]), op=ALU.mult)
            wn2h = wk.tile([P, T1, 3], F16, tag="wn2h", name="wn2h", bufs=1)
            nc.vector.tensor_copy(out=wn2h[:], in_=wn2[:])

            p3f = wk.tile([P, T1 * 3], I16, tag="p3f", name="p3f", bufs=1)
            nc.vector.tensor_copy(out=p3f[:].rearrange("p (t j) -> p t j", j=3),
                                  in_=p8f[:, :, 0:3])
            idxtf2 = wrap_idx(wk, p3f[:], ibf2_d, "idxtf2")
            gi2 = wk.tile([P, T1, 3, 256], F16, tag="gi2", name="gi2", bufs=1)
            gi2v = gi2[:].rearrange("p t j c -> p (t j) c")
            for k in range(3):
                nc.gpsimd.dma_gather(gi2v[:, k * 8:(k + 1) * 8, :], tab2_d[:],
                                     idxtf2[:, k * 64:(k + 1) * 64],
                                     1024, 1024, 256)
            # weighted 3-NN sum: t0*w0 + t1*w1 + t2*w2
            tmp0 = wk.tile([P, T1, 256], F16, tag="tmp0", name="tmp0", bufs=1)
            nc.vector.tensor_tensor(
                out=tmp0[:], in0=gi2[:, :, 0, :],
                in1=wn2h[:, :, 0:1].to_broadcast([P, T1, 256]), op=ALU.mult)
            tmp1 = wk.tile([P, T1, 256], F16, tag="tmp1", name="tmp1", bufs=1)
            nc.vector.tensor_tensor(
                out=tmp1[:], in0=gi2[:, :, 1, :],
                in1=wn2h[:, :, 1:2].to_broadcast([P, T1, 256]), op=ALU.mult)
            nc.vector.tensor_tensor(out=tmp0[:], in0=tmp0[:], in1=tmp1[:],
                                    op=ALU.add)
            nc.vector.tensor_tensor(
                out=tmp1[:], in0=gi2[:, :, 2, :],
                in1=wn2h[:, :, 2:3].to_broadcast([P, T1, 256]), op=ALU.mult)
            it2 = wk.tile([P, T1, 256], F16, tag="it2", name="it2", bufs=1)
            nc.vector.tensor_tensor(out=it2[:], in0=tmp0[:], in1=tmp1[:],
                                    op=ALU.add)
            itT2 = [wk.tile([P, S1], F16, tag=f"itT2{h}", name=f"itT2{h}",
                            bufs=1) for h in range(2)]
            for t in range(T1):
                for h in range(2):
                    pf = trans16(it2[:, t, h * 128:(h + 1) * 128])
                    nc.scalar.activation(out=itT2[h][:, t * P:(t + 1) * P],
                                         in_=pf, func=AF.Copy)
            for c in range(2):
                cs = slice(c * 512, (c + 1) * 512)
                pz = mmtile()
                nc.tensor.matmul(out=pz[:], lhsT=f2w1[0][:], rhs=feat1T[:, cs],
                                 start=True, stop=False)
                nc.tensor.matmul(out=pz[:], lhsT=f2w1[1][:], rhs=itT2[0][:, cs],
                                 start=False, stop=False)
                nc.tensor.matmul(out=pz[:], lhsT=f2w1[2][:], rhs=itT2[1][:, cs],
                                 start=False, stop=True)
                hf = wk.tile([P, 512], F16, tag="fp2h", name="fp2h", bufs=3)
                nc.scalar.activation(out=hf[:], in_=pz[:], func=AF.Relu,
                                     bias=f2b1[:])
                pz2 = mmtile()
                nc.tensor.matmul(out=pz2[:], lhsT=f2w2[:], rhs=hf[:],
                                 start=True, stop=True)
                nc.scalar.activation(out=f1upT[:, cs], in_=pz2[:], func=AF.Relu,
                                     bias=f2b2[:])
            stagef = wk.tile([P, T1, 128], F16, tag="stagef", name="stagef",
                             bufs=1)
            for t in range(T1):
                pf = trans16(f1upT[:, t * P:(t + 1) * P])
                nc.scalar.activation(out=stagef[:, t, :], in_=pf, func=AF.Copy)
            nc.sync.dma_start(out=tabf_d.rearrange("(t p) c -> p t c", p=P),
                              in_=stagef[:])

        # ============ stage 5: FP1 (16 pipelined groups) ============
        with tc.tile_pool(name="fp1", bufs=2) as wk:
            def fp1_select(g):
                v81 = wk.tile([P, GT, 8], F32, tag="v81", name="v81")
                p81 = wk.tile([P, GT, 8], U16, tag="p81", name="p81")
                for j in range(GT):
                    qt = g * GT + j
                    pm4 = bigtile()
                    for hh in range(2):
                        hs = slice(hh * 512, (hh + 1) * 512)
                        nc.tensor.matmul(out=pm4[:, hs],
                                         lhsT=bigT[0:3, qt * P:(qt + 1) * P],
                                         rhs=rhs2a[:, hs],
                                         start=True, stop=False)
                        nc.tensor.matmul(out=pm4[:, hs], lhsT=ones2[:],
                                         rhs=sqn1[:, hs],
                                         start=False, stop=True)
                    nc.vector.max(out=v81[:, j, :], in_=pm4[:])
                    nc.vector.max_index(out=p81[:, j, :], in_max=v81[:, j, :],
                                        in_values=pm4[:])

                sqs = sq_pm[:, g * GT:(g + 1) * GT]
                d21 = wk.tile([P, GT, 3], F32, tag="d21", name="d21")
                nc.vector.tensor_tensor(
                    out=d21[:], in0=sqs.unsqueeze(2).to_broadcast([P, GT, 3]),
                    in1=v81[:, :, 0:3], op=ALU.subtract)
                nc.scalar.activation(out=d21[:], in_=d21[:], func=AF.Relu)
                nc.scalar.activation(out=d21[:], in_=d21[:], func=AF.Sqrt)
                nc.vector.tensor_scalar_max(d21[:], d21[:], 1e-10)
                wn1 = wk.tile([P, GT, 3], F32, tag="wn1", name="wn1")
                nc.vector.reciprocal(out=wn1[:], in_=d21[:])
                ws1 = wk.tile([P, GT], F32, tag="ws1", name="ws1")
                nc.vector.tensor_reduce(out=ws1[:], in_=wn1[:], axis=AX.X,
                                        op=ALU.add)
                nc.vector.reciprocal(out=ws1[:], in_=ws1[:])
                nc.vector.tensor_tensor(
                    out=wn1[:], in0=wn1[:],
                    in1=ws1[:].unsqueeze(2).to_broadcast([P, GT, 3]),
                    op=ALU.mult)
                wn1h = wk.tile([P, GT, 3], F16, tag="wn1h", name="wn1h")
                nc.vector.tensor_copy(out=wn1h[:], in_=wn1[:])

                p31 = wk.tile([P, GT * 3], I16, tag="p31", name="p31")
                nc.vector.tensor_copy(
                    out=p31[:].rearrange("p (t j) -> p t j", j=3),
                    in_=p81[:, :, 0:3])
                idxtf1 = wrap_idx(wk, p31[:], ibf1_ds[g], "idxtf1")
                gi1 = wk.tile([P, GT, 3, 128], F16, tag="gi1", name="gi1")
                gi1v = gi1[:].rearrange("p t j c -> p (t j) c")
                for k in range(3):
                    nc.gpsimd.dma_gather(gi1v[:, k * 8:(k + 1) * 8, :],
                                         tabf_d[:],
                                         idxtf1[:, k * 64:(k + 1) * 64],
                                         1024, 1024, 128)
                return gi1, wn1h

            def fp1_post(g, gi1, wn1h):
                ta = wk.tile([P, GT, 128], F16, tag="ta", name="ta")
                nc.vector.tensor_tensor(
                    out=ta[:], in0=gi1[:, :, 0, :],
                    in1=wn1h[:, :, 0:1].to_broadcast([P, GT, 128]),
                    op=ALU.mult)
                tb = wk.tile([P, GT, 128], F16, tag="tb", name="tb")
                nc.vector.tensor_tensor(
                    out=tb[:], in0=gi1[:, :, 1, :],
                    in1=wn1h[:, :, 1:2].to_broadcast([P, GT, 128]),
                    op=ALU.mult)
                nc.vector.tensor_tensor(out=ta[:], in0=ta[:], in1=tb[:],
                                        op=ALU.add)
                nc.vector.tensor_tensor(
                    out=tb[:], in0=gi1[:, :, 2, :],
                    in1=wn1h[:, :, 2:3].to_broadcast([P, GT, 128]),
                    op=ALU.mult)
                it1 = wk.tile([P, GT, 128], F16, tag="it1", name="it1")
                nc.vector.tensor_tensor(out=it1[:], in0=ta[:], in1=tb[:],
                                        op=ALU.add)
                itT1 = wk.tile([P, GT * 128], F16, tag="itT1", name="itT1")
                for t in range(GT):
                    pf = trans16(it1[:, t, :])
                    nc.scalar.activation(out=itT1[:, t * P:(t + 1) * P],
                                         in_=pf, func=AF.Copy)
                for c in range(2):
                    cs = slice(c * 512, (c + 1) * 512)
                    gcs = slice(g * GT * P + c * 512,
                                g * GT * P + (c + 1) * 512)
                    pz = mmtile()
                    nc.tensor.matmul(out=pz[:], lhsT=f1w1b[:], rhs=itT1[:, cs],
                                     start=True, stop=False)
                    nc.tensor.matmul(out=pz[:], lhsT=f1w1a[:],
                                     rhs=f0TS[:, gcs], start=False, stop=True)
                    hf = wk.tile([P, 512], F16, tag="fp1h", name="fp1h",
                                 bufs=3)
                    nc.scalar.activation(out=hf[:], in_=pz[:], func=AF.Relu,
                                         bias=f1b1[:])
                    pz2 = mmtile()
                    nc.tensor.matmul(out=pz2[:], lhsT=f1w2[:], rhs=hf[:],
                                     start=True, stop=True)
                    nc.scalar.activation(out=fuTS[:, gcs], in_=pz2[:],
                                         func=AF.Relu, bias=f1b2[:])
                    red = wk.tile([P, 1], F32, tag="gfred", name="gfred",
                                  bufs=3)
                    nc.vector.tensor_reduce(out=red[:], in_=fuTS[:, gcs],
                                            axis=AX.X, op=ALU.max)
                    if g == 0 and c == 0:
                        nc.vector.tensor_copy(out=gfacc[:], in_=red[:])
                    else:
                        nc.vector.tensor_tensor(out=gfacc[:], in0=gfacc[:],
                                                in1=red[:], op=ALU.max)

            fcarry = None
            for g in range(NG + 1):
                fnxt = fp1_select(g) if g < NG else None
                if fcarry is not None:
                    fp1_post(g - 1, *fcarry)
                fcarry = fnxt

        # ============ stage 6: head ============
        with tc.tile_pool(name="head", bufs=2) as wk:
            gfh = wk.tile([P, 1], F16, tag="gfh", name="gfh", bufs=1)
            nc.vector.tensor_copy(out=gfh[:], in_=gfacc[:])
            pc = mmtile()
            nc.tensor.matmul(out=pc[:, 0:1], lhsT=hw1b[:], rhs=gfh[:],
                             start=True, stop=True)
            nc.vector.tensor_tensor(out=biasH[:], in0=pc[:, 0:1], in1=hb1[:],
                                    op=ALU.add)
            for g in range(8):
                ostage = wk.tile([P, 16, 13], F32, tag="ostage", name="ostage")
                for c4 in range(4):
                    c = g * 4 + c4
                    cs = slice(c * 512, (c + 1) * 512)
                    pz = mmtile()
                    nc.tensor.matmul(out=pz[:], lhsT=hw1a[:], rhs=fuTS[:, cs],
                                     start=True, stop=True)
                    h1 = wk.tile([P, 512], F16, tag="hh1", name="hh1", bufs=3)
                    nc.scalar.activation(out=h1[:], in_=pz[:], func=AF.Relu,
                                         bias=biasH[:])
                    pz2 = mmtile()
                    nc.tensor.matmul(out=pz2[:64, :], lhsT=hw2[:], rhs=h1[:],
                                     start=True, stop=True)
                    h2 = wk.tile([64, 512], F16, tag="hh2", name="hh2", bufs=3)
                    nc.scalar.activation(out=h2[:], in_=pz2[:64, :],
                                         func=AF.Relu, bias=hb2[:])
                    pz3 = mmtile()
                    nc.tensor.matmul(out=pz3[:13, :], lhsT=hw3[:], rhs=h2[:],
                                     start=True, stop=True)
                    oT = wk.tile([13, 512], F16, tag="hoT", name="hoT", bufs=3)
                    nc.vector.tensor_tensor(
                        out=oT[:], in0=pz3[:13, :],
                        in1=hb3[:, 0:1].to_broadcast([13, 512]), op=ALU.add)
                    po = psT.tile([P, 512], F16, tag="trans", name="trans")
                    for t in range(4):
                        nc.tensor.matmul(out=po[:, t * 16:t * 16 + 13],
                                         lhsT=oT[:, t * 128:(t + 1) * 128],
                                         rhs=identh[0:13, 0:13],
                                         is_transpose=True, start=True,
                                         stop=True)
                    nc.scalar.activation(
                        out=ostage[:, c4 * 4:(c4 + 1) * 4, :],
                        in_=po[:, 0:64].rearrange(
                            "p (t c) -> p t c", c=16)[:, :, 0:13],
                        func=AF.Copy)
                nc.sync.dma_start(
                    out=out_d.rearrange("(t p) c -> p t c", p=P)[
                        :, g * 16:(g + 1) * 16, :],
                    in_=ostage[:])

    return nc


# ---------------------------------------------------------------- host side
_CACHED_NC = None


def _get_nc():
    global _CACHED_NC
    if _CACHED_NC is None:
        nc = build_nc()
        nc.finalize()
        _CACHED_NC = nc
    return _CACHED_NC


def _per_core_inputs(b, inputs):
    x = np.asarray(inputs["x"][b]).astype(np.float16)
    i1 = np.asarray(inputs["idx_s1"][b]).astype(np.int16)
    i1w = np.tile(i1.reshape(S1 // 16, 16).T, (8, 1))
    i2 = np.asarray(inputs["idx_s2"][b]).astype(np.int16)
    i2w = np.tile(i2.reshape(S2 // 16, 16).T, (8, 1))
    f16 = lambda a: np.ascontiguousarray(np.asarray(a), dtype=np.float16)
    f32 = lambda a: np.ascontiguousarray(np.asarray(a), dtype=np.float32)
    return {
        "xh": np.ascontiguousarray(x),
        "xTh": np.ascontiguousarray(x.T),
        "i1w": np.ascontiguousarray(i1w),
        "i2w": np.ascontiguousarray(i2w),
        "embw": f16(inputs["embed_w"]),
        "embb": f32(inputs["embed_b"]).reshape(64, 1),
        "w1": f16(inputs["sa1_w1"]),
        "b1r": f16(inputs["sa1_b1"]).reshape(1, 128),
        "w2": f16(inputs["sa1_w2"]),
        "b2": f32(inputs["sa1_b2"]).reshape(128, 1),
        "v1": f16(inputs["sa2_w1"]),
        "c1r": f16(inputs["sa2_b1"]).reshape(1, 256),
        "v2": f16(inputs["sa2_w2"]),
        "c2": np.ascontiguousarray(f32(inputs["sa2_b2"]).reshape(2, 128).T),
        "f2w1": f16(inputs["fp2_w1"]),
        "f2b1": f32(inputs["fp2_b1"]).reshape(128, 1),
        "f2w2": f16(inputs["fp2_w2"]),
        "f2b2": f32(inputs["fp2_b2"]).reshape(128, 1),
        "f1w1": f16(inputs["fp1_w1"]),
        "f1b1": f32(inputs["fp1_b1"]).reshape(128, 1),
        "f1w2": f16(inputs["fp1_w2"]),
        "f1b2": f32(inputs["fp1_b2"]).reshape(128, 1),
        "hw1": f16(inputs["head_w1"]),
        "hb1": f32(inputs["head_b1"]).reshape(128, 1),
        "hw2": f16(inputs["head_w2"]),
        "hb2": f32(inputs["head_b2"]).reshape(64, 1),
        "hw3": f16(inputs["head_w3"]),
        "hb3": f32(inputs["head_b3"]).reshape(13, 1),
    }


def run(inputs, trace=False, **kw):
    nc = _get_nc()
    B = inputs["x"].shape[0]
    in_maps = [_per_core_inputs(b, inputs) for b in range(B)]
    res = run_bass_kernel_spmd(nc, in_maps, core_ids=list(range(B)),
                               trace=trace, **kw)
    out = np.stack([res.results[b]["out"] for b in range(B)])
    return out, res


def kernel(**inputs):
    return run(inputs)[0]


if __name__ == "__main__":
    build_nc()
    print("built ok")


# revision 14
# speedup vs baseline: 1.8119x; 1.1060x over previous
"""PointNet++-lite segmentation on 8 Trainium2 cores (batch-parallel, one
point cloud per core). Self-contained: hardcodes shapes from the problem spec.

Per-core pipeline (all on device):
  embed MLP -> SA1 (KNN top-32 of 16384, gather, 2-layer MLP, max-pool)
  -> SA2 (KNN top-32 of 1024) -> FP2/FP1 (3-NN inverse-distance interp)
  -> global-max head MLP -> (16384, 13) logits.

fp16 datapath: all PE matmuls run on fp16 operands (4x the fp32 rate), with
fp32 PSUM accumulation.  KNN ranking uses m = 2 q.x - |x|^2 (row-constant
|q|^2 dropped); |x|^2 enters the fp16 matmul split into hi+lo fp16 halves so
m keeps ~22 mantissa bits (self-distances stay ~0, exact inverse-distance
weights).  Neighbor tables are fp16 rows in DRAM; SA1/SA2 gathers use
dma_gather transpose mode which lands features on partitions, removing the
per-neighbor PE transposes.  Selection runs on the vector engine max8 /
max_index over 1024-wide PSUM blocks.
"""

from contextlib import ExitStack

import numpy as np

import concourse.bass as bass
import concourse.mybir as mybir
from concourse.bacc import Bacc
from concourse.bass_utils import run_bass_kernel_spmd
from concourse.masks import make_identity
from concourse.tile import TileContext

F32 = mybir.dt.float32
F16 = mybir.dt.float16
U16 = mybir.dt.uint16
U8 = mybir.dt.uint8
I16 = mybir.dt.int16
AF = mybir.ActivationFunctionType
ALU = mybir.AluOpType
AX = mybir.AxisListType

P = 128
N = 16384
S1, K1 = 1024, 32
S2, K2 = 256, 32
NCLS = 13
NEG = -3.0e38

NT = N // P        # 128 point tiles
T1 = S1 // P       # 8 SA1 query tiles
T2 = S2 // P       # 2 SA2 query tiles
NG = 16            # FP1 groups (8 tiles each)
GT = NT // NG      # tiles per FP1 group


def build_nc():
    nc = Bacc()

    xh_in = nc.dram_tensor("xh", [N, 6], F16, kind="ExternalInput")
    xTh_in = nc.dram_tensor("xTh", [6, N], F16, kind="ExternalInput")
    i1_in = nc.dram_tensor("i1w", [P, S1 // 16], I16, kind="ExternalInput")
    i2_in = nc.dram_tensor("i2w", [P, S2 // 16], I16, kind="ExternalInput")
    wdecl16 = [
        ("embw", [6, 64]),
        ("w1", [67, 128]), ("b1r", [1, 128]), ("w2", [128, 128]),
        ("v1", [131, 256]), ("c1r", [1, 256]), ("v2", [256, 256]),
        ("f2w1", [384, 128]), ("f2w2", [128, 128]),
        ("f1w1", [192, 128]), ("f1w2", [128, 128]),
        ("hw1", [256, 128]), ("hw2", [128, 64]), ("hw3", [64, 13]),
    ]
    wdecl32 = [
        ("embb", [64, 1]), ("b2", [128, 1]), ("c2", [128, 2]),
        ("f2b1", [128, 1]), ("f2b2", [128, 1]),
        ("f1b1", [128, 1]), ("f1b2", [128, 1]),
        ("hb1", [128, 1]), ("hb2", [64, 1]), ("hb3", [13, 1]),
    ]
    din = {nm: nc.dram_tensor(nm, sh, F16, kind="ExternalInput")
           for nm, sh in wdecl16}
    din.update({nm: nc.dram_tensor(nm, sh, F32, kind="ExternalInput")
                for nm, sh in wdecl32})
    out_d = nc.dram_tensor("out", [N, NCLS], F32, kind="ExternalOutput")

    tab0_d = nc.dram_tensor("tab0", [N, 128], F16)
    tab1_d = nc.dram_tensor("tab1", [S1, 256], F16)
    tab2_d = nc.dram_tensor("tab2", [S2, 256], F16)
    tabf_d = nc.dram_tensor("tabf", [S1, 128], F16)
    ib1_ds = [nc.dram_tensor(f"ib1_{t}", [16, 256], I16) for t in range(T1)]
    ib2_ds = [nc.dram_tensor(f"ib2_{t}", [16, 256], I16) for t in range(T2)]
    ibf2_d = nc.dram_tensor("ibf2", [16, 192], I16)
    ones2_d = nc.dram_tensor("ones2_d", [2, 128], F16)
    ones1_d = nc.dram_tensor("ones1_d", [1, 512], F16)
    ibf1_ds = [nc.dram_tensor(f"ibf1_{g}", [16, 192], I16) for g in range(NG)]

    with TileContext(nc) as tc, ExitStack() as ctx:
        cst = ctx.enter_context(tc.tile_pool(name="cst", bufs=1))
        psB = ctx.enter_context(tc.tile_pool(name="psB", bufs=2, space="PSUM"))
        psA = ctx.enter_context(tc.tile_pool(name="psA", bufs=2, space="PSUM"))
        psT = ctx.enter_context(tc.tile_pool(name="psT", bufs=2, space="PSUM"))

        identh = cst.tile([P, P], F16, tag="identh", name="identh")
        make_identity(nc, identh[:])

        def bigtile():
            return psB.tile([P, 1024], F32, tag="big", name="big")

        def mmtile():
            return psA.tile([P, 512], F32, tag="mm", name="mm")

        def trans16(in_ap):
            """PE transpose of fp16 data: in_(p,f) -> fp16 psum (f,p)."""
            pt = psT.tile([P, 512], F16, tag="trans", name="trans")
            k = in_ap.shape[0]
            f = in_ap.shape[-1]
            nc.tensor.matmul(out=pt[:f, :k], lhsT=in_ap, rhs=identh[:k, :k],
                             is_transpose=True, start=True, stop=True)
            return pt[:f, :k]


        # ---------------- constants / weights ----------------
        def load(name, src, shape, dtype=F16):
            t = cst.tile(list(shape), dtype, tag=name, name=name)
            nc.sync.dma_start(out=t[:], in_=src)
            return t

        embw = load("embw", din["embw"][:], (6, 64))
        embb = load("embb", din["embb"][:], (64, 1), F32)
        w2sb = load("w2sb", din["w2"][:], (128, 128))
        b2sb = load("b2sb", din["b2"][:], (128, 1), F32)
        c2sb = load("c2sb", din["c2"][:], (128, 2), F32)
        f2b1 = load("f2b1", din["f2b1"][:], (128, 1), F32)
        f2w2 = load("f2w2", din["f2w2"][:], (128, 128))
        f2b2 = load("f2b2", din["f2b2"][:], (128, 1), F32)
        f1b1 = load("f1b1", din["f1b1"][:], (128, 1), F32)
        f1w2 = load("f1w2", din["f1w2"][:], (128, 128))
        f1b2 = load("f1b2", din["f1b2"][:], (128, 1), F32)
        hb1 = load("hb1", din["hb1"][:], (128, 1), F32)
        hw2 = load("hw2", din["hw2"][:], (128, 64))
        hb2 = load("hb2", din["hb2"][:], (64, 1), F32)
        hw3 = load("hw3", din["hw3"][:], (64, 13))
        hb3 = load("hb3", din["hb3"][:], (13, 1), F32)

        # SA1 grouped-MLP weights: rows [feat(64), rel_xyz(3)]
        w1aug = cst.tile([67, 128], F16, tag="w1aug", name="w1aug")
        nc.sync.dma_start(out=w1aug[0:64, :], in_=din["w1"][3:67, :])
        nc.sync.dma_start(out=w1aug[64:67, :], in_=din["w1"][0:3, :])
        w1q = cst.tile([4, 128], F16, tag="w1q", name="w1q")
        nc.sync.dma_start(out=w1q[0:3, :], in_=din["w1"][0:3, :])
        nc.sync.dma_start(out=w1q[3:4, :], in_=din["b1r"][:])

        v1A = [load(f"v1A{h}", din["v1"][3:131, h * 128:(h + 1) * 128],
                    (128, 128)) for h in range(2)]
        v1rel, v1q = [], []
        for h in range(2):
            sl = slice(h * 128, (h + 1) * 128)
            t = cst.tile([3, 128], F16, tag=f"v1rel{h}", name=f"v1rel{h}")
            nc.sync.dma_start(out=t[:], in_=din["v1"][0:3, sl])
            v1rel.append(t)
            t2 = cst.tile([4, 128], F16, tag=f"v1q{h}", name=f"v1q{h}")
            nc.sync.dma_start(out=t2[0:3, :], in_=din["v1"][0:3, sl])
            nc.sync.dma_start(out=t2[3:4, :], in_=din["c1r"][0:1, sl])
            v1q.append(t2)
        v2sb = [[load(f"v2{k}{h}",
                      din["v2"][k * 128:(k + 1) * 128, h * 128:(h + 1) * 128],
                      (128, 128)) for h in range(2)] for k in range(2)]
        f2w1 = [load(f"f2w1{k}", din["f2w1"][k * 128:(k + 1) * 128, :],
                     (128, 128)) for k in range(3)]
        f1w1a = load("f1w1a", din["f1w1"][0:64, :], (64, 128))
        f1w1b = load("f1w1b", din["f1w1"][64:192, :], (128, 128))
        hw1a = load("hw1a", din["hw1"][0:128, :], (128, 128))
        hw1b = load("hw1b", din["hw1"][128:256, :], (128, 128))
        i1sb = load("i1sb", i1_in[:], (P, S1 // 16), I16)
        i2sb = load("i2sb", i2_in[:], (P, S2 // 16), I16)

        # block offsets for 1024-wide max8 blocks: j*1024, 8 copies each
        offs = cst.tile([P, 128], U16, tag="offs", name="offs")
        nc.gpsimd.iota(out=offs[:], pattern=[[1024, 16], [0, 8]], base=0,
                       channel_multiplier=0)

        ones2 = cst.tile([2, P], F16, tag="ones2", name="ones2")
        nc.vector.memset(ones2[:], 1.0)
        nc.sync.dma_start(out=ones2_d[:], in_=ones2[:])
        onesr = cst.tile([1, 512], F16, tag="onesr", name="onesr")
        nc.vector.memset(onesr[:], 1.0)
        nc.sync.dma_start(out=ones1_d[:], in_=onesr[:])

        def wrap_idx(pool, src_i16, bounce_d, tag):
            """src (128, M) i16 -> replicated wrapped idx tile (128, 8*M)
            via DRAM bounce.  Flat gather slot j*128+q reads src[q, j]."""
            M = src_i16.shape[-1]
            bw = bounce_d[:, :].rearrange("c (j e) -> c j e", e=8)
            for ph in range(8):
                nc.sync.dma_start(out=bw[:, 0:M, ph],
                                  in_=src_i16[ph * 16:(ph + 1) * 16, :])
            idxt = pool.tile([P, 8 * M], I16, tag=tag, name=tag)
            for g in range(8):
                nc.sync.dma_start(out=idxt[g * 16:(g + 1) * 16, :],
                                  in_=bounce_d[:, :])
            return idxt

        # persistent cross-stage tensors
        sq_pm = cst.tile([P, NT], F32, tag="sq_pm", name="sq_pm")
        bigT = cst.tile([5, N], F16, tag="bigT", name="bigT")
        f0TS = cst.tile([64, N], F16, tag="f0TS", name="f0TS")
        fuTS = cst.tile([P, N], F16, tag="fuTS", name="fuTS")
        gq = cst.tile([P, T1, 128], F16, tag="gq", name="gq")
        xyz1a = cst.tile([3, S1], F16, tag="xyz1a", name="xyz1a")
        sqn1 = cst.tile([2, S1], F16, tag="sqn1", name="sqn1")
        rhs2a = cst.tile([3, S1], F16, tag="rhs2a", name="rhs2a")
        rhsF2a = cst.tile([3, S2], F16, tag="rhsF2a", name="rhsF2a")
        rhsF2b = cst.tile([2, S2], F16, tag="rhsF2b", name="rhsF2b")
        sq1_pm = cst.tile([P, T1], F32, tag="sq1_pm", name="sq1_pm")
        feat1T = cst.tile([P, S1], F16, tag="feat1T", name="feat1T")
        f1upT = cst.tile([P, S1], F16, tag="f1upT", name="f1upT")
        gfacc = cst.tile([P, 1], F32, tag="gfacc", name="gfacc")
        biasH = cst.tile([P, 1], F32, tag="biasH", name="biasH")

        # ============ stage 0+1: geometry, embed, tab0 ============
        with tc.tile_pool(name="st01", bufs=2) as wk:
            xzh = wk.tile([P, NT, 6], F16, tag="xzh", name="xzh", bufs=1)
            nc.sync.dma_start(
                out=xzh[:], in_=xh_in.rearrange("(t p) c -> p t c", p=P))
            sqt = wk.tile([P, NT, 3], F32, tag="sqt", name="sqt", bufs=1)
            nc.vector.tensor_tensor(out=sqt[:], in0=xzh[:, :, 0:3],
                                    in1=xzh[:, :, 0:3], op=ALU.mult)
            nc.vector.tensor_reduce(out=sq_pm[:], in_=sqt[:], axis=AX.X,
                                    op=ALU.add)
            # split |x|^2 into fp16 hi+lo halves (negated for the m matmul)
            hi16 = wk.tile([P, NT], F16, tag="hi16", name="hi16", bufs=1)
            nc.vector.tensor_copy(out=hi16[:], in_=sq_pm[:])
            hi32 = wk.tile([P, NT], F32, tag="hi32", name="hi32", bufs=1)
            nc.vector.tensor_copy(out=hi32[:], in_=hi16[:])
            lo32 = wk.tile([P, NT], F32, tag="lo32", name="lo32", bufs=1)
            nc.vector.tensor_tensor(out=lo32[:], in0=sq_pm[:], in1=hi32[:],
                                    op=ALU.subtract)
            lo16 = wk.tile([P, NT], F16, tag="lo16", name="lo16", bufs=1)
            nc.vector.tensor_copy(out=lo16[:], in_=lo32[:])
            nhi = wk.tile([P, NT], F16, tag="nhi", name="nhi", bufs=1)
            nc.scalar.activation(out=nhi[:], in_=hi16[:], func=AF.Copy,
                                 scale=-1.0)
            nlo = wk.tile([P, NT], F16, tag="nlo", name="nlo", bufs=1)
            nc.scalar.activation(out=nlo[:], in_=lo16[:], func=AF.Copy,
                                 scale=-1.0)

            nc.sync.dma_start(out=bigT[0:3, :], in_=xTh_in[0:3, :])
            nc.sync.dma_start(
                out=bigT[3:4, :].rearrange("r (t p) -> r t p", p=P),
                in_=nhi[:])
            nc.sync.dma_start(
                out=bigT[4:5, :].rearrange("r (t p) -> r t p", p=P),
                in_=nlo[:])

            for g in range(16):
                stage = wk.tile([P, 8, 128], F16, tag="tab0stage",
                                name="tab0stage")
                sl8 = slice(g * 8, (g + 1) * 8)
                nc.vector.tensor_copy(out=stage[:, :, 64:67],
                                      in_=xzh[:, sl8, 0:3])
                nc.vector.tensor_copy(out=stage[:, :, 67:68],
                                      in_=hi16[:, sl8].unsqueeze(2))
                nc.vector.tensor_copy(out=stage[:, :, 68:69],
                                      in_=lo16[:, sl8].unsqueeze(2))
                nc.vector.memset(stage[:, :, 69:128], 0.0)
                for cc in range(2):
                    c = g * 2 + cc
                    xc = wk.tile([6, 512], F16, tag="xc", name="xc", bufs=3)
                    nc.sync.dma_start(out=xc[:],
                                      in_=xTh_in[:, c * 512:(c + 1) * 512])
                    pe = mmtile()
                    nc.tensor.matmul(out=pe[:64, :], lhsT=embw[:], rhs=xc[:],
                                     start=True, stop=True)
                    nc.scalar.activation(out=f0TS[:, c * 512:(c + 1) * 512],
                                         in_=pe[:64, :], func=AF.Relu,
                                         bias=embb[:])
                    pt = psT.tile([P, 512], F16, tag="trans", name="trans")
                    for t4 in range(4):
                        sl = slice(c * 512 + t4 * 128, c * 512 + (t4 + 1) * 128)
                        nc.tensor.matmul(out=pt[:, t4 * 64:(t4 + 1) * 64],
                                         lhsT=f0TS[:, sl], rhs=identh[:64, :64],
                                         is_transpose=True, start=True,
                                         stop=True)
                    nc.scalar.activation(
                        out=stage[:, cc * 4:(cc + 1) * 4, 0:64],
                        in_=pt[:, 0:256].rearrange("p (j c) -> p j c", c=64),
                        func=AF.Copy)
                nc.sync.dma_start(
                    out=tab0_d.rearrange("(t p) c -> p t c", p=P)[:, sl8, :],
                    in_=stage[:])

        # ============ stage 2: SA1 ============
        with tc.tile_pool(name="sa1", bufs=2) as wk:
            nc.gpsimd.dma_gather(gq[:], tab0_d[:], i1sb[:], S1, S1, 128)
            nc.vector.tensor_tensor(out=sq1_pm[:], in0=gq[:, :, 67],
                                    in1=gq[:, :, 68], op=ALU.add)
            qxyz = cst.tile([3, S1], F16, tag="qxyz", name="qxyz")
            qsq = cst.tile([2, S1], F16, tag="qsq", name="qsq")
            for r in range(3):
                nc.sync.dma_start(
                    out=qxyz[r:r + 1, :].rearrange("r (t q) -> r t q", q=P),
                    in_=gq[:, :, 64 + r])
            for r in range(2):
                nc.sync.dma_start(
                    out=qsq[r:r + 1, :].rearrange("r (t q) -> r t q", q=P),
                    in_=gq[:, :, 67 + r])
            nc.scalar.activation(out=xyz1a[:], in_=qxyz[:], func=AF.Copy)
            nc.scalar.activation(out=sqn1[:], in_=qsq[:], func=AF.Copy,
                                 scale=-1.0)
            nc.scalar.activation(out=rhs2a[:], in_=qxyz[:], func=AF.Copy,
                                 scale=2.0)

            def sa1_select(qt):
                sl1 = slice(qt * P, (qt + 1) * P)
                lhsq = wk.tile([5, P], F16, tag="lhsq", name="lhsq")
                nc.scalar.activation(out=lhsq[0:3, :], in_=qxyz[:, sl1],
                                     func=AF.Copy, scale=2.0)
                nc.sync.dma_start(out=lhsq[3:5, :], in_=ones2_d[:])
                nqb = wk.tile([4, 512], F16, tag="nqb", name="nqb")
                nc.scalar.activation(
                    out=nqb[0:3, :].rearrange("r (j q) -> r j q", q=P),
                    in_=qxyz[:, sl1].unsqueeze(1).to_broadcast([3, 4, P]),
                    func=AF.Copy, scale=-1.0)
                nc.sync.dma_start(out=nqb[3:4, :], in_=ones1_d[:])

                candV = wk.tile([P, 128], F32, tag="candV", name="candV")
                candI = wk.tile([P, 128], U16, tag="candI", name="candI")
                for c in range(16):
                    pm = bigtile()
                    for hh in range(2):
                        nc.tensor.matmul(
                            out=pm[:, hh * 512:(hh + 1) * 512], lhsT=lhsq[:],
                            rhs=bigT[:, c * 1024 + hh * 512:
                                     c * 1024 + (hh + 1) * 512],
                            start=True, stop=True)
                    nc.vector.max(out=candV[:, c * 8:(c + 1) * 8], in_=pm[:])
                    nc.vector.max_index(out=candI[:, c * 8:(c + 1) * 8],
                                        in_max=candV[:, c * 8:(c + 1) * 8],
                                        in_values=pm[:])
                nc.vector.tensor_tensor(out=candI[:], in0=candI[:], in1=offs[:],
                                        op=ALU.add)
                candVw = wk.tile([P, 128], F32, tag="candVw", name="candVw")
                nc.vector.tensor_copy(out=candVw[:], in_=candV[:])
                selV = wk.tile([P, K1], F32, tag="selV", name="selV")
                for r in range(4):
                    rs = slice(r * 8, (r + 1) * 8)
                    nc.vector.max(out=selV[:, rs], in_=candVw[:])
                    if r < 3:
                        nc.vector.match_replace(out=candVw[:],
                                                in_to_replace=selV[:, rs],
                                                in_values=candVw[:],
                                                imm_value=NEG)
                mask = wk.tile([P, 128], U8, tag="selmask", name="selmask")
                nc.vector.tensor_scalar(out=mask[:], in0=candV[:],
                                        scalar1=selV[:, 31:32], scalar2=None,
                                        op0=ALU.is_ge)
                candIf = wk.tile([P, 128], F32, tag="candIf", name="candIf")
                nc.vector.tensor_copy(out=candIf[:], in_=candI[:])
                arr = wk.tile([P, 128], F32, tag="selarr", name="selarr")
                nc.vector.memset(arr[:], -1.0)
                nc.vector.copy_predicated(out=arr[:], mask=mask[:],
                                          data=candIf[:])
                selIf = wk.tile([P, K1], F32, tag="selIf", name="selIf")
                for r in range(4):
                    rs = slice(r * 8, (r + 1) * 8)
                    nc.vector.max(out=selIf[:, rs], in_=arr[:])
                    if r < 3:
                        nc.vector.match_replace(out=arr[:],
                                                in_to_replace=selIf[:, rs],
                                                in_values=arr[:],
                                                imm_value=-1.0)
                nbr16 = wk.tile([P, K1], I16, tag="nbr16", name="nbr16")
                nc.vector.tensor_copy(out=nbr16[:], in_=selIf[:])
                idxt = wrap_idx(wk, nbr16[:], ib1_ds[qt], "idxt1")
                gn = wk.tile([P, K1, 128], F16, tag="gn", name="gn",
                             bufs=3)
                for k in range(4):
                    nc.gpsimd.dma_gather(gn[:, k * 8:(k + 1) * 8, :],
                                         tab0_d[:],
                                         idxt[:, k * 64:(k + 1) * 64],
                                         1024, 1024, 128)
                return gn, nqb

            def sa1_mlp(qt, gn, nqb):
                sl1 = slice(qt * P, (qt + 1) * P)
                acc = wk.tile([P, P], F32, tag="sa1acc", name="sa1acc")
                for c in range(8):
                    pg = psT.tile([P, 512], F16, tag="trans", name="trans")
                    for j in range(4):
                        nc.tensor.matmul(out=pg[0:67, j * 128:(j + 1) * 128],
                                         lhsT=gn[:, c * 4 + j, 0:67],
                                         rhs=identh[:], is_transpose=True,
                                         start=True, stop=True)
                    gtc = wk.tile([67, 512], F16, tag="gtc", name="gtc",
                                  bufs=3)
                    nc.scalar.activation(out=gtc[:], in_=pg[0:67, :],
                                         func=AF.Copy)
                    pz = mmtile()
                    nc.tensor.matmul(out=pz[:], lhsT=w1aug[:], rhs=gtc[:],
                                     start=True, stop=False)
                    nc.tensor.matmul(out=pz[:], lhsT=w1q[:], rhs=nqb[:, 0:512],
                                     start=False, stop=True)
                    h1 = wk.tile([P, 512], F16, tag="h1", name="h1", bufs=3)
                    nc.scalar.activation(out=h1[:], in_=pz[:], func=AF.Relu)
                    pz2 = mmtile()
                    nc.tensor.matmul(out=pz2[:], lhsT=w2sb[:], rhs=h1[:],
                                     start=True, stop=True)
                    red = wk.tile([P, P], F32, tag="sa1red", name="sa1red",
                                  bufs=3)
                    nc.vector.tensor_reduce(
                        out=red[:],
                        in_=pz2[:].rearrange("f (s q) -> f q s", q=P),
                        axis=AX.X, op=ALU.max)
                    if c == 0:
                        nc.vector.tensor_copy(out=acc[:], in_=red[:])
                    else:
                        nc.vector.tensor_tensor(out=acc[:], in0=acc[:],
                                                in1=red[:], op=ALU.max)
                nc.scalar.activation(out=feat1T[:, sl1], in_=acc[:],
                                     func=AF.Relu, bias=b2sb[:])

            pend = []
            for qt in range(T1 + 2):
                if qt < T1:
                    pend.append((qt, sa1_select(qt)))
                if qt >= 2:
                    j, args = pend.pop(0)
                    sa1_mlp(j, *args)

            stage1 = wk.tile([P, T1, 256], F16, tag="stage1", name="stage1",
                             bufs=1)
            for t in range(T1):
                pf = trans16(feat1T[:, t * P:(t + 1) * P])
                nc.scalar.activation(out=stage1[:, t, 0:128], in_=pf,
                                     func=AF.Copy)
            nc.vector.tensor_copy(out=stage1[:, :, 128:133],
                                  in_=gq[:, :, 64:69])
            nc.vector.memset(stage1[:, :, 133:256], 0.0)
            nc.sync.dma_start(out=tab1_d.rearrange("(t p) c -> p t c", p=P),
                              in_=stage1[:])

        # ============ stage 3: SA2 ============
        with tc.tile_pool(name="sa2", bufs=2) as wk:
            gq2 = wk.tile([P, T2, 256], F16, tag="gq2", name="gq2", bufs=1)
            nc.gpsimd.dma_gather(gq2[:], tab1_d[:], i2sb[:], S2, S2, 256)
            sq2_pm = wk.tile([P, T2], F32, tag="sq2_pm", name="sq2_pm", bufs=1)
            nc.vector.tensor_tensor(out=sq2_pm[:], in0=gq2[:, :, 131],
                                    in1=gq2[:, :, 132], op=ALU.add)
            q2xyz = wk.tile([3, S2], F16, tag="q2xyz", name="q2xyz", bufs=1)
            q2sq = wk.tile([2, S2], F16, tag="q2sq", name="q2sq", bufs=1)
            for r in range(3):
                nc.sync.dma_start(
                    out=q2xyz[r:r + 1, :].rearrange("r (t q) -> r t q", q=P),
                    in_=gq2[:, :, 128 + r])
            for r in range(2):
                nc.sync.dma_start(
                    out=q2sq[r:r + 1, :].rearrange("r (t q) -> r t q", q=P),
                    in_=gq2[:, :, 131 + r])
            nc.scalar.activation(out=rhsF2a[:], in_=q2xyz[:],
                                 func=AF.Copy, scale=2.0)
            nc.scalar.activation(out=rhsF2b[:], in_=q2sq[:],
                                 func=AF.Copy, scale=-1.0)

            feat2T = [cst.tile([P, S2], F16, tag=f"feat2T{h}",
                               name=f"feat2T{h}") for h in range(2)]
            for t2 in range(T2):
                sl2 = slice(t2 * P, (t2 + 1) * P)
                lhsqB = wk.tile([3, P], F16, tag="lhsqB", name="lhsqB")
                nc.scalar.activation(out=lhsqB[:], in_=q2xyz[:, sl2],
                                     func=AF.Copy, scale=2.0)
                nqb2 = wk.tile([4, 512], F16, tag="nqb2", name="nqb2")
                nc.scalar.activation(
                    out=nqb2[0:3, :].rearrange("r (j q) -> r j q", q=P),
                    in_=q2xyz[:, sl2].unsqueeze(1).to_broadcast([3, 4, P]),
                    func=AF.Copy, scale=-1.0)
                nc.sync.dma_start(out=nqb2[3:4, :], in_=ones1_d[:])

                pm2 = bigtile()
                for hh in range(2):
                    hs = slice(hh * 512, (hh + 1) * 512)
                    nc.tensor.matmul(out=pm2[:, hs], lhsT=lhsqB[:],
                                     rhs=xyz1a[:, hs],
                                     start=True, stop=False)
                    nc.tensor.matmul(out=pm2[:, hs], lhsT=ones2[:],
                                     rhs=sqn1[:, hs],
                                     start=False, stop=True)
                selV2 = wk.tile([P, K2], F32, tag="selV2", name="selV2")
                selI2 = wk.tile([P, K2], U16, tag="selI2", name="selI2")
                for r in range(4):
                    rs = slice(r * 8, (r + 1) * 8)
                    nc.vector.max(out=selV2[:, rs], in_=pm2[:])
                    nc.vector.max_index(out=selI2[:, rs], in_max=selV2[:, rs],
                                        in_values=pm2[:])
                    if r < 3:
                        nc.vector.match_replace(out=pm2[:],
                                                in_to_replace=selV2[:, rs],
                                                in_values=pm2[:],
                                                imm_value=NEG)
                nbr2 = wk.tile([P, K2], I16, tag="nbr2", name="nbr2")
                nc.vector.tensor_copy(out=nbr2[:], in_=selI2[:])
                idxt2 = wrap_idx(wk, nbr2[:], ib2_ds[t2], "idxt2")
                gn2 = wk.tile([P, K2, 256], F16, tag="gn2", name="gn2")
                for k in range(4):
                    nc.gpsimd.dma_gather(gn2[:, k * 8:(k + 1) * 8, :],
                                         tab1_d[:],
                                         idxt2[:, k * 64:(k + 1) * 64],
                                         1024, 1024, 256)

                acc2 = [wk.tile([P, P], F32, tag=f"sa2acc{h}",
                                name=f"sa2acc{h}") for h in range(2)]
                for c in range(8):
                    pga = psT.tile([P, 512], F16, tag="trans", name="trans")
                    pgb = psT.tile([P, 512], F16, tag="trans", name="trans")
                    for j in range(4):
                        nc.tensor.matmul(out=pga[:, j * 128:(j + 1) * 128],
                                         lhsT=gn2[:, c * 4 + j, 0:128],
                                         rhs=identh[:], is_transpose=True,
                                         start=True, stop=True)
                        nc.tensor.matmul(out=pgb[0:3, j * 128:(j + 1) * 128],
                                         lhsT=gn2[:, c * 4 + j, 128:131],
                                         rhs=identh[:], is_transpose=True,
                                         start=True, stop=True)
                    gta = wk.tile([P, 512], F16, tag="gta", name="gta", bufs=3)
                    gtb = wk.tile([3, 512], F16, tag="gtb", name="gtb", bufs=3)
                    nc.scalar.activation(out=gta[:], in_=pga[:], func=AF.Copy)
                    nc.scalar.activation(out=gtb[:], in_=pgb[0:3, :],
                                         func=AF.Copy)
                    h1c = []
                    for h in range(2):
                        pz = mmtile()
                        nc.tensor.matmul(out=pz[:], lhsT=v1A[h][:], rhs=gta[:],
                                         start=True, stop=False)
                        nc.tensor.matmul(out=pz[:], lhsT=v1rel[h][:],
                                         rhs=gtb[:], start=False, stop=False)
                        nc.tensor.matmul(out=pz[:], lhsT=v1q[h][:],
                                         rhs=nqb2[:, 0:512],
                                         start=False, stop=True)
                        hh_ = wk.tile([P, 512], F16, tag=f"h1c{h}",
                                      name=f"h1c{h}", bufs=3)
                        nc.scalar.activation(out=hh_[:], in_=pz[:],
                                             func=AF.Relu)
                        h1c.append(hh_)
                    for h in range(2):
                        pz = mmtile()
                        nc.tensor.matmul(out=pz[:], lhsT=v2sb[0][h][:],
                                         rhs=h1c[0][:], start=True, stop=False)
                        nc.tensor.matmul(out=pz[:], lhsT=v2sb[1][h][:],
                                         rhs=h1c[1][:], start=False, stop=True)
                        red = wk.tile([P, P], F32, tag="sa2red",
                                      name="sa2red", bufs=3)
                        nc.vector.tensor_reduce(
                            out=red[:],
                            in_=pz[:].rearrange("f (s q) -> f q s", q=P),
                            axis=AX.X, op=ALU.max)
                        if c == 0:
                            nc.vector.tensor_copy(out=acc2[h][:], in_=red[:])
                        else:
                            nc.vector.tensor_tensor(out=acc2[h][:],
                                                    in0=acc2[h][:],
                                                    in1=red[:], op=ALU.max)
                for h in range(2):
                    nc.scalar.activation(out=feat2T[h][:, sl2],
                                         in_=acc2[h][:], func=AF.Relu,
                                         bias=c2sb[:, h:h + 1])

            stage2 = wk.tile([P, T2, 256], F16, tag="stage2", name="stage2",
                             bufs=1)
            for t2 in range(T2):
                for h in range(2):
                    pf = trans16(feat2T[h][:, t2 * P:(t2 + 1) * P])
                    nc.scalar.activation(
                        out=stage2[:, t2, h * 128:(h + 1) * 128], in_=pf,
                        func=AF.Copy)
            nc.sync.dma_start(out=tab2_d.rearrange("(t p) c -> p t c", p=P),
                              in_=stage2[:])

        # ============ stage 4: FP2 ============
        with tc.tile_pool(name="fp2", bufs=2) as wk:
            v8f = wk.tile([P, T1, 8], F32, tag="v8f", name="v8f", bufs=1)
            p8f = wk.tile([P, T1, 8], U16, tag="p8f", name="p8f", bufs=1)
            for qt in range(T1):
                pm3 = mmtile()
                nc.tensor.matmul(out=pm3[:, 0:S2],
                                 lhsT=xyz1a[:, qt * P:(qt + 1) * P],
                                 rhs=rhsF2a[:], start=True, stop=False)
                nc.tensor.matmul(out=pm3[:, 0:S2], lhsT=ones2[:],
                                 rhs=rhsF2b[:], start=False, stop=True)
                nc.vector.max(out=v8f[:, qt, :], in_=pm3[:, 0:S2])
                nc.vector.max_index(out=p8f[:, qt, :], in_max=v8f[:, qt, :],
                                    in_values=pm3[:, 0:S2])

            # inverse-distance weights
            d2f = wk.tile([P, T1, 3], F32, tag="d2f", name="d2f", bufs=1)
            nc.vector.tensor_tensor(
                out=d2f[:], in0=sq1_pm[:].unsqueeze(2).to_broadcast([P, T1, 3]),
                in1=v8f[:, :, 0:3], op=ALU.subtract)
            nc.scalar.activation(out=d2f[:], in_=d2f[:], func=AF.Relu)
            nc.scalar.activation(out=d2f[:], in_=d2f[:], func=AF.Sqrt)
            nc.vector.tensor_scalar_max(d2f[:], d2f[:], 1e-10)
            wn2 = wk.tile([P, T1, 3], F32, tag="wn2", name="wn2", bufs=1)
            nc.vector.reciprocal(out=wn2[:], in_=d2f[:])
            ws2 = wk.tile([P, T1], F32, tag="ws2", name="ws2", bufs=1)
            nc.vector.tensor_reduce(out=ws2[:], in_=wn2[:], axis=AX.X,
                                    op=ALU.add)
            nc.vector.reciprocal(out=ws2[:], in_=ws2[:])
            nc.vector.tensor_tensor(
                out=wn2[:], in0=wn2[:],
                in1=ws2[:].unsqueeze(2).to_broadcast([P, T1, 3]), op=ALU.mult)
            wn2h = wk.tile([P, T1, 3], F16, tag="wn2h", name="wn2h", bufs=1)
            nc.vector.tensor_copy(out=wn2h[:], in_=wn2[:])

            p3f = wk.tile([P, T1 * 3], I16, tag="p3f", name="p3f", bufs=1)
            nc.vector.tensor_copy(out=p3f[:].rearrange("p (t j) -> p t j", j=3),
                                  in_=p8f[:, :, 0:3])
            idxtf2 = wrap_idx(wk, p3f[:], ibf2_d, "idxtf2")
            gi2 = wk.tile([P, T1, 3, 256], F16, tag="gi2", name="gi2", bufs=1)
            gi2v = gi2[:].rearrange("p t j c -> p (t j) c")
            for k in range(3):
                nc.gpsimd.dma_gather(gi2v[:, k * 8:(k + 1) * 8, :], tab2_d[:],
                                     idxtf2[:, k * 64:(k + 1) * 64],
                                     1024, 1024, 256)
            # weighted 3-NN sum: t0*w0 + t1*w1 + t2*w2
            tmp0 = wk.tile([P, T1, 256], F16, tag="tmp0", name="tmp0", bufs=1)
            nc.vector.tensor_tensor(
                out=tmp0[:], in0=gi2[:, :, 0, :],
                in1=wn2h[:, :, 0:1].to_broadcast([P, T1, 256]), op=ALU.mult)
            tmp1 = wk.tile([P, T1, 256], F16, tag="tmp1", name="tmp1", bufs=1)
            nc.vector.tensor_tensor(
                out=tmp1[:], in0=gi2[:, :, 1, :],
                in1=wn2h[:, :, 1:2].to_broadcast([P, T1, 256]), op=ALU.mult)
            nc.vector.tensor_tensor(out=tmp0[:], in0=tmp0[:], in1=tmp1[:],
                                    op=ALU.add)
            nc.vector.tensor_tensor(
                out=tmp1[:], in0=gi2[:, :, 2, :],
                in1=wn2h[:, :, 2:3].to_broadcast([P, T1, 256]), op=ALU.mult)
            it2 = wk.tile([P, T1, 256], F16, tag="it2", name="it2", bufs=1)
            nc.vector.tensor_tensor(out=it2[:], in0=tmp0[:], in1=tmp1[:],
                                    op=ALU.add)
            itT2 = [wk.tile([P, S1], F16, tag=f"itT2{h}", name=f"itT2{h}",
                            bufs=1) for h in range(2)]
            for t in range(T1):
                for h in range(2):
                    pf = trans16(it2[:, t, h * 128:(h + 1) * 128])
                    nc.scalar.activation(out=itT2[h][:, t * P:(t + 1) * P],
                                         in_=pf, func=AF.Copy)
            for c in range(2):
                cs = slice(c * 512, (c + 1) * 512)
                pz = mmtile()
                nc.tensor.matmul(out=pz[:], lhsT=f2w1[0][:], rhs=feat1T[:, cs],
                                 start=True, stop=False)
                nc.tensor.matmul(out=pz[:], lhsT=f2w1[1][:], rhs=itT2[0][:, cs],
                                 start=False, stop=False)
                nc.tensor.matmul(out=pz[:], lhsT=f2w1[2][:], rhs=itT2[1][:, cs],
                                 start=False, stop=True)
                hf = wk.tile([P, 512], F16, tag="fp2h", name="fp2h", bufs=3)
                nc.scalar.activation(out=hf[:], in_=pz[:], func=AF.Relu,
                                     bias=f2b1[:])
                pz2 = mmtile()
                nc.tensor.matmul(out=pz2[:], lhsT=f2w2[:], rhs=hf[:],
                                 start=True, stop=True)
                nc.scalar.activation(out=f1upT[:, cs], in_=pz2[:], func=AF.Relu,
                                     bias=f2b2[:])
            stagef = wk.tile([P, T1, 128], F16, tag="stagef", name="stagef",
                             bufs=1)
            for t in range(T1):
                pf = trans16(f1upT[:, t * P:(t + 1) * P])
                nc.scalar.activation(out=stagef[:, t, :], in_=pf, func=AF.Copy)
            nc.sync.dma_start(out=tabf_d.rearrange("(t p) c -> p t c", p=P),
                              in_=stagef[:])

        # ============ stage 5: FP1 (16 pipelined groups) ============
        with tc.tile_pool(name="fp1", bufs=2) as wk:
            def fp1_select(g):
                v81 = wk.tile([P, GT, 8], F32, tag="v81", name="v81")
                p81 = wk.tile([P, GT, 8], U16, tag="p81", name="p81")
                for j in range(GT):
                    qt = g * GT + j
                    pm4 = bigtile()
                    for hh in range(2):
                        hs = slice(hh * 512, (hh + 1) * 512)
                        nc.tensor.matmul(out=pm4[:, hs],
                                         lhsT=bigT[0:3, qt * P:(qt + 1) * P],
                                         rhs=rhs2a[:, hs],
                                         start=True, stop=False)
                        nc.tensor.matmul(out=pm4[:, hs], lhsT=ones2[:],
                                         rhs=sqn1[:, hs],
                                         start=False, stop=True)
                    nc.vector.max(out=v81[:, j, :], in_=pm4[:])
                    nc.vector.max_index(out=p81[:, j, :], in_max=v81[:, j, :],
                                        in_values=pm4[:])

                sqs = sq_pm[:, g * GT:(g + 1) * GT]
                d21 = wk.tile([P, GT, 3], F32, tag="d21", name="d21")
                nc.vector.tensor_tensor(
                    out=d21[:], in0=sqs.unsqueeze(2).to_broadcast([P, GT, 3]),
                    in1=v81[:, :, 0:3], op=ALU.subtract)
                nc.scalar.activation(out=d21[:], in_=d21[:], func=AF.Relu)
                nc.scalar.activation(out=d21[:], in_=d21[:], func=AF.Sqrt)
                nc.vector.tensor_scalar_max(d21[:], d21[:], 1e-10)
                wn1 = wk.tile([P, GT, 3], F32, tag="wn1", name="wn1")
                nc.vector.reciprocal(out=wn1[:], in_=d21[:])
                ws1 = wk.tile([P, GT], F32, tag="ws1", name="ws1")
                nc.vector.tensor_reduce(out=ws1[:], in_=wn1[:], axis=AX.X,
                                        op=ALU.add)
                nc.vector.reciprocal(out=ws1[:], in_=ws1[:])
                nc.vector.tensor_tensor(
                    out=wn1[:], in0=wn1[:],
                    in1=ws1[:].unsqueeze(2).to_broadcast([P, GT, 3]),
                    op=ALU.mult)
                wn1h = wk.tile([P, GT, 3], F16, tag="wn1h", name="wn1h",
                               bufs=3)
                nc.vector.tensor_copy(out=wn1h[:], in_=wn1[:])

                p31 = wk.tile([P, GT * 3], I16, tag="p31", name="p31")
                nc.vector.tensor_copy(
                    out=p31[:].rearrange("p (t j) -> p t j", j=3),
                    in_=p81[:, :, 0:3])
                idxtf1 = wrap_idx(wk, p31[:], ibf1_ds[g], "idxtf1")
                gi1 = wk.tile([P, GT, 3, 128], F16, tag="gi1", name="gi1",
                              bufs=3)
                gi1v = gi1[:].rearrange("p t j c -> p (t j) c")
                for k in range(3):
                    nc.gpsimd.dma_gather(gi1v[:, k * 8:(k + 1) * 8, :],
                                         tabf_d[:],
                                         idxtf1[:, k * 64:(k + 1) * 64],
                                         1024, 1024, 128)
                return gi1, wn1h

            def fp1_post(g, gi1, wn1h):
                ta = wk.tile([P, GT, 128], F16, tag="ta", name="ta")
                nc.vector.tensor_tensor(
                    out=ta[:], in0=gi1[:, :, 0, :],
                    in1=wn1h[:, :, 0:1].to_broadcast([P, GT, 128]),
                    op=ALU.mult)
                tb = wk.tile([P, GT, 128], F16, tag="tb", name="tb")
                nc.vector.tensor_tensor(
                    out=tb[:], in0=gi1[:, :, 1, :],
                    in1=wn1h[:, :, 1:2].to_broadcast([P, GT, 128]),
                    op=ALU.mult)
                nc.vector.tensor_tensor(out=ta[:], in0=ta[:], in1=tb[:],
                                        op=ALU.add)
                nc.vector.tensor_tensor(
                    out=tb[:], in0=gi1[:, :, 2, :],
                    in1=wn1h[:, :, 2:3].to_broadcast([P, GT, 128]),
                    op=ALU.mult)
                it1 = wk.tile([P, GT, 128], F16, tag="it1", name="it1")
                nc.vector.tensor_tensor(out=it1[:], in0=ta[:], in1=tb[:],
                                        op=ALU.add)
                itT1 = wk.tile([P, GT * 128], F16, tag="itT1", name="itT1")
                for t in range(GT):
                    pf = trans16(it1[:, t, :])
                    nc.scalar.activation(out=itT1[:, t * P:(t + 1) * P],
                                         in_=pf, func=AF.Copy)
                for c in range(2):
                    cs = slice(c * 512, (c + 1) * 512)
                    gcs = slice(g * GT * P + c * 512,
                                g * GT * P + (c + 1) * 512)
                    pz = mmtile()
                    nc.tensor.matmul(out=pz[:], lhsT=f1w1b[:], rhs=itT1[:, cs],
                                     start=True, stop=False)
                    nc.tensor.matmul(out=pz[:], lhsT=f1w1a[:],
                                     rhs=f0TS[:, gcs], start=False, stop=True)
                    hf = wk.tile([P, 512], F16, tag="fp1h", name="fp1h",
                                 bufs=3)
                    nc.scalar.activation(out=hf[:], in_=pz[:], func=AF.Relu,
                                         bias=f1b1[:])
                    pz2 = mmtile()
                    nc.tensor.matmul(out=pz2[:], lhsT=f1w2[:], rhs=hf[:],
                                     start=True, stop=True)
                    nc.scalar.activation(out=fuTS[:, gcs], in_=pz2[:],
                                         func=AF.Relu, bias=f1b2[:])
                    red = wk.tile([P, 1], F32, tag="gfred", name="gfred",
                                  bufs=3)
                    nc.vector.tensor_reduce(out=red[:], in_=fuTS[:, gcs],
                                            axis=AX.X, op=ALU.max)
                    if g == 0 and c == 0:
                        nc.vector.tensor_copy(out=gfacc[:], in_=red[:])
                    else:
                        nc.vector.tensor_tensor(out=gfacc[:], in0=gfacc[:],
                                                in1=red[:], op=ALU.max)

            fpend = []
            for g in range(NG + 2):
                if g < NG:
                    fpend.append((g, fp1_select(g)))
                if g >= 2:
                    j, args = fpend.pop(0)
                    fp1_post(j, *args)

        # ============ stage 6: head ============
        with tc.tile_pool(name="head", bufs=2) as wk:
            gfh = wk.tile([P, 1], F16, tag="gfh", name="gfh", bufs=1)
            nc.vector.tensor_copy(out=gfh[:], in_=gfacc[:])
            pc = mmtile()
            nc.tensor.matmul(out=pc[:, 0:1], lhsT=hw1b[:], rhs=gfh[:],
                             start=True, stop=True)
            nc.vector.tensor_tensor(out=biasH[:], in0=pc[:, 0:1], in1=hb1[:],
                                    op=ALU.add)
            for g in range(8):
                ostage = wk.tile([P, 16, 13], F32, tag="ostage", name="ostage")
                for c4 in range(4):
                    c = g * 4 + c4
                    cs = slice(c * 512, (c + 1) * 512)
                    pz = mmtile()
                    nc.tensor.matmul(out=pz[:], lhsT=hw1a[:], rhs=fuTS[:, cs],
                                     start=True, stop=True)
                    h1 = wk.tile([P, 512], F16, tag="hh1", name="hh1", bufs=3)
                    nc.scalar.activation(out=h1[:], in_=pz[:], func=AF.Relu,
                                         bias=biasH[:])
                    pz2 = mmtile()
                    nc.tensor.matmul(out=pz2[:64, :], lhsT=hw2[:], rhs=h1[:],
                                     start=True, stop=True)
                    h2 = wk.tile([64, 512], F16, tag="hh2", name="hh2", bufs=3)
                    nc.scalar.activation(out=h2[:], in_=pz2[:64, :],
                                         func=AF.Relu, bias=hb2[:])
                    pz3 = mmtile()
                    nc.tensor.matmul(out=pz3[:13, :], lhsT=hw3[:], rhs=h2[:],
                                     start=True, stop=True)
                    oT = wk.tile([13, 512], F16, tag="hoT", name="hoT", bufs=3)
                    nc.vector.tensor_tensor(
                        out=oT[:], in0=pz3[:13, :],
                        in1=hb3[:, 0:1].to_broadcast([13, 512]), op=ALU.add)
                    po = psT.tile([P, 512], F16, tag="trans", name="trans")
                    for t in range(4):
                        nc.tensor.matmul(out=po[:, t * 16:t * 16 + 13],
                                         lhsT=oT[:, t * 128:(t + 1) * 128],
                                         rhs=identh[0:13, 0:13],
                                         is_transpose=True, start=True,
                                         stop=True)
                    nc.scalar.activation(
                        out=ostage[:, c4 * 4:(c4 + 1) * 4, :],
                        in_=po[:, 0:64].rearrange(
                            "p (t c) -> p t c", c=16)[:, :, 0:13],
                        func=AF.Copy)
                nc.sync.dma_start(
                    out=out_d.rearrange("(t p) c -> p t c", p=P)[
                        :, g * 16:(g + 1) * 16, :],
                    in_=ostage[:])

    return nc


# ---------------------------------------------------------------- host side
_CACHED_NC = None


def _get_nc():
    global _CACHED_NC
    if _CACHED_NC is None:
        nc = build_nc()
        nc.finalize()
        _CACHED_NC = nc
    return _CACHED_NC


def _per_core_inputs(b, inputs):
    x = np.asarray(inputs["x"][b]).astype(np.float16)
    i1 = np.asarray(inputs["idx_s1"][b]).astype(np.int16)
    i1w = np.tile(i1.reshape(S1 // 16, 16).T, (8, 1))
    i2 = np.asarray(inputs["idx_s2"][b]).astype(np.int16)
    i2w = np.tile(i2.reshape(S2 // 16, 16).T, (8, 1))
    f16 = lambda a: np.ascontiguousarray(np.asarray(a), dtype=np.float16)
    f32 = lambda a: np.ascontiguousarray(np.asarray(a), dtype=np.float32)
    return {
        "xh": np.ascontiguousarray(x),
        "xTh": np.ascontiguousarray(x.T),
        "i1w": np.ascontiguousarray(i1w),
        "i2w": np.ascontiguousarray(i2w),
        "embw": f16(inputs["embed_w"]),
        "embb": f32(inputs["embed_b"]).reshape(64, 1),
        "w1": f16(inputs["sa1_w1"]),
        "b1r": f16(inputs["sa1_b1"]).reshape(1, 128),
        "w2": f16(inputs["sa1_w2"]),
        "b2": f32(inputs["sa1_b2"]).reshape(128, 1),
        "v1": f16(inputs["sa2_w1"]),
        "c1r": f16(inputs["sa2_b1"]).reshape(1, 256),
        "v2": f16(inputs["sa2_w2"]),
        "c2": np.ascontiguousarray(f32(inputs["sa2_b2"]).reshape(2, 128).T),
        "f2w1": f16(inputs["fp2_w1"]),
        "f2b1": f32(inputs["fp2_b1"]).reshape(128, 1),
        "f2w2": f16(inputs["fp2_w2"]),
        "f2b2": f32(inputs["fp2_b2"]).reshape(128, 1),
        "f1w1": f16(inputs["fp1_w1"]),
        "f1b1": f32(inputs["fp1_b1"]).reshape(128, 1),
        "f1w2": f16(inputs["fp1_w2"]),
        "f1b2": f32(inputs["fp1_b2"]).reshape(128, 1),
        "hw1": f16(inputs["head_w1"]),
        "hb1": f32(inputs["head_b1"]).reshape(128, 1),
        "hw2": f16(inputs["head_w2"]),
        "hb2": f32(inputs["head_b2"]).reshape(64, 1),
        "hw3": f16(inputs["head_w3"]),
        "hb3": f32(inputs["head_b3"]).reshape(13, 1),
    }


def run(inputs, trace=False, **kw):
    nc = _get_nc()
    B = inputs["x"].shape[0]
    in_maps = [_per_core_inputs(b, inputs) for b in range(B)]
    res = run_bass_kernel_spmd(nc, in_maps, core_ids=list(range(B)),
                               trace=trace, **kw)
    out = np.stack([res.results[b]["out"] for b in range(B)])
    return out, res


def kernel(**inputs):
    return run(inputs)[0]


if __name__ == "__main__":
    build_nc()
    print("built ok")


# revision 17
# speedup vs baseline: 1.8365x; 1.0136x over previous
"""PointNet++-lite segmentation on 8 Trainium2 cores (batch-parallel, one
point cloud per core). Self-contained: hardcodes shapes from the problem spec.

Per-core pipeline (all on device):
  embed MLP -> SA1 (KNN top-32 of 16384, gather, 2-layer MLP, max-pool)
  -> SA2 (KNN top-32 of 1024) -> FP2/FP1 (3-NN inverse-distance interp)
  -> global-max head MLP -> (16384, 13) logits.

fp16 datapath: all PE matmuls run on fp16 operands (4x the fp32 rate), with
fp32 PSUM accumulation.  KNN ranking uses m = 2 q.x - |x|^2 (row-constant
|q|^2 dropped); |x|^2 enters the fp16 matmul split into hi+lo fp16 halves so
m keeps ~22 mantissa bits (self-distances stay ~0, exact inverse-distance
weights).  Neighbor tables are fp16 rows in DRAM; SA1/SA2 gathers use
dma_gather transpose mode which lands features on partitions, removing the
per-neighbor PE transposes.  Selection runs on the vector engine max8 /
max_index over 1024-wide PSUM blocks.
"""

from contextlib import ExitStack

import numpy as np

import concourse.bass as bass
import concourse.mybir as mybir
from concourse.bacc import Bacc
from concourse.bass_utils import run_bass_kernel_spmd
from concourse.masks import make_identity
from concourse.tile import TileContext

F32 = mybir.dt.float32
F16 = mybir.dt.float16
U16 = mybir.dt.uint16
U8 = mybir.dt.uint8
I16 = mybir.dt.int16
AF = mybir.ActivationFunctionType
ALU = mybir.AluOpType
AX = mybir.AxisListType

P = 128
N = 16384
S1, K1 = 1024, 32
S2, K2 = 256, 32
NCLS = 13
NEG = -3.0e38

NT = N // P        # 128 point tiles
T1 = S1 // P       # 8 SA1 query tiles
T2 = S2 // P       # 2 SA2 query tiles
NG = 16            # FP1 groups (8 tiles each)
GT = NT // NG      # tiles per FP1 group


def build_nc():
    nc = Bacc()

    xh_in = nc.dram_tensor("xh", [N, 6], F16, kind="ExternalInput")
    xTh_in = nc.dram_tensor("xTh", [6, N], F16, kind="ExternalInput")
    i1_in = nc.dram_tensor("i1w", [P, S1 // 16], I16, kind="ExternalInput")
    i2_in = nc.dram_tensor("i2w", [P, S2 // 16], I16, kind="ExternalInput")
    wdecl16 = [
        ("embw", [6, 64]),
        ("w1", [67, 128]), ("b1r", [1, 128]), ("w2", [128, 128]),
        ("v1", [131, 256]), ("c1r", [1, 256]), ("v2", [256, 256]),
        ("f2w1", [384, 128]), ("f2w2", [128, 128]),
        ("f1w1", [192, 128]), ("f1w2", [128, 128]),
        ("hw1", [256, 128]), ("hw2", [128, 64]), ("hw3", [64, 13]),
    ]
    wdecl32 = [
        ("embb", [64, 1]), ("b2", [128, 1]), ("c2", [128, 2]),
        ("f2b1", [128, 1]), ("f2b2", [128, 1]),
        ("f1b1", [128, 1]), ("f1b2", [128, 1]),
        ("hb1", [128, 1]), ("hb2", [64, 1]), ("hb3", [13, 1]),
    ]
    din = {nm: nc.dram_tensor(nm, sh, F16, kind="ExternalInput")
           for nm, sh in wdecl16}
    din.update({nm: nc.dram_tensor(nm, sh, F32, kind="ExternalInput")
                for nm, sh in wdecl32})
    out_d = nc.dram_tensor("out", [N, NCLS], F32, kind="ExternalOutput")

    tab0_d = nc.dram_tensor("tab0", [N, 128], F16)
    tab1_d = nc.dram_tensor("tab1", [S1, 256], F16)
    tab2_d = nc.dram_tensor("tab2", [S2, 256], F16)
    tabf_d = nc.dram_tensor("tabf", [S1, 128], F16)
    ib1_ds = [nc.dram_tensor(f"ib1_{t}", [16, 256], I16) for t in range(T1)]
    ib2_ds = [nc.dram_tensor(f"ib2_{t}", [16, 256], I16) for t in range(T2)]
    ibf2_d = nc.dram_tensor("ibf2", [16, 192], I16)
    ones2_d = nc.dram_tensor("ones2_d", [2, 128], F16)
    ones1_d = nc.dram_tensor("ones1_d", [1, 512], F16)
    ibf1_ds = [nc.dram_tensor(f"ibf1_{g}", [16, 192], I16) for g in range(NG)]

    with TileContext(nc) as tc, ExitStack() as ctx:
        cst = ctx.enter_context(tc.tile_pool(name="cst", bufs=1))
        psB = ctx.enter_context(tc.tile_pool(name="psB", bufs=2, space="PSUM"))
        psA = ctx.enter_context(tc.tile_pool(name="psA", bufs=2, space="PSUM"))
        psT = ctx.enter_context(tc.tile_pool(name="psT", bufs=2, space="PSUM"))

        identh = cst.tile([P, P], F16, tag="identh", name="identh")
        make_identity(nc, identh[:])

        def bigtile():
            return psB.tile([P, 1024], F32, tag="big", name="big")

        def mmtile():
            return psA.tile([P, 512], F32, tag="mm", name="mm")

        def trans16(in_ap):
            """PE transpose of fp16 data: in_(p,f) -> fp16 psum (f,p)."""
            pt = psT.tile([P, 512], F16, tag="trans", name="trans")
            k = in_ap.shape[0]
            f = in_ap.shape[-1]
            nc.tensor.matmul(out=pt[:f, :k], lhsT=in_ap, rhs=identh[:k, :k],
                             is_transpose=True, start=True, stop=True)
            return pt[:f, :k]


        # ---------------- constants / weights ----------------
        def load(name, src, shape, dtype=F16):
            t = cst.tile(list(shape), dtype, tag=name, name=name)
            nc.sync.dma_start(out=t[:], in_=src)
            return t

        embw = load("embw", din["embw"][:], (6, 64))
        embb = load("embb", din["embb"][:], (64, 1), F32)
        w2sb = load("w2sb", din["w2"][:], (128, 128))
        b2sb = load("b2sb", din["b2"][:], (128, 1), F32)
        c2sb = load("c2sb", din["c2"][:], (128, 2), F32)
        f2b1 = load("f2b1", din["f2b1"][:], (128, 1), F32)
        f2w2 = load("f2w2", din["f2w2"][:], (128, 128))
        f2b2 = load("f2b2", din["f2b2"][:], (128, 1), F32)
        f1b1 = load("f1b1", din["f1b1"][:], (128, 1), F32)
        f1w2 = load("f1w2", din["f1w2"][:], (128, 128))
        f1b2 = load("f1b2", din["f1b2"][:], (128, 1), F32)
        hb1 = load("hb1", din["hb1"][:], (128, 1), F32)
        hw2 = load("hw2", din["hw2"][:], (128, 64))
        hb2 = load("hb2", din["hb2"][:], (64, 1), F32)
        hw3 = load("hw3", din["hw3"][:], (64, 13))
        hb3 = load("hb3", din["hb3"][:], (13, 1), F32)

        # SA1 grouped-MLP weights: rows [feat(64), rel_xyz(3)]
        w1aug = cst.tile([67, 128], F16, tag="w1aug", name="w1aug")
        nc.sync.dma_start(out=w1aug[0:64, :], in_=din["w1"][3:67, :])
        nc.sync.dma_start(out=w1aug[64:67, :], in_=din["w1"][0:3, :])
        w1q = cst.tile([4, 128], F16, tag="w1q", name="w1q")
        nc.sync.dma_start(out=w1q[0:3, :], in_=din["w1"][0:3, :])
        nc.sync.dma_start(out=w1q[3:4, :], in_=din["b1r"][:])

        v1A = [load(f"v1A{h}", din["v1"][3:131, h * 128:(h + 1) * 128],
                    (128, 128)) for h in range(2)]
        v1rel, v1q = [], []
        for h in range(2):
            sl = slice(h * 128, (h + 1) * 128)
            t = cst.tile([3, 128], F16, tag=f"v1rel{h}", name=f"v1rel{h}")
            nc.sync.dma_start(out=t[:], in_=din["v1"][0:3, sl])
            v1rel.append(t)
            t2 = cst.tile([4, 128], F16, tag=f"v1q{h}", name=f"v1q{h}")
            nc.sync.dma_start(out=t2[0:3, :], in_=din["v1"][0:3, sl])
            nc.sync.dma_start(out=t2[3:4, :], in_=din["c1r"][0:1, sl])
            v1q.append(t2)
        v2sb = [[load(f"v2{k}{h}",
                      din["v2"][k * 128:(k + 1) * 128, h * 128:(h + 1) * 128],
                      (128, 128)) for h in range(2)] for k in range(2)]
        f2w1 = [load(f"f2w1{k}", din["f2w1"][k * 128:(k + 1) * 128, :],
                     (128, 128)) for k in range(3)]
        f1w1a = load("f1w1a", din["f1w1"][0:64, :], (64, 128))
        f1w1b = load("f1w1b", din["f1w1"][64:192, :], (128, 128))
        hw1a = load("hw1a", din["hw1"][0:128, :], (128, 128))
        hw1b = load("hw1b", din["hw1"][128:256, :], (128, 128))
        i1sb = load("i1sb", i1_in[:], (P, S1 // 16), I16)
        i2sb = load("i2sb", i2_in[:], (P, S2 // 16), I16)

        # block offsets for 1024-wide max8 blocks: j*1024, 8 copies each
        offs = cst.tile([P, 128], U16, tag="offs", name="offs")
        nc.gpsimd.iota(out=offs[:], pattern=[[1024, 16], [0, 8]], base=0,
                       channel_multiplier=0)

        ones2 = cst.tile([2, P], F16, tag="ones2", name="ones2")
        nc.vector.memset(ones2[:], 1.0)
        nc.sync.dma_start(out=ones2_d[:], in_=ones2[:])
        onesr = cst.tile([1, 512], F16, tag="onesr", name="onesr")
        nc.vector.memset(onesr[:], 1.0)
        nc.sync.dma_start(out=ones1_d[:], in_=onesr[:])

        def wrap_idx(pool, src_i16, bounce_d, tag):
            """src (128, M) i16 -> replicated wrapped idx tile (128, 8*M)
            via DRAM bounce.  Flat gather slot j*128+q reads src[q, j]."""
            M = src_i16.shape[-1]
            bw = bounce_d[:, :].rearrange("c (j e) -> c j e", e=8)
            for ph in range(8):
                nc.sync.dma_start(out=bw[:, 0:M, ph],
                                  in_=src_i16[ph * 16:(ph + 1) * 16, :])
            idxt = pool.tile([P, 8 * M], I16, tag=tag, name=tag)
            for g in range(8):
                nc.sync.dma_start(out=idxt[g * 16:(g + 1) * 16, :],
                                  in_=bounce_d[:, :])
            return idxt

        # persistent cross-stage tensors
        sq_pm = cst.tile([P, NT], F32, tag="sq_pm", name="sq_pm")
        bigT = cst.tile([5, N], F16, tag="bigT", name="bigT")
        f0TS = cst.tile([64, N], F16, tag="f0TS", name="f0TS")
        fuTS = cst.tile([P, N], F16, tag="fuTS", name="fuTS")
        gq = cst.tile([P, T1, 128], F16, tag="gq", name="gq")
        xyz1a = cst.tile([3, S1], F16, tag="xyz1a", name="xyz1a")
        sqn1 = cst.tile([2, S1], F16, tag="sqn1", name="sqn1")
        rhs2a = cst.tile([3, S1], F16, tag="rhs2a", name="rhs2a")
        rhsF2a = cst.tile([3, S2], F16, tag="rhsF2a", name="rhsF2a")
        rhsF2b = cst.tile([2, S2], F16, tag="rhsF2b", name="rhsF2b")
        sq1_pm = cst.tile([P, T1], F32, tag="sq1_pm", name="sq1_pm")
        feat1T = cst.tile([P, S1], F16, tag="feat1T", name="feat1T")
        f1upT = cst.tile([P, S1], F16, tag="f1upT", name="f1upT")
        gfacc = cst.tile([P, 1], F32, tag="gfacc", name="gfacc")
        biasH = cst.tile([P, 1], F32, tag="biasH", name="biasH")

        # ============ stage 0+1: geometry, embed, tab0 ============
        with tc.tile_pool(name="st01", bufs=2) as wk:
            xzh = wk.tile([P, NT, 6], F16, tag="xzh", name="xzh", bufs=1)
            nc.sync.dma_start(
                out=xzh[:], in_=xh_in.rearrange("(t p) c -> p t c", p=P))
            sqt = wk.tile([P, NT, 3], F32, tag="sqt", name="sqt", bufs=1)
            nc.vector.tensor_tensor(out=sqt[:], in0=xzh[:, :, 0:3],
                                    in1=xzh[:, :, 0:3], op=ALU.mult)
            nc.vector.tensor_reduce(out=sq_pm[:], in_=sqt[:], axis=AX.X,
                                    op=ALU.add)
            # split |x|^2 into fp16 hi+lo halves (negated for the m matmul)
            hi16 = wk.tile([P, NT], F16, tag="hi16", name="hi16", bufs=1)
            nc.vector.tensor_copy(out=hi16[:], in_=sq_pm[:])
            hi32 = wk.tile([P, NT], F32, tag="hi32", name="hi32", bufs=1)
            nc.vector.tensor_copy(out=hi32[:], in_=hi16[:])
            lo32 = wk.tile([P, NT], F32, tag="lo32", name="lo32", bufs=1)
            nc.vector.tensor_tensor(out=lo32[:], in0=sq_pm[:], in1=hi32[:],
                                    op=ALU.subtract)
            lo16 = wk.tile([P, NT], F16, tag="lo16", name="lo16", bufs=1)
            nc.vector.tensor_copy(out=lo16[:], in_=lo32[:])
            nhi = wk.tile([P, NT], F16, tag="nhi", name="nhi", bufs=1)
            nc.scalar.activation(out=nhi[:], in_=hi16[:], func=AF.Copy,
                                 scale=-1.0)
            nlo = wk.tile([P, NT], F16, tag="nlo", name="nlo", bufs=1)
            nc.scalar.activation(out=nlo[:], in_=lo16[:], func=AF.Copy,
                                 scale=-1.0)

            nc.sync.dma_start(out=bigT[0:3, :], in_=xTh_in[0:3, :])
            nc.sync.dma_start(
                out=bigT[3:4, :].rearrange("r (t p) -> r t p", p=P),
                in_=nhi[:])
            nc.sync.dma_start(
                out=bigT[4:5, :].rearrange("r (t p) -> r t p", p=P),
                in_=nlo[:])

            for g in range(16):
                stage = wk.tile([P, 8, 128], F16, tag="tab0stage",
                                name="tab0stage")
                sl8 = slice(g * 8, (g + 1) * 8)
                nc.vector.tensor_copy(out=stage[:, :, 64:67],
                                      in_=xzh[:, sl8, 0:3])
                nc.vector.tensor_copy(out=stage[:, :, 67:68],
                                      in_=hi16[:, sl8].unsqueeze(2))
                nc.vector.tensor_copy(out=stage[:, :, 68:69],
                                      in_=lo16[:, sl8].unsqueeze(2))
                nc.vector.memset(stage[:, :, 69:128], 0.0)
                for cc in range(2):
                    c = g * 2 + cc
                    xc = wk.tile([6, 512], F16, tag="xc", name="xc", bufs=3)
                    nc.sync.dma_start(out=xc[:],
                                      in_=xTh_in[:, c * 512:(c + 1) * 512])
                    pe = mmtile()
                    nc.tensor.matmul(out=pe[:64, :], lhsT=embw[:], rhs=xc[:],
                                     start=True, stop=True)
                    nc.scalar.activation(out=f0TS[:, c * 512:(c + 1) * 512],
                                         in_=pe[:64, :], func=AF.Relu,
                                         bias=embb[:])
                    pt = psT.tile([P, 512], F16, tag="trans", name="trans")
                    for t4 in range(4):
                        sl = slice(c * 512 + t4 * 128, c * 512 + (t4 + 1) * 128)
                        nc.tensor.matmul(out=pt[:, t4 * 64:(t4 + 1) * 64],
                                         lhsT=f0TS[:, sl], rhs=identh[:64, :64],
                                         is_transpose=True, start=True,
                                         stop=True)
                    nc.scalar.activation(
                        out=stage[:, cc * 4:(cc + 1) * 4, 0:64],
                        in_=pt[:, 0:256].rearrange("p (j c) -> p j c", c=64),
                        func=AF.Copy)
                nc.sync.dma_start(
                    out=tab0_d.rearrange("(t p) c -> p t c", p=P)[:, sl8, :],
                    in_=stage[:])

        # ============ stage 2: SA1 ============
        with tc.tile_pool(name="sa1", bufs=2) as wk:
            nc.gpsimd.dma_gather(gq[:], tab0_d[:], i1sb[:], S1, S1, 128)
            nc.vector.tensor_tensor(out=sq1_pm[:], in0=gq[:, :, 67],
                                    in1=gq[:, :, 68], op=ALU.add)
            qxyz = cst.tile([3, S1], F16, tag="qxyz", name="qxyz")
            qsq = cst.tile([2, S1], F16, tag="qsq", name="qsq")
            for r in range(3):
                nc.sync.dma_start(
                    out=qxyz[r:r + 1, :].rearrange("r (t q) -> r t q", q=P),
                    in_=gq[:, :, 64 + r])
            for r in range(2):
                nc.sync.dma_start(
                    out=qsq[r:r + 1, :].rearrange("r (t q) -> r t q", q=P),
                    in_=gq[:, :, 67 + r])
            nc.scalar.activation(out=xyz1a[:], in_=qxyz[:], func=AF.Copy)
            nc.scalar.activation(out=sqn1[:], in_=qsq[:], func=AF.Copy,
                                 scale=-1.0)
            nc.scalar.activation(out=rhs2a[:], in_=qxyz[:], func=AF.Copy,
                                 scale=2.0)

            def sa1_select(qt):
                sl1 = slice(qt * P, (qt + 1) * P)
                lhsq = wk.tile([5, P], F16, tag="lhsq", name="lhsq")
                nc.scalar.activation(out=lhsq[0:3, :], in_=qxyz[:, sl1],
                                     func=AF.Copy, scale=2.0)
                nc.sync.dma_start(out=lhsq[3:5, :], in_=ones2_d[:])
                nqb = wk.tile([4, 512], F16, tag="nqb", name="nqb")
                nc.scalar.activation(
                    out=nqb[0:3, :].rearrange("r (j q) -> r j q", q=P),
                    in_=qxyz[:, sl1].unsqueeze(1).to_broadcast([3, 4, P]),
                    func=AF.Copy, scale=-1.0)
                nc.sync.dma_start(out=nqb[3:4, :], in_=ones1_d[:])

                candV = wk.tile([P, 128], F32, tag="candV", name="candV")
                candI = wk.tile([P, 128], U16, tag="candI", name="candI")
                for c in range(16):
                    pm = bigtile()
                    for hh in range(2):
                        nc.tensor.matmul(
                            out=pm[:, hh * 512:(hh + 1) * 512], lhsT=lhsq[:],
                            rhs=bigT[:, c * 1024 + hh * 512:
                                     c * 1024 + (hh + 1) * 512],
                            start=True, stop=True)
                    nc.vector.max(out=candV[:, c * 8:(c + 1) * 8], in_=pm[:])
                    nc.vector.max_index(out=candI[:, c * 8:(c + 1) * 8],
                                        in_max=candV[:, c * 8:(c + 1) * 8],
                                        in_values=pm[:])
                nc.vector.tensor_tensor(out=candI[:], in0=candI[:], in1=offs[:],
                                        op=ALU.add)
                candVw = wk.tile([P, 128], F32, tag="candVw", name="candVw")
                nc.vector.tensor_copy(out=candVw[:], in_=candV[:])
                selV = wk.tile([P, K1], F32, tag="selV", name="selV")
                for r in range(4):
                    rs = slice(r * 8, (r + 1) * 8)
                    nc.vector.max(out=selV[:, rs], in_=candVw[:])
                    if r < 3:
                        nc.vector.match_replace(out=candVw[:],
                                                in_to_replace=selV[:, rs],
                                                in_values=candVw[:],
                                                imm_value=NEG)
                mask = wk.tile([P, 128], U8, tag="selmask", name="selmask")
                nc.vector.tensor_scalar(out=mask[:], in0=candV[:],
                                        scalar1=selV[:, 31:32], scalar2=None,
                                        op0=ALU.is_ge)
                candIf = wk.tile([P, 128], F32, tag="candIf", name="candIf")
                nc.vector.tensor_copy(out=candIf[:], in_=candI[:])
                arr = wk.tile([P, 128], F32, tag="selarr", name="selarr")
                nc.vector.memset(arr[:], -1.0)
                nc.vector.copy_predicated(out=arr[:], mask=mask[:],
                                          data=candIf[:])
                selIf = wk.tile([P, K1], F32, tag="selIf", name="selIf")
                for r in range(4):
                    rs = slice(r * 8, (r + 1) * 8)
                    nc.vector.max(out=selIf[:, rs], in_=arr[:])
                    if r < 3:
                        nc.vector.match_replace(out=arr[:],
                                                in_to_replace=selIf[:, rs],
                                                in_values=arr[:],
                                                imm_value=-1.0)
                nbr16 = wk.tile([P, K1], I16, tag="nbr16", name="nbr16")
                nc.vector.tensor_copy(out=nbr16[:], in_=selIf[:])
                idxt = wrap_idx(wk, nbr16[:], ib1_ds[qt], "idxt1")
                gns = [wk.tile([P, 8, 128], F16, tag=f"gn{k}",
                               name=f"gn{k}", bufs=3) for k in range(4)]
                for k in range(4):
                    nc.gpsimd.dma_gather(gns[k][:], tab0_d[:],
                                         idxt[:, k * 64:(k + 1) * 64],
                                         1024, 1024, 128)
                return gns, nqb

            def sa1_mlp(qt, gns, nqb):
                sl1 = slice(qt * P, (qt + 1) * P)
                acc = wk.tile([P, P], F32, tag="sa1acc", name="sa1acc")
                for c in range(8):
                    gn = gns[c // 2]
                    pg = psT.tile([P, 512], F16, tag="trans", name="trans")
                    for j in range(4):
                        nc.tensor.matmul(out=pg[0:67, j * 128:(j + 1) * 128],
                                         lhsT=gn[:, (c % 2) * 4 + j, 0:67],
                                         rhs=identh[:], is_transpose=True,
                                         start=True, stop=True)
                    gtc = wk.tile([67, 512], F16, tag="gtc", name="gtc",
                                  bufs=3)
                    nc.scalar.activation(out=gtc[:], in_=pg[0:67, :],
                                         func=AF.Copy)
                    pz = mmtile()
                    nc.tensor.matmul(out=pz[:], lhsT=w1aug[:], rhs=gtc[:],
                                     start=True, stop=False)
                    nc.tensor.matmul(out=pz[:], lhsT=w1q[:], rhs=nqb[:, 0:512],
                                     start=False, stop=True)
                    h1 = wk.tile([P, 512], F16, tag="h1", name="h1", bufs=3)
                    nc.scalar.activation(out=h1[:], in_=pz[:], func=AF.Relu)
                    pz2 = mmtile()
                    nc.tensor.matmul(out=pz2[:], lhsT=w2sb[:], rhs=h1[:],
                                     start=True, stop=True)
                    red = wk.tile([P, P], F32, tag="sa1red", name="sa1red",
                                  bufs=3)
                    nc.vector.tensor_reduce(
                        out=red[:],
                        in_=pz2[:].rearrange("f (s q) -> f q s", q=P),
                        axis=AX.X, op=ALU.max)
                    if c == 0:
                        nc.vector.tensor_copy(out=acc[:], in_=red[:])
                    else:
                        nc.vector.tensor_tensor(out=acc[:], in0=acc[:],
                                                in1=red[:], op=ALU.max)
                nc.scalar.activation(out=feat1T[:, sl1], in_=acc[:],
                                     func=AF.Relu, bias=b2sb[:])

            pend = []
            for qt in range(T1 + 2):
                if qt < T1:
                    pend.append((qt, sa1_select(qt)))
                if qt >= 2:
                    j, args = pend.pop(0)
                    sa1_mlp(j, *args)

            stage1 = wk.tile([P, T1, 256], F16, tag="stage1", name="stage1",
                             bufs=1)
            for t in range(T1):
                pf = trans16(feat1T[:, t * P:(t + 1) * P])
                nc.scalar.activation(out=stage1[:, t, 0:128], in_=pf,
                                     func=AF.Copy)
            nc.vector.tensor_copy(out=stage1[:, :, 128:133],
                                  in_=gq[:, :, 64:69])
            nc.vector.memset(stage1[:, :, 133:256], 0.0)
            nc.sync.dma_start(out=tab1_d.rearrange("(t p) c -> p t c", p=P),
                              in_=stage1[:])

        # ============ stage 3: SA2 ============
        with tc.tile_pool(name="sa2", bufs=2) as wk:
            gq2 = wk.tile([P, T2, 256], F16, tag="gq2", name="gq2", bufs=1)
            nc.gpsimd.dma_gather(gq2[:], tab1_d[:], i2sb[:], S2, S2, 256)
            sq2_pm = wk.tile([P, T2], F32, tag="sq2_pm", name="sq2_pm", bufs=1)
            nc.vector.tensor_tensor(out=sq2_pm[:], in0=gq2[:, :, 131],
                                    in1=gq2[:, :, 132], op=ALU.add)
            q2xyz = wk.tile([3, S2], F16, tag="q2xyz", name="q2xyz", bufs=1)
            q2sq = wk.tile([2, S2], F16, tag="q2sq", name="q2sq", bufs=1)
            for r in range(3):
                nc.sync.dma_start(
                    out=q2xyz[r:r + 1, :].rearrange("r (t q) -> r t q", q=P),
                    in_=gq2[:, :, 128 + r])
            for r in range(2):
                nc.sync.dma_start(
                    out=q2sq[r:r + 1, :].rearrange("r (t q) -> r t q", q=P),
                    in_=gq2[:, :, 131 + r])
            nc.scalar.activation(out=rhsF2a[:], in_=q2xyz[:],
                                 func=AF.Copy, scale=2.0)
            nc.scalar.activation(out=rhsF2b[:], in_=q2sq[:],
                                 func=AF.Copy, scale=-1.0)

            feat2T = [cst.tile([P, S2], F16, tag=f"feat2T{h}",
                               name=f"feat2T{h}") for h in range(2)]

            def sa2_select(t2):
                sl2 = slice(t2 * P, (t2 + 1) * P)
                lhsqB = wk.tile([3, P], F16, tag="lhsqB", name="lhsqB")
                nc.scalar.activation(out=lhsqB[:], in_=q2xyz[:, sl2],
                                     func=AF.Copy, scale=2.0)
                nqb2 = wk.tile([4, 512], F16, tag="nqb2", name="nqb2")
                nc.scalar.activation(
                    out=nqb2[0:3, :].rearrange("r (j q) -> r j q", q=P),
                    in_=q2xyz[:, sl2].unsqueeze(1).to_broadcast([3, 4, P]),
                    func=AF.Copy, scale=-1.0)
                nc.sync.dma_start(out=nqb2[3:4, :], in_=ones1_d[:])

                pm2 = bigtile()
                for hh in range(2):
                    hs = slice(hh * 512, (hh + 1) * 512)
                    nc.tensor.matmul(out=pm2[:, hs], lhsT=lhsqB[:],
                                     rhs=xyz1a[:, hs],
                                     start=True, stop=False)
                    nc.tensor.matmul(out=pm2[:, hs], lhsT=ones2[:],
                                     rhs=sqn1[:, hs],
                                     start=False, stop=True)
                selV2 = wk.tile([P, K2], F32, tag="selV2", name="selV2")
                selI2 = wk.tile([P, K2], U16, tag="selI2", name="selI2")
                for r in range(4):
                    rs = slice(r * 8, (r + 1) * 8)
                    nc.vector.max(out=selV2[:, rs], in_=pm2[:])
                    nc.vector.max_index(out=selI2[:, rs], in_max=selV2[:, rs],
                                        in_values=pm2[:])
                    if r < 3:
                        nc.vector.match_replace(out=pm2[:],
                                                in_to_replace=selV2[:, rs],
                                                in_values=pm2[:],
                                                imm_value=NEG)
                nbr2 = wk.tile([P, K2], I16, tag="nbr2", name="nbr2")
                nc.vector.tensor_copy(out=nbr2[:], in_=selI2[:])
                idxt2 = wrap_idx(wk, nbr2[:], ib2_ds[t2], "idxt2")
                gn2s = [wk.tile([P, 8, 256], F16, tag=f"gn2{k}",
                                name=f"gn2{k}") for k in range(4)]
                for k in range(4):
                    nc.gpsimd.dma_gather(gn2s[k][:], tab1_d[:],
                                         idxt2[:, k * 64:(k + 1) * 64],
                                         1024, 1024, 256)
                return gn2s, nqb2

            def sa2_mlp(t2, gn2s, nqb2):
                sl2 = slice(t2 * P, (t2 + 1) * P)
                acc2 = [wk.tile([P, P], F32, tag=f"sa2acc{h}",
                                name=f"sa2acc{h}") for h in range(2)]
                for c in range(8):
                    gn2 = gn2s[c // 2]
                    pga = psT.tile([P, 512], F16, tag="trans", name="trans")
                    pgb = psT.tile([P, 512], F16, tag="trans", name="trans")
                    for j in range(4):
                        jj = (c % 2) * 4 + j
                        nc.tensor.matmul(out=pga[:, j * 128:(j + 1) * 128],
                                         lhsT=gn2[:, jj, 0:128],
                                         rhs=identh[:], is_transpose=True,
                                         start=True, stop=True)
                        nc.tensor.matmul(out=pgb[0:3, j * 128:(j + 1) * 128],
                                         lhsT=gn2[:, jj, 128:131],
                                         rhs=identh[:], is_transpose=True,
                                         start=True, stop=True)
                    gta = wk.tile([P, 512], F16, tag="gta", name="gta", bufs=3)
                    gtb = wk.tile([3, 512], F16, tag="gtb", name="gtb", bufs=3)
                    nc.scalar.activation(out=gta[:], in_=pga[:], func=AF.Copy)
                    nc.scalar.activation(out=gtb[:], in_=pgb[0:3, :],
                                         func=AF.Copy)
                    h1c = []
                    for h in range(2):
                        pz = mmtile()
                        nc.tensor.matmul(out=pz[:], lhsT=v1A[h][:], rhs=gta[:],
                                         start=True, stop=False)
                        nc.tensor.matmul(out=pz[:], lhsT=v1rel[h][:],
                                         rhs=gtb[:], start=False, stop=False)
                        nc.tensor.matmul(out=pz[:], lhsT=v1q[h][:],
                                         rhs=nqb2[:, 0:512],
                                         start=False, stop=True)
                        hh_ = wk.tile([P, 512], F16, tag=f"h1c{h}",
                                      name=f"h1c{h}", bufs=3)
                        nc.scalar.activation(out=hh_[:], in_=pz[:],
                                             func=AF.Relu)
                        h1c.append(hh_)
                    for h in range(2):
                        pz = mmtile()
                        nc.tensor.matmul(out=pz[:], lhsT=v2sb[0][h][:],
                                         rhs=h1c[0][:], start=True, stop=False)
                        nc.tensor.matmul(out=pz[:], lhsT=v2sb[1][h][:],
                                         rhs=h1c[1][:], start=False, stop=True)
                        red = wk.tile([P, P], F32, tag="sa2red",
                                      name="sa2red", bufs=3)
                        nc.vector.tensor_reduce(
                            out=red[:],
                            in_=pz[:].rearrange("f (s q) -> f q s", q=P),
                            axis=AX.X, op=ALU.max)
                        if c == 0:
                            nc.vector.tensor_copy(out=acc2[h][:], in_=red[:])
                        else:
                            nc.vector.tensor_tensor(out=acc2[h][:],
                                                    in0=acc2[h][:],
                                                    in1=red[:], op=ALU.max)
                for h in range(2):
                    nc.scalar.activation(out=feat2T[h][:, sl2],
                                         in_=acc2[h][:], func=AF.Relu,
                                         bias=c2sb[:, h:h + 1])

            s2p = [sa2_select(t2) for t2 in range(T2)]
            for t2 in range(T2):
                sa2_mlp(t2, *s2p[t2])

            stage2 = wk.tile([P, T2, 256], F16, tag="stage2", name="stage2",
                             bufs=1)
            for t2 in range(T2):
                for h in range(2):
                    pf = trans16(feat2T[h][:, t2 * P:(t2 + 1) * P])
                    nc.scalar.activation(
                        out=stage2[:, t2, h * 128:(h + 1) * 128], in_=pf,
                        func=AF.Copy)
            nc.sync.dma_start(out=tab2_d.rearrange("(t p) c -> p t c", p=P),
                              in_=stage2[:])

        # ============ stage 4: FP2 ============
        with tc.tile_pool(name="fp2", bufs=2) as wk:
            v8f = wk.tile([P, T1, 8], F32, tag="v8f", name="v8f", bufs=1)
            p8f = wk.tile([P, T1, 8], U16, tag="p8f", name="p8f", bufs=1)
            for qt in range(T1):
                pm3 = mmtile()
                nc.tensor.matmul(out=pm3[:, 0:S2],
                                 lhsT=xyz1a[:, qt * P:(qt + 1) * P],
                                 rhs=rhsF2a[:], start=True, stop=False)
                nc.tensor.matmul(out=pm3[:, 0:S2], lhsT=ones2[:],
                                 rhs=rhsF2b[:], start=False, stop=True)
                nc.vector.max(out=v8f[:, qt, :], in_=pm3[:, 0:S2])
                nc.vector.max_index(out=p8f[:, qt, :], in_max=v8f[:, qt, :],
                                    in_values=pm3[:, 0:S2])

            # inverse-distance weights
            d2f = wk.tile([P, T1, 3], F32, tag="d2f", name="d2f", bufs=1)
            nc.vector.tensor_tensor(
                out=d2f[:], in0=sq1_pm[:].unsqueeze(2).to_broadcast([P, T1, 3]),
                in1=v8f[:, :, 0:3], op=ALU.subtract)
            nc.scalar.activation(out=d2f[:], in_=d2f[:], func=AF.Relu)
            nc.scalar.activation(out=d2f[:], in_=d2f[:], func=AF.Sqrt)
            nc.vector.tensor_scalar_max(d2f[:], d2f[:], 1e-10)
            wn2 = wk.tile([P, T1, 3], F32, tag="wn2", name="wn2", bufs=1)
            nc.vector.reciprocal(out=wn2[:], in_=d2f[:])
            ws2 = wk.tile([P, T1], F32, tag="ws2", name="ws2", bufs=1)
            nc.vector.tensor_reduce(out=ws2[:], in_=wn2[:], axis=AX.X,
                                    op=ALU.add)
            nc.vector.reciprocal(out=ws2[:], in_=ws2[:])
            nc.vector.tensor_tensor(
                out=wn2[:], in0=wn2[:],
                in1=ws2[:].unsqueeze(2).to_broadcast([P, T1, 3]), op=ALU.mult)
            wn2h = wk.tile([P, T1, 3], F16, tag="wn2h", name="wn2h", bufs=1)
            nc.vector.tensor_copy(out=wn2h[:], in_=wn2[:])

            p3f = wk.tile([P, T1 * 3], I16, tag="p3f", name="p3f", bufs=1)
            nc.vector.tensor_copy(out=p3f[:].rearrange("p (t j) -> p t j", j=3),
                                  in_=p8f[:, :, 0:3])
            idxtf2 = wrap_idx(wk, p3f[:], ibf2_d, "idxtf2")
            gi2 = wk.tile([P, T1, 3, 256], F16, tag="gi2", name="gi2", bufs=1)
            gi2v = gi2[:].rearrange("p t j c -> p (t j) c")
            for k in range(3):
                nc.gpsimd.dma_gather(gi2v[:, k * 8:(k + 1) * 8, :], tab2_d[:],
                                     idxtf2[:, k * 64:(k + 1) * 64],
                                     1024, 1024, 256)
            # weighted 3-NN sum: t0*w0 + t1*w1 + t2*w2
            tmp0 = wk.tile([P, T1, 256], F16, tag="tmp0", name="tmp0", bufs=1)
            nc.vector.tensor_tensor(
                out=tmp0[:], in0=gi2[:, :, 0, :],
                in1=wn2h[:, :, 0:1].to_broadcast([P, T1, 256]), op=ALU.mult)
            tmp1 = wk.tile([P, T1, 256], F16, tag="tmp1", name="tmp1", bufs=1)
            nc.vector.tensor_tensor(
                out=tmp1[:], in0=gi2[:, :, 1, :],
                in1=wn2h[:, :, 1:2].to_broadcast([P, T1, 256]), op=ALU.mult)
            nc.vector.tensor_tensor(out=tmp0[:], in0=tmp0[:], in1=tmp1[:],
                                    op=ALU.add)
            nc.vector.tensor_tensor(
                out=tmp1[:], in0=gi2[:, :, 2, :],
                in1=wn2h[:, :, 2:3].to_broadcast([P, T1, 256]), op=ALU.mult)
            it2 = wk.tile([P, T1, 256], F16, tag="it2", name="it2", bufs=1)
            nc.vector.tensor_tensor(out=it2[:], in0=tmp0[:], in1=tmp1[:],
                                    op=ALU.add)
            itT2 = [wk.tile([P, S1], F16, tag=f"itT2{h}", name=f"itT2{h}",
                            bufs=1) for h in range(2)]
            for t in range(T1):
                for h in range(2):
                    pf = trans16(it2[:, t, h * 128:(h + 1) * 128])
                    nc.scalar.activation(out=itT2[h][:, t * P:(t + 1) * P],
                                         in_=pf, func=AF.Copy)
            for c in range(2):
                cs = slice(c * 512, (c + 1) * 512)
                pz = mmtile()
                nc.tensor.matmul(out=pz[:], lhsT=f2w1[0][:], rhs=feat1T[:, cs],
                                 start=True, stop=False)
                nc.tensor.matmul(out=pz[:], lhsT=f2w1[1][:], rhs=itT2[0][:, cs],
                                 start=False, stop=False)
                nc.tensor.matmul(out=pz[:], lhsT=f2w1[2][:], rhs=itT2[1][:, cs],
                                 start=False, stop=True)
                hf = wk.tile([P, 512], F16, tag="fp2h", name="fp2h", bufs=3)
                nc.scalar.activation(out=hf[:], in_=pz[:], func=AF.Relu,
                                     bias=f2b1[:])
                pz2 = mmtile()
                nc.tensor.matmul(out=pz2[:], lhsT=f2w2[:], rhs=hf[:],
                                 start=True, stop=True)
                nc.scalar.activation(out=f1upT[:, cs], in_=pz2[:], func=AF.Relu,
                                     bias=f2b2[:])
            stagef = wk.tile([P, T1, 128], F16, tag="stagef", name="stagef",
                             bufs=1)
            for t in range(T1):
                pf = trans16(f1upT[:, t * P:(t + 1) * P])
                nc.scalar.activation(out=stagef[:, t, :], in_=pf, func=AF.Copy)
            nc.sync.dma_start(out=tabf_d.rearrange("(t p) c -> p t c", p=P),
                              in_=stagef[:])

        # ============ stage 5: FP1 (16 pipelined groups) ============
        with tc.tile_pool(name="fp1", bufs=2) as wk:
            def fp1_select(g):
                v81 = wk.tile([P, GT, 8], F32, tag="v81", name="v81")
                p81 = wk.tile([P, GT, 8], U16, tag="p81", name="p81")
                for j in range(GT):
                    qt = g * GT + j
                    pm4 = bigtile()
                    for hh in range(2):
                        hs = slice(hh * 512, (hh + 1) * 512)
                        nc.tensor.matmul(out=pm4[:, hs],
                                         lhsT=bigT[0:3, qt * P:(qt + 1) * P],
                                         rhs=rhs2a[:, hs],
                                         start=True, stop=False)
                        nc.tensor.matmul(out=pm4[:, hs], lhsT=ones2[:],
                                         rhs=sqn1[:, hs],
                                         start=False, stop=True)
                    nc.vector.max(out=v81[:, j, :], in_=pm4[:])
                    nc.vector.max_index(out=p81[:, j, :], in_max=v81[:, j, :],
                                        in_values=pm4[:])

                sqs = sq_pm[:, g * GT:(g + 1) * GT]
                d21 = wk.tile([P, GT, 3], F32, tag="d21", name="d21")
                nc.vector.tensor_tensor(
                    out=d21[:], in0=sqs.unsqueeze(2).to_broadcast([P, GT, 3]),
                    in1=v81[:, :, 0:3], op=ALU.subtract)
                nc.scalar.activation(out=d21[:], in_=d21[:], func=AF.Relu)
                nc.scalar.activation(out=d21[:], in_=d21[:], func=AF.Sqrt)
                nc.vector.tensor_scalar_max(d21[:], d21[:], 1e-10)
                wn1 = wk.tile([P, GT, 3], F32, tag="wn1", name="wn1")
                nc.vector.reciprocal(out=wn1[:], in_=d21[:])
                ws1 = wk.tile([P, GT], F32, tag="ws1", name="ws1")
                nc.vector.tensor_reduce(out=ws1[:], in_=wn1[:], axis=AX.X,
                                        op=ALU.add)
                nc.vector.reciprocal(out=ws1[:], in_=ws1[:])
                nc.vector.tensor_tensor(
                    out=wn1[:], in0=wn1[:],
                    in1=ws1[:].unsqueeze(2).to_broadcast([P, GT, 3]),
                    op=ALU.mult)
                wn1h = wk.tile([P, GT, 3], F16, tag="wn1h", name="wn1h",
                               bufs=3)
                nc.vector.tensor_copy(out=wn1h[:], in_=wn1[:])

                p31 = wk.tile([P, GT * 3], I16, tag="p31", name="p31")
                nc.vector.tensor_copy(
                    out=p31[:].rearrange("p (t j) -> p t j", j=3),
                    in_=p81[:, :, 0:3])
                idxtf1 = wrap_idx(wk, p31[:], ibf1_ds[g], "idxtf1")
                gi1s = [wk.tile([P, 8, 128], F16, tag=f"gi1{k}",
                                name=f"gi1{k}", bufs=3) for k in range(3)]
                for k in range(3):
                    nc.gpsimd.dma_gather(gi1s[k][:], tabf_d[:],
                                         idxtf1[:, k * 64:(k + 1) * 64],
                                         1024, 1024, 128)
                return gi1s, wn1h

            def fp1_interp(g, gi1s, wn1h):
                # The gathered slot order is (t j): slot m = t*3 + j; gather k
                # holds slots k*8..k*8+7.  View the three tiles as one
                # (t, j) sequence via per-tile strided slices.
                gcat = [gi1s[m * 8 // 8] for m in range(3)]
                ta = wk.tile([P, GT, 128], F16, tag="ta", name="ta")
                tb = wk.tile([P, GT, 128], F16, tag="tb", name="tb")
                it1 = wk.tile([P, GT, 128], F16, tag="it1", name="it1",
                              bufs=3)
                first = True
                for k in range(3):
                    # neighbor k of tile t lives at slot m = t*3 + k,
                    # i.e. gather tile m//8, row m%8
                    for seg in range(3):
                        # contiguous runs of slots with the same gather tile
                        pass
                    # build via per-(t) slices grouped by gather tile
                    dst = tb if not first else ta
                    for gt_i in range(3):
                        lo = gt_i * 8
                        ms = [t * 3 + k for t in range(GT)
                              if lo <= t * 3 + k < lo + 8]
                        if not ms:
                            continue
                        t0 = ms[0] // 3
                        t1_ = ms[-1] // 3
                        nc.vector.tensor_tensor(
                            out=dst[:, t0:t1_ + 1, :],
                            in0=gi1s[gt_i][:].rearrange(
                                "p m c -> p m c")[:, ms[0] - lo:
                                                  ms[-1] - lo + 1:3, :],
                            in1=wn1h[:, t0:t1_ + 1, k:k + 1].to_broadcast(
                                [P, t1_ + 1 - t0, 128]),
                            op=ALU.mult)
                    if first:
                        first = False
                    else:
                        nc.vector.tensor_tensor(out=ta[:], in0=ta[:],
                                                in1=tb[:], op=ALU.add)
                nc.vector.tensor_copy(out=it1[:], in_=ta[:])
                return it1

            def fp1_mlp(g, it1):
                itT1 = wk.tile([P, GT * 128], F16, tag="itT1", name="itT1")
                for t in range(GT):
                    pf = trans16(it1[:, t, :])
                    nc.scalar.activation(out=itT1[:, t * P:(t + 1) * P],
                                         in_=pf, func=AF.Copy)
                for c in range(2):
                    cs = slice(c * 512, (c + 1) * 512)
                    gcs = slice(g * GT * P + c * 512,
                                g * GT * P + (c + 1) * 512)
                    pz = mmtile()
                    nc.tensor.matmul(out=pz[:], lhsT=f1w1b[:], rhs=itT1[:, cs],
                                     start=True, stop=False)
                    nc.tensor.matmul(out=pz[:], lhsT=f1w1a[:],
                                     rhs=f0TS[:, gcs], start=False, stop=True)
                    hf = wk.tile([P, 512], F16, tag="fp1h", name="fp1h",
                                 bufs=3)
                    nc.scalar.activation(out=hf[:], in_=pz[:], func=AF.Relu,
                                         bias=f1b1[:])
                    pz2 = mmtile()
                    nc.tensor.matmul(out=pz2[:], lhsT=f1w2[:], rhs=hf[:],
                                     start=True, stop=True)
                    nc.scalar.activation(out=fuTS[:, gcs], in_=pz2[:],
                                         func=AF.Relu, bias=f1b2[:])
                    red = wk.tile([P, 1], F32, tag="gfred", name="gfred",
                                  bufs=3)
                    nc.vector.tensor_reduce(out=red[:], in_=fuTS[:, gcs],
                                            axis=AX.X, op=ALU.max)
                    if g == 0 and c == 0:
                        nc.vector.tensor_copy(out=gfacc[:], in_=red[:])
                    else:
                        nc.vector.tensor_tensor(out=gfacc[:], in0=gfacc[:],
                                                in1=red[:], op=ALU.max)

            fpend = []
            for g in range(NG + 2):
                it1 = None
                if g >= 2:
                    j, args = fpend.pop(0)
                    it1 = fp1_interp(j, *args)
                if g < NG:
                    fpend.append((g, fp1_select(g)))
                if it1 is not None:
                    fp1_mlp(j, it1)

        # ============ stage 6: head ============
        with tc.tile_pool(name="head", bufs=2) as wk:
            gfh = wk.tile([P, 1], F16, tag="gfh", name="gfh", bufs=1)
            nc.vector.tensor_copy(out=gfh[:], in_=gfacc[:])
            pc = mmtile()
            nc.tensor.matmul(out=pc[:, 0:1], lhsT=hw1b[:], rhs=gfh[:],
                             start=True, stop=True)
            nc.vector.tensor_tensor(out=biasH[:], in0=pc[:, 0:1], in1=hb1[:],
                                    op=ALU.add)
            for g in range(8):
                ostage = wk.tile([P, 16, 13], F32, tag="ostage", name="ostage")
                for c4 in range(4):
                    c = g * 4 + c4
                    cs = slice(c * 512, (c + 1) * 512)
                    pz = mmtile()
                    nc.tensor.matmul(out=pz[:], lhsT=hw1a[:], rhs=fuTS[:, cs],
                                     start=True, stop=True)
                    h1 = wk.tile([P, 512], F16, tag="hh1", name="hh1", bufs=3)
                    nc.scalar.activation(out=h1[:], in_=pz[:], func=AF.Relu,
                                         bias=biasH[:])
                    pz2 = bigtile()
                    nc.tensor.matmul(out=pz2[:64, 0:512], lhsT=hw2[:],
                                     rhs=h1[:], start=True, stop=True)
                    h2 = wk.tile([64, 512], F16, tag="hh2", name="hh2", bufs=3)
                    nc.scalar.activation(out=h2[:], in_=pz2[:64, 0:512],
                                         func=AF.Relu, bias=hb2[:])
                    pz3 = bigtile()
                    nc.tensor.matmul(out=pz3[:13, 0:512], lhsT=hw3[:],
                                     rhs=h2[:], start=True, stop=True)
                    oT = wk.tile([13, 512], F16, tag="hoT", name="hoT", bufs=3)
                    nc.vector.tensor_tensor(
                        out=oT[:], in0=pz3[:13, 0:512],
                        in1=hb3[:, 0:1].to_broadcast([13, 512]), op=ALU.add)
                    po = psT.tile([P, 512], F16, tag="trans", name="trans")
                    for t in range(4):
                        nc.tensor.matmul(out=po[:, t * 16:t * 16 + 13],
                                         lhsT=oT[:, t * 128:(t + 1) * 128],
                                         rhs=identh[0:13, 0:13],
                                         is_transpose=True, start=True,
                                         stop=True)
                    nc.scalar.activation(
                        out=ostage[:, c4 * 4:(c4 + 1) * 4, :],
                        in_=po[:, 0:64].rearrange(
                            "p (t c) -> p t c", c=16)[:, :, 0:13],
                        func=AF.Copy)
                nc.sync.dma_start(
                    out=out_d.rearrange("(t p) c -> p t c", p=P)[
                        :, g * 16:(g + 1) * 16, :],
                    in_=ostage[:])

    return nc


# ---------------------------------------------------------------- host side
_CACHED_NC = None


def _get_nc():
    global _CACHED_NC
    if _CACHED_NC is None:
        nc = build_nc()
        nc.finalize()
        _CACHED_NC = nc
    return _CACHED_NC


def _per_core_inputs(b, inputs):
    x = np.asarray(inputs["x"][b]).astype(np.float16)
    i1 = np.asarray(inputs["idx_s1"][b]).astype(np.int16)
    i1w = np.tile(i1.reshape(S1 // 16, 16).T, (8, 1))
    i2 = np.asarray(inputs["idx_s2"][b]).astype(np.int16)
    i2w = np.tile(i2.reshape(S2 // 16, 16).T, (8, 1))
    f16 = lambda a: np.ascontiguousarray(np.asarray(a), dtype=np.float16)
    f32 = lambda a: np.ascontiguousarray(np.asarray(a), dtype=np.float32)
    return {
        "xh": np.ascontiguousarray(x),
        "xTh": np.ascontiguousarray(x.T),
        "i1w": np.ascontiguousarray(i1w),
        "i2w": np.ascontiguousarray(i2w),
        "embw": f16(inputs["embed_w"]),
        "embb": f32(inputs["embed_b"]).reshape(64, 1),
        "w1": f16(inputs["sa1_w1"]),
        "b1r": f16(inputs["sa1_b1"]).reshape(1, 128),
        "w2": f16(inputs["sa1_w2"]),
        "b2": f32(inputs["sa1_b2"]).reshape(128, 1),
        "v1": f16(inputs["sa2_w1"]),
        "c1r": f16(inputs["sa2_b1"]).reshape(1, 256),
        "v2": f16(inputs["sa2_w2"]),
        "c2": np.ascontiguousarray(f32(inputs["sa2_b2"]).reshape(2, 128).T),
        "f2w1": f16(inputs["fp2_w1"]),
        "f2b1": f32(inputs["fp2_b1"]).reshape(128, 1),
        "f2w2": f16(inputs["fp2_w2"]),
        "f2b2": f32(inputs["fp2_b2"]).reshape(128, 1),
        "f1w1": f16(inputs["fp1_w1"]),
        "f1b1": f32(inputs["fp1_b1"]).reshape(128, 1),
        "f1w2": f16(inputs["fp1_w2"]),
        "f1b2": f32(inputs["fp1_b2"]).reshape(128, 1),
        "hw1": f16(inputs["head_w1"]),
        "hb1": f32(inputs["head_b1"]).reshape(128, 1),
        "hw2": f16(inputs["head_w2"]),
        "hb2": f32(inputs["head_b2"]).reshape(64, 1),
        "hw3": f16(inputs["head_w3"]),
        "hb3": f32(inputs["head_b3"]).reshape(13, 1),
    }


def run(inputs, trace=False, **kw):
    nc = _get_nc()
    B = inputs["x"].shape[0]
    in_maps = [_per_core_inputs(b, inputs) for b in range(B)]
    res = run_bass_kernel_spmd(nc, in_maps, core_ids=list(range(B)),
                               trace=trace, **kw)
    out = np.stack([res.results[b]["out"] for b in range(B)])
    return out, res


def kernel(**inputs):
    return run(inputs)[0]


if __name__ == "__main__":
    build_nc()
    print("built ok")
